# revision 1
# baseline (speedup 1.0000x reference)
"""TRN2 Bass kernel: 8-layer Chambolle-Pock MRI reconstruction on 8 NeuronCores.

Sharding: frames (8/core) for FFTs + elementwise; k-space rows (8192 px/core)
for the low-rank prox (Gram trick + matmul-only spectral filter, no eigensolver).
Cross-core per layer: AllToAll (y1 frame->pixel), AllToAll (x1k V pixel->frame),
AllReduce (64x64 Gram), 2 small AllGathers (temporal halo). Mostly bf16 storage
with fp32 PSUM accumulation; d-hat kept fp32.
"""
import numpy as np

import concourse.bass as bass
import concourse.bacc as bacc
import concourse.mybir as mybir
import concourse.tile as tile
from concourse import bass_utils

F32 = mybir.dt.float32
BF16 = mybir.dt.bfloat16
U32 = mybir.dt.uint32
AF = mybir.ActivationFunctionType
ALU = mybir.AluOpType

NCORE = 8
NF = 64
NFL = 8
N = 256
NLAYERS = 8
SIGMA = float(np.float32(1.0 / np.sqrt(8.0)))
TAU = SIGMA
C1 = float(np.float32(1.0 / (1.0 + SIGMA)))
C2 = C1 * SIGMA
M_POWER = 8
SIGN_COEFFS = [(3.4445, -4.7750, 2.0315)] * 7 + [(1.875, -1.25, 0.375)] * 3
RG = [list(range(NCORE))]


def _fit_q(deg=14, lo=0.068, hi=1.05, npts=6000):
    xs_in = np.linspace(lo, hi, npts)
    xs_out = np.linspace(0, 0.05, 300)

    def cheb(x, d, b=hi):
        t = (2 * x - b) / b
        V = np.zeros((len(x), d + 1))
        V[:, 0] = 1
        if d >= 1:
            V[:, 1] = t
        for k in range(2, d + 1):
            V[:, k] = 2 * t * V[:, k - 1] - V[:, k - 2]
        return V

    Vi = cheb(xs_in, deg)
    Vo = cheb(xs_out, deg)
    A = np.vstack([np.sqrt(xs_in)[:, None] * Vi, 1e-5 * Vo])
    y = np.concatenate([np.ones(npts), np.zeros(len(xs_out))])
    coef, *_ = np.linalg.lstsq(A, y, rcond=None)
    return [float(c) for c in coef], hi


Q_COEF, Q_HI = _fit_q()


def _dft():
    k = np.arange(N)
    W = np.exp(-2j * np.pi * np.outer(k, k) / N)
    return W.real.astype(np.float32), W.imag.astype(np.float32)


def _chunk(a):
    return np.ascontiguousarray(np.stack([a[0:128], a[128:256]], axis=1))


def build(n_layers=NLAYERS, single_core=False):
    nc = bacc.Bacc("TRN2", target_bir_lowering=False, debug=False,
                   num_devices=1 if single_core else NCORE)

    dk_r = nc.dram_tensor("dk_r", [128, 2, NFL, N], F32, kind="ExternalInput")
    dk_i = nc.dram_tensor("dk_i", [128, 2, NFL, N], F32, kind="ExternalInput")
    dp_r = nc.dram_tensor("dp_r", [128, 4096], F32, kind="ExternalInput")
    dp_i = nc.dram_tensor("dp_i", [128, 4096], F32, kind="ExternalInput")
    lamS_in = nc.dram_tensor("lamS", [1, NLAYERS], F32, kind="ExternalInput")
    lamL_in = nc.dram_tensor("lamL", [1, NLAYERS], F32, kind="ExternalInput")
    cmeta = nc.dram_tensor("cmeta", [1, 4], F32, kind="ExternalInput")
    cmeta_u = nc.dram_tensor("cmeta_u", [1, 2], U32, kind="ExternalInput")
    out_r = nc.dram_tensor("out_r", [128, 2, NFL, N], F32, kind="ExternalOutput")
    out_i = nc.dram_tensor("out_i", [128, 2, NFL, N], F32, kind="ExternalOutput")

    Fr, Fi = _dft()
    sA = C1 * SIGMA / N
    sI = -TAU / N
    id128 = np.eye(128, dtype=np.float32)
    consts_np = {
        "SA0": (_chunk(np.concatenate([Fr * sA, Fi * sA], 1)), BF16),
        "SA1": (_chunk(np.concatenate([-Fi * sA, Fr * sA], 1)), BF16),
        "SI0": (_chunk(np.concatenate([Fr * sI, -Fi * sI], 1)), BF16),
        "SI1": (_chunk(np.concatenate([Fi * sI, Fr * sI], 1)), BF16),
        "Br": (_chunk(Fr), F32),
        "Bi": (_chunk(Fi), F32),
        "Bn": (_chunk(-Fi), F32),
        "I128b": (id128, BF16),
        "Ic1": (id128 * C1, BF16),
        "Insig": (id128 * (-C1 * SIGMA), F32),
        "I1b": (id128, BF16),
        "ITb": (id128 * TAU, BF16),
        "ITnb": (id128 * (-TAU), BF16),
        "I64": (np.eye(64, dtype=np.float32), F32),
    }
    handles = {}
    for k, (v, dt) in consts_np.items():
        handles[k] = nc.inline_tensor(v.astype(mybir.dt.np(dt)), name="c" + k)

    dh_r = nc.dram_tensor("dh_r", [128, 2, NFL, N], F32)
    dh_i = nc.dram_tensor("dh_i", [128, 2, NFL, N], F32)
    b1_in = nc.dram_tensor("b1_in", [NCORE, 2, NFL, 32, N], BF16)
    b1_out = nc.dram_tensor("b1_out", [NCORE, 2, NFL, 32, N], BF16)
    b2_in = nc.dram_tensor("b2_in", [NCORE, 2, NFL, 2, 16, N], BF16)
    b2_out = nc.dram_tensor("b2_out", [NCORE, 2, NFL, 2, 16, N], BF16)
    g_in = nc.dram_tensor("g_in", [64, 128], F32)
    g_out = nc.dram_tensor("g_out", [64, 128], F32, addr_space="Shared")
    nrm_in = nc.dram_tensor("nrm_in", [1, 1], F32)
    nrm_out = nc.dram_tensor("nrm_out", [1, 1], F32, addr_space="Shared")
    hu_in = nc.dram_tensor("hu_in", [1, 2, 128, 2, N], BF16)
    hu_out = nc.dram_tensor("hu_out", [NCORE, 2, 128, 2, N], BF16, addr_space="Shared")
    hy_in = nc.dram_tensor("hy_in", [1, 2, 128, 2, N], BF16)
    hy_out = nc.dram_tensor("hy_out", [NCORE, 2, 128, 2, N], BF16, addr_space="Shared")

    with tile.TileContext(nc) as tc:
        import contextlib
        stack = contextlib.ExitStack()

        def pool(name, bufs, space="SBUF"):
            return stack.enter_context(tc.tile_pool(name=name, bufs=bufs, space=space))

        cp = pool("cp", 1)
        sbp = pool("sbp", 1)
        stp = pool("stp", 1)
        rp = pool("rp", 2)         # small rotating pieces
        bigp = pool("bigp", 1)
        tkp = pool("tkp", 2)
        wp = pool("wp", 1)
        psp = pool("psp", 4, space="PSUM")
        psm = pool("psm", 1, space="PSUM")
        psw = pool("psw", 3, space="PSUM")

        def coll(kind, op, replica_groups, ins, outs):
            if single_core:
                nc.sync.dma_start(out=outs[0], in_=ins[0]) if ins[0].size() == outs[0].size() \
                    else nc.sync.dma_start(out=outs[0][0:1], in_=ins[0][0:1])
            else:
                nc.gpsimd.collective_compute(kind, op, replica_groups=replica_groups,
                                             ins=ins, outs=outs)

        C = {}
        for k, (v, dt) in consts_np.items():
            t = cp.tile(list(v.shape), dt, name="k" + k, tag="k" + k)
            if v.ndim == 2:
                nc.sync.dma_start(out=t[:, :], in_=handles[k][:, :])
            else:
                nc.sync.dma_start(out=t[:, :, :], in_=handles[k][:, :, :])
            C[k] = t

        y1r = sbp.tile([128, 2, NFL, N], BF16, name="y1r", tag="y1r")
        y1i = sbp.tile([128, 2, NFL, N], BF16, name="y1i", tag="y1i")
        u2r = sbp.tile([128, 2, NFL, N], BF16, name="u2r", tag="u2r")
        u2i = sbp.tile([128, 2, NFL, N], BF16, name="u2i", tag="u2i")
        y2r = sbp.tile([128, 2, NFL, N], BF16, name="y2r", tag="y2r")
        y2i = sbp.tile([128, 2, NFL, N], BF16, name="y2i", tag="y2i")
        x2r = sbp.tile([128, 2, NFL, N], BF16, name="x2r", tag="x2r")
        x2i = sbp.tile([128, 2, NFL, N], BF16, name="x2i", tag="x2i")
        xpr = sbp.tile([128, 4096], BF16, name="xpr", tag="xpr")
        xpi = sbp.tile([128, 4096], BF16, name="xpi", tag="xpi")
        y1p_r = sbp.tile([128, 4096], BF16, name="y1p_r", tag="y1p_r")
        y1p_i = sbp.tile([128, 4096], BF16, name="y1p_i", tag="y1p_i")
        hu_r = sbp.tile([128, 2, N], BF16, name="hu_r", tag="hu_r")
        hu_i = sbp.tile([128, 2, N], BF16, name="hu_i", tag="hu_i")
        hy_r = sbp.tile([128, 2, N], BF16, name="hy_r", tag="hy_r")
        hy_i = sbp.tile([128, 2, N], BF16, name="hy_i", tag="hy_i")
        IT7 = sbp.tile([128, 128], BF16, name="IT7", tag="IT7")
        ITn0 = sbp.tile([128, 128], BF16, name="ITn0", tag="ITn0")
        sc = sbp.tile([128, 8], F32, name="sc", tag="sc")

        def bc(col):
            return sc[:, col:col + 1]

        # ---------------- init ----------------
        dsr = bigp.tile([128, 2, NFL, N], F32, name="dsr", tag="bigs", bufs=2)
        dsi = bigp.tile([128, 2, NFL, N], F32, name="dsi", tag="bigs", bufs=2)
        nc.sync.dma_start(out=dsr[:, :, :, :], in_=dk_r[:, :, :, :])
        nc.sync.dma_start(out=dsi[:, :, :, :], in_=dk_i[:, :, :, :])
        acc = stp.tile([128, 2], F32, name="acc", tag="acc")
        sq = bigp.tile([128, 2048], F32, name="sq", tag="big", bufs=1)
        # accumulate |d|^2 per partition in 4 half-passes to keep scrap small
        nc.scalar.activation(sq[:, :], dsr[:, :, 0:4, :], AF.Square, accum_out=acc[:, 0:1])
        sq_b = bigp.tile([128, 2048], F32, name="sq_b", tag="big", bufs=1)
        nc.scalar.activation(sq_b[:, :], dsr[:, :, 4:8, :], AF.Square, accum_out=acc[:, 1:2])
        acs = stp.tile([128, 2], F32, name="acs", tag="acs")
        nc.vector.tensor_add(acs[:, 0:1], acc[:, 0:1], acc[:, 1:2])
        sq_c = bigp.tile([128, 2048], F32, name="sq_c", tag="big", bufs=1)
        nc.scalar.activation(sq_c[:, :], dsi[:, :, 0:4, :], AF.Square, accum_out=acc[:, 0:1])
        sq_d = bigp.tile([128, 2048], F32, name="sq_d", tag="big", bufs=1)
        nc.scalar.activation(sq_d[:, :], dsi[:, :, 4:8, :], AF.Square, accum_out=acc[:, 1:2])
        nc.vector.tensor_add(acs[:, 1:2], acc[:, 0:1], acc[:, 1:2])
        nc.vector.tensor_add(acs[:, 0:1], acs[:, 0:1], acs[:, 1:2])
        on1 = stp.tile([128, 1], F32, name="on1", tag="on1")
        nc.vector.memset(on1[:, :], 1.0)
        onr = stp.tile([1, 128], F32, name="onr", tag="onr")
        nc.vector.memset(onr[0:1, :], 1.0)
        on64 = stp.tile([64, 1], F32, name="on64", tag="on64")
        nc.vector.memset(on64[:, :], 1.0)
        on64r = stp.tile([1, 64], F32, name="on64r", tag="on64r")
        nc.vector.memset(on64r[0:1, :], 1.0)

        ps0 = psw.tile([128, 8], F32, name="ps0", tag="pw")
        nc.tensor.matmul(ps0[0:1, 0:1], acs[:, 0:1], on1[:, :], start=True, stop=True)
        nrm_st = stp.tile([1, 1], F32, name="nrm_st", tag="nrm")
        nc.vector.tensor_copy(nrm_st[0:1, :], ps0[0:1, 0:1])
        nc.sync.dma_start(out=nrm_in[:, :], in_=nrm_st[0:1, :])
        coll("AllReduce", ALU.add, RG, [nrm_in.ap()], [nrm_out.ap()])
        nc.sync.dma_start(out=nrm_st[0:1, :], in_=nrm_out[:, :])
        rec = stp.tile([1, 1], F32, name="rec", tag="rec")
        nc.vector.reciprocal(rec[0:1, :], nrm_st[0:1, :])
        nc.scalar.activation(rec[0:1, :], rec[0:1, :], AF.Sqrt)
        ps1 = psw.tile([128, 8], F32, name="ps1", tag="pw")
        nc.tensor.matmul(ps1[:, 0:1], onr[0:1, :], rec[0:1, :], start=True, stop=True)
        nc.vector.tensor_copy(bc(0), ps1[:, 0:1])

        nc.vector.tensor_scalar_mul(dsr[:, :, :, :], dsr[:, :, :, :], bc(0))
        nc.vector.tensor_scalar_mul(dsi[:, :, :, :], dsi[:, :, :, :], bc(0))
        nc.sync.dma_start(out=dh_r[:, :, :, :], in_=dsr[:, :, :, :])
        nc.sync.dma_start(out=dh_i[:, :, :, :], in_=dsi[:, :, :, :])
        # b2_out prefill: V0 = c2 * d-hat (bf16) via piece staging
        for p, src in ((0, dsr), (1, dsi)):
            for s in range(NCORE):
                m, q = s // 4, s % 4
                vp = bigp.tile([32, 2, NFL, N], BF16, name=f"v0_{p}_{s}", tag="big", bufs=1)
                nc.vector.tensor_scalar_mul(vp[:, :, :, :],
                                            src[32 * q:32 * (q + 1), :, :, :], C2)
                nc.sync.dma_start(
                    out=b2_out[s, p, :, :, :, :].rearrange("f b k y -> (b k) f y"),
                    in_=vp[:, m, :, :])
        nc.gpsimd.dma_start(out=y1p_r[:, :], in_=dp_r[:, :])
        nc.gpsimd.dma_start(out=y1p_i[:, :], in_=dp_i[:, :])
        nc.vector.tensor_scalar_mul(xpr[:, :], y1p_r[:, :], bc(0))
        nc.vector.tensor_scalar_mul(xpi[:, :], y1p_i[:, :], bc(0))

        cm = stp.tile([1, 4], F32, name="cm", tag="cm")
        nc.sync.dma_start(out=cm[0:1, :], in_=cmeta[:, :])
        cmu = stp.tile([1, 2], U32, name="cmu", tag="cmu")
        nc.sync.dma_start(out=cmu[0:1, :], in_=cmeta_u[:, :])
        ps2 = psw.tile([128, 8], F32, name="ps2", tag="pw")
        nc.tensor.matmul(ps2[:, 0:2], onr[0:1, :], cm[0:1, 2:4], start=True, stop=True)
        nc.vector.tensor_copy(bc(1), ps2[:, 0:1])
        nc.vector.tensor_copy(bc(2), ps2[:, 1:2])
        nc.vector.tensor_scalar_mul(bc(3), ps2[:, 1:2], SIGMA)
        nc.vector.tensor_scalar_mul(IT7[:, :], C["ITb"][:, :], bc(2))
        nc.vector.tensor_scalar_mul(ITn0[:, :], C["ITnb"][:, :], bc(1))

        lam_t = stp.tile([1, 2 * NLAYERS], F32, name="lam_t", tag="lam")
        nc.sync.dma_start(out=lam_t[0:1, 0:NLAYERS], in_=lamS_in[:, :])
        nc.sync.dma_start(out=lam_t[0:1, NLAYERS:2 * NLAYERS], in_=lamL_in[:, :])
        nc.vector.tensor_relu(lam_t[0:1, :], lam_t[0:1, :])

        def fwd_stageA(dst, pr_t, pi_t, s0, s1, li):
            for f in range(NFL):
                for sl in range(2):
                    ps = psp.tile([128, 512], F32, name=f"pA{li}_{f}_{sl}", tag="ps")
                    nc.tensor.matmul(ps[:, :], pr_t[:, 0, f, sl * 128:(sl + 1) * 128],
                                     s0[:, 0, :], start=True, stop=False)
                    nc.tensor.matmul(ps[:, :], pi_t[:, 0, f, sl * 128:(sl + 1) * 128],
                                     s1[:, 0, :], start=False, stop=False)
                    nc.tensor.matmul(ps[:, :], pr_t[:, 1, f, sl * 128:(sl + 1) * 128],
                                     s0[:, 1, :], start=False, stop=False)
                    nc.tensor.matmul(ps[:, :], pi_t[:, 1, f, sl * 128:(sl + 1) * 128],
                                     s1[:, 1, :], start=False, stop=True)
                    nc.scalar.copy(out=dst[:, sl, f, :], in_=ps[:, :])

        def stageB(ps_r, ps_i, Yt, m, fp, inv, stop=False):
            for c in range(2):
                wr = C["Br"][:, c, m * 128:(m + 1) * 128]
                wi = C["Bi"][:, c, m * 128:(m + 1) * 128]
                wn = C["Bn"][:, c, m * 128:(m + 1) * 128]
                rr = Yt[:, c, 2 * fp:2 * fp + 2, 0:256]
                ri = Yt[:, c, 2 * fp:2 * fp + 2, 256:512]
                la = (c == 1) and stop
                if not inv:
                    nc.tensor.matmul(ps_r, wr, rr, start=(c == 0), stop=False)
                    nc.tensor.matmul(ps_r, wn, ri, start=False, stop=la)
                    nc.tensor.matmul(ps_i, wi, rr, start=(c == 0), stop=False)
                    nc.tensor.matmul(ps_i, wr, ri, start=False, stop=la)
                else:
                    nc.tensor.matmul(ps_r, wr, rr, start=(c == 0), stop=False)
                    nc.tensor.matmul(ps_r, wi, ri, start=False, stop=la)
                    nc.tensor.matmul(ps_i, wn, rr, start=(c == 0), stop=False)
                    nc.tensor.matmul(ps_i, wr, ri, start=False, stop=la)

        # startpoint: x2 = u2 = ifft2(d-hat) via bf16 copy of d-hat
        # use u2 tiles as scratch bf16 copies of d-hat for the init ifft
        nc.vector.tensor_copy(u2r[:, :, :, :], dsr[:, :, :, :])
        nc.vector.tensor_copy(u2i[:, :, :, :], dsi[:, :, :, :])
        Qt0 = bigp.tile([128, 2, NFL, 512], F32, name="Qt0", tag="big", bufs=1)
        fwd_stageA(Qt0, u2r, u2i, C["SI0"], C["SI1"], "ini")
        for m in range(2):
            for fp in range(4):
                pr = psp.tile([128, 512], F32, name=f"pi0r{m}{fp}", tag="ps")
                pi = psp.tile([128, 512], F32, name=f"pi0i{m}{fp}", tag="ps")
                stageB(pr[:, :], pi[:, :], Qt0, m, fp, inv=True, stop=True)
                sl2 = (slice(None), m, slice(2 * fp, 2 * fp + 2), slice(None))
                nc.scalar.activation(x2r[sl2], pr[:, :], AF.Copy, scale=-1.0 / TAU)
                nc.scalar.activation(x2i[sl2], pi[:, :], AF.Copy, scale=-1.0 / TAU)
                nc.vector.tensor_scalar_mul(u2r[sl2], pr[:, :], -1.0 / TAU)
                nc.vector.tensor_scalar_mul(u2i[sl2], pi[:, :], -1.0 / TAU)
        for t in (y1r, y1i, y2r, y2i):
            nc.vector.memset(t[:, :, :, :], 0.0)

        eng = nc.sync
        r_up = eng.alloc_register("r_up")
        eng.reg_load(r_up, cmu[0:1, 0:1])
        idx_up = eng.snap(r_up, donate=True, min_val=0, max_val=NCORE - 1)
        r_dn = eng.alloc_register("r_dn")
        eng.reg_load(r_dn, cmu[0:1, 1:2])
        idx_dn = eng.snap(r_dn, donate=True, min_val=0, max_val=NCORE - 1)

        def push_u2_halo():
            nc.sync.dma_start(out=hu_in[0, 0, :, :, :], in_=u2r[:, :, 0, :])
            nc.sync.dma_start(out=hu_in[0, 1, :, :, :], in_=u2i[:, :, 0, :])
            coll("AllGather", ALU.bypass, RG, [hu_in.ap()], [hu_out.ap()])
            nc.sync.dma_start(
                out=hu_r[:, :, :],
                in_=hu_out[bass.ds(idx_up, 1), 0, :, :, :].rearrange("o p c y -> (o p) c y"))
            nc.sync.dma_start(
                out=hu_i[:, :, :],
                in_=hu_out[bass.ds(idx_up, 1), 1, :, :, :].rearrange("o p c y -> (o p) c y"))

        push_u2_halo()

        def cmm(dst, A, B, nm):
            ps = psw.tile([64, 128], F32, name="cm" + nm, tag="pw")
            nc.tensor.matmul(ps[:, 0:64], A[:, 0, :], B[:, 0, :], start=True, stop=False)
            nc.tensor.matmul(ps[:, 0:64], A[:, 1, :], B[:, 1, :], start=False, stop=True)
            nc.tensor.matmul(ps[:, 64:128], A[:, 0, :], B[:, 1, :], start=True, stop=False)
            nc.tensor.matmul(ps[:, 64:128], A[:, 2, :], B[:, 0, :], start=False, stop=True)
            nc.vector.tensor_copy(dst[:, 0:2, :], ps[:, :])
            nc.vector.tensor_scalar_mul(dst[:, 2, :], ps[:, 64:128], -1.0)

        wm = {}

        def newmat(tag, alias=None):
            key = alias or tag
            if key not in wm:
                wm[key] = wp.tile([64, 3, 64], F32, name="wm_" + key, tag="wm_" + key)
            return wm[key]

        # ======================= layers =======================
        for li in range(n_layers):
            last = (li == n_layers - 1)

            # ---------- phase K: fwd fft(u2) + y1 update + A2A#1 ----------
            Yt = bigp.tile([128, 2, NFL, 512], F32, name=f"Yt{li}", tag="big", bufs=1)
            fwd_stageA(Yt, u2r, u2i, C["SA0"], C["SA1"], f"f{li}")
            for m in range(2):
                for fp in range(4):
                    fsl = slice(2 * fp, 2 * fp + 2)
                    sl2 = (slice(None), m, fsl, slice(None))
                    pr = psp.tile([128, 512], F32, name=f"pk_r{li}{m}{fp}", tag="ps")
                    pi = psp.tile([128, 512], F32, name=f"pk_i{li}{m}{fp}", tag="ps")
                    stageB(pr[:, :], pi[:, :], Yt, m, fp, inv=False)
                    vkp_r = rp.tile([128, 2, N], BF16, name=f"vkr{li}{m}{fp}", tag="vkr", bufs=2)
                    vkp_i = rp.tile([128, 2, N], BF16, name=f"vki{li}{m}{fp}", tag="vki", bufs=2)
                    for sq_ in range(4):
                        nc.sync.dma_start(
                            out=vkp_r[32 * sq_:32 * (sq_ + 1), :, :],
                            in_=b2_out[4 * m + sq_, 0, fsl, :, :, :]
                                .rearrange("f b k y -> (b k) f y"))
                        nc.sync.dma_start(
                            out=vkp_i[32 * sq_:32 * (sq_ + 1), :, :],
                            in_=b2_out[4 * m + sq_, 1, fsl, :, :, :]
                                .rearrange("f b k y -> (b k) f y"))
                    dgp_r = rp.tile([128, 2, N], F32, name=f"dgr{li}{m}{fp}", tag="dgr", bufs=1)
                    dgp_i = rp.tile([128, 2, N], F32, name=f"dgi{li}{m}{fp}", tag="dgi", bufs=1)
                    nc.sync.dma_start(out=dgp_r[:, :, :], in_=dh_r[:, m, fsl, :])
                    nc.sync.dma_start(out=dgp_i[:, :, :], in_=dh_i[:, m, fsl, :])
                    nc.tensor.matmul(pr[:, :], C["Ic1"][:, :], y1r[sl2], start=False, stop=False)
                    nc.tensor.matmul(pr[:, :], C["I1b"][:, :], vkp_r[:, :, :], start=False, stop=False)
                    nc.tensor.matmul(pr[:, :], C["Insig"][:, :], dgp_r[:, :, :], start=False, stop=True)
                    nc.tensor.matmul(pi[:, :], C["Ic1"][:, :], y1i[sl2], start=False, stop=False)
                    nc.tensor.matmul(pi[:, :], C["I1b"][:, :], vkp_i[:, :, :], start=False, stop=False)
                    nc.tensor.matmul(pi[:, :], C["Insig"][:, :], dgp_i[:, :, :], start=False, stop=True)
                    nc.vector.tensor_copy(y1r[sl2], pr[:, :])
                    nc.vector.tensor_copy(y1i[sl2], pi[:, :])
                    for q in range(4):
                        d = 4 * m + q
                        nc.sync.dma_start(
                            out=b1_in[d, 0, fsl, :, :].rearrange("f k y -> k f y"),
                            in_=y1r[32 * q:32 * (q + 1), m, fsl, :])
                        nc.sync.dma_start(
                            out=b1_in[d, 1, fsl, :, :].rearrange("f k y -> k f y"),
                            in_=y1i[32 * q:32 * (q + 1), m, fsl, :])
            coll("AllToAll", ALU.bypass, RG, [b1_in.ap()], [b1_out.ap()])

            # ---------- image branch ----------
            lnl = stp.tile([1, 1], F32, name=f"lnl{li}", tag="lnl")
            nc.scalar.activation(lnl[0:1, :], lam_t[0:1, li:li + 1], AF.Ln)
            lnb = stp.tile([128, 1], F32, name=f"lnb{li}", tag="lnb")
            ps3 = psw.tile([128, 8], F32, name=f"ps3{li}", tag="pw")
            nc.tensor.matmul(ps3[:, 0:1], onr[0:1, :], lnl[0:1, :], start=True, stop=True)
            nc.vector.tensor_copy(lnb[:, 0:1], ps3[:, 0:1])
            # y2 update, piecewise over (m, fp)
            for m in range(2):
                for fp in range(4):
                    fsl = slice(2 * fp, 2 * fp + 2)
                    sl2 = (slice(None), m, fsl, slice(None))
                    af_r = rp.tile([128, 2, N], BF16, name=f"af_r{li}{m}{fp}", tag="afr", bufs=1)
                    af_i = rp.tile([128, 2, N], BF16, name=f"af_i{li}{m}{fp}", tag="afi", bufs=1)
                    mg = rp.tile([128, 2, N], F32, name=f"mg{li}{m}{fp}", tag="mg", bufs=1)
                    mg2 = psm.tile([128, 512], F32, name=f"mg2{li}{m}{fp}", tag="psm")
                    if fp < 3:
                        nc.vector.tensor_sub(af_r[:, :, :], u2r[:, m, 2 * fp + 1:2 * fp + 3, :], u2r[sl2])
                        nc.vector.tensor_sub(af_i[:, :, :], u2i[:, m, 2 * fp + 1:2 * fp + 3, :], u2i[sl2])
                        nc.vector.scalar_tensor_tensor(af_r[:, :, :], af_r[:, :, :], SIGMA,
                                                       y2r[sl2], op0=ALU.mult, op1=ALU.add)
                        nc.vector.scalar_tensor_tensor(af_i[:, :, :], af_i[:, :, :], SIGMA,
                                                       y2i[sl2], op0=ALU.mult, op1=ALU.add)
                    else:
                        nc.vector.tensor_sub(af_r[:, 0, :], u2r[:, m, 7, :], u2r[:, m, 6, :])
                        nc.vector.tensor_sub(af_i[:, 0, :], u2i[:, m, 7, :], u2i[:, m, 6, :])
                        nc.vector.tensor_sub(af_r[:, 1, :], hu_r[:, m, :], u2r[:, m, 7, :])
                        nc.vector.tensor_sub(af_i[:, 1, :], hu_i[:, m, :], u2i[:, m, 7, :])
                        nc.vector.scalar_tensor_tensor(af_r[:, 0, :], af_r[:, 0, :], SIGMA,
                                                       y2r[:, m, 6, :], op0=ALU.mult, op1=ALU.add)
                        nc.vector.scalar_tensor_tensor(af_i[:, 0, :], af_i[:, 0, :], SIGMA,
                                                       y2i[:, m, 6, :], op0=ALU.mult, op1=ALU.add)
                        nc.vector.scalar_tensor_tensor(af_r[:, 1, :], af_r[:, 1, :], bc(3),
                                                       y2r[:, m, 7, :], op0=ALU.mult, op1=ALU.add)
                        nc.vector.scalar_tensor_tensor(af_i[:, 1, :], af_i[:, 1, :], bc(3),
                                                       y2i[:, m, 7, :], op0=ALU.mult, op1=ALU.add)
                    nc.vector.tensor_mul(mg[:, :, :], af_r[:, :, :], af_r[:, :, :])
                    nc.scalar.activation(mg2[:, :], af_i[:, :, :], AF.Square)
                    nc.vector.tensor_add(mg[:, :, :].rearrange("p a b -> p (a b)"), mg[:, :, :].rearrange("p a b -> p (a b)"), mg2[:, :])
                    nc.scalar.activation(mg[:, :, :], mg[:, :, :], AF.Ln)
                    nc.scalar.activation(mg[:, :, :], mg[:, :, :], AF.Exp,
                                         bias=lnb[:, 0:1], scale=-0.5)
                    nc.vector.tensor_scalar_min(mg[:, :, :], mg[:, :, :], 1.0)
                    nc.vector.tensor_mul(y2r[sl2], af_r[:, :, :], mg[:, :, :])
                    nc.vector.tensor_mul(y2i[sl2], af_i[:, :, :], mg[:, :, :])
            # y2 halo AG
            nc.sync.dma_start(out=hy_in[0, 0, :, :, :], in_=y2r[:, :, 7, :])
            nc.sync.dma_start(out=hy_in[0, 1, :, :, :], in_=y2i[:, :, 7, :])
            coll("AllGather", ALU.bypass, RG, [hy_in.ap()], [hy_out.ap()])
            nc.sync.dma_start(
                out=hy_r[:, :, :],
                in_=hy_out[bass.ds(idx_dn, 1), 0, :, :, :].rearrange("o p c y -> (o p) c y"))
            nc.sync.dma_start(
                out=hy_i[:, :, :],
                in_=hy_out[bass.ds(idx_dn, 1), 1, :, :, :].rearrange("o p c y -> (o p) c y"))


            # ---------- pixel side: deposit, argg1k, Gram ----------
            for p, yt in ((0, y1p_r), (1, y1p_i)):
                for b in range(2):
                    for s_ in range(NCORE):
                        nc.sync.dma_start(
                            out=yt[64 * b + 8 * s_:64 * b + 8 * s_ + 8, :],
                            in_=b1_out[s_, p, :, 16 * b:16 * (b + 1), :]
                                .rearrange("f k y -> f (k y)"))
            nc.vector.scalar_tensor_tensor(y1p_r[:, :], y1p_r[:, :], -TAU, xpr[:, :],
                                           op0=ALU.mult, op1=ALU.add)
            nc.vector.scalar_tensor_tensor(y1p_i[:, :], y1p_i[:, :], -TAU, xpi[:, :],
                                           op0=ALU.mult, op1=ALU.add)
            psG = psw.tile([64, 128], F32, name=f"psG{li}", tag="pw")
            for k in range(32):
                ks = slice(128 * k, 128 * (k + 1))
                psT = psp.tile([128, 512], BF16, name=f"psT{li}_{k}", tag="ps")
                nc.tensor.transpose(psT[:, 0:128], y1p_r[:, ks], C["I128b"][:, :])
                nc.tensor.transpose(psT[:, 128:256], y1p_i[:, ks], C["I128b"][:, :])
                Tk = tkp.tile([128, 3, 128], BF16, name=f"Tk{li}_{k}", tag="Tk")
                nc.vector.tensor_copy(Tk[:, 0:2, :], psT[:, 0:256])
                nc.vector.tensor_scalar_mul(Tk[:, 2, :], psT[:, 0:128], -1.0)
                for b in range(2):
                    bs = slice(64 * b, 64 * (b + 1))
                    nc.tensor.matmul(psG[:, :], Tk[:, 0, bs], Tk[:, 0:2, bs],
                                     start=(k == 0 and b == 0), stop=False)
                    nc.tensor.matmul(psG[:, :], Tk[:, 1, bs], Tk[:, 1:3, bs],
                                     start=False, stop=(k == 31 and b == 1))
            gl = stp.tile([64, 128], F32, name=f"gl{li}", tag="gl")
            nc.vector.tensor_copy(gl[:, :], psG[:, :])
            nc.sync.dma_start(out=g_in[:, :], in_=gl[:, :])
            coll("AllReduce", ALU.add, RG, [g_in.ap()], [g_out.ap()])
            G = newmat("G")
            nc.sync.dma_start(out=G[:, 0:2, :].rearrange("p a b -> p (a b)"),
                              in_=g_out[:, :])
            nc.vector.tensor_scalar_mul(G[:, 2, :], G[:, 1, :], -1.0)

            # ---------- W chain ----------
            trs = stp.tile([64, 1], F32, name=f"trs{li}", tag="trs")
            scrap = stp.tile([64, 64], F32, name=f"scrap{li}", tag="scrap")
            lg = stp.tile([1, 4], F32, name=f"lg{li}", tag="lg")
            rtr = stp.tile([1, 1], F32, name=f"rtr{li}", tag="rtr")
            bres = stp.tile([64, 1], F32, name=f"bres{li}", tag="bres")
            psb = psw.tile([64, 8], F32, name=f"psb{li}", tag="pw")
            pst = psw.tile([1, 8], F32, name=f"pst{li}", tag="pw")

            def trace_of(Mt, dstcol, li=li, scrap=scrap, trs=trs, pst=pst):
                nc.vector.scalar_tensor_tensor(scrap[:, :], Mt[:, 0, :], 1.0,
                                               C["I64"][:, :], op0=ALU.mult,
                                               op1=ALU.mult, accum_out=trs[:, 0:1])
                nc.tensor.matmul(pst[0:1, dstcol:dstcol + 1], trs[:, :], on64[:, :],
                                 start=True, stop=True)

            def bcast64(src_ap, dst, col, psb=psb):
                nc.tensor.matmul(psb[:, col:col + 1], on64r[0:1, :], src_ap,
                                 start=True, stop=True)
                nc.vector.tensor_copy(dst[:, 0:1], psb[:, col:col + 1])

            Bm = newmat("Bm")
            B2 = newmat("B2")
            trace_of(G, 0)
            nc.vector.tensor_copy(lg[0:1, 0:1], pst[0:1, 0:1])
            nc.scalar.activation(lg[0:1, 1:2], lg[0:1, 0:1], AF.Ln)
            nc.vector.reciprocal(rtr[0:1, :], lg[0:1, 0:1])
            bcast64(rtr[0:1, :], bres, 0)
            for pl in range(3):
                nc.vector.tensor_scalar_mul(Bm[:, pl, :], G[:, pl, :], bres[:, 0:1])
            for it in range(M_POWER):
                cmm(B2, Bm, Bm, f"q{li}_{it}")
                trace_of(B2, 1)
                nc.vector.tensor_copy(lg[0:1, 2:3], pst[0:1, 1:2])
                nc.scalar.activation(lg[0:1, 3:4], lg[0:1, 2:3], AF.Ln)
                nc.vector.tensor_scalar(lg[0:1, 1:2], lg[0:1, 1:2], 2.0, None, op0=ALU.mult)
                nc.vector.tensor_add(lg[0:1, 1:2], lg[0:1, 1:2], lg[0:1, 3:4])
                nc.vector.reciprocal(rtr[0:1, :], lg[0:1, 2:3])
                bcast64(rtr[0:1, :], bres, 1)
                for pl in range(3):
                    nc.vector.tensor_scalar_mul(Bm[:, pl, :], B2[:, pl, :], bres[:, 0:1])
            lam_s = stp.tile([1, 1], F32, name=f"lam_s{li}", tag="lam_s")
            nc.scalar.activation(lam_s[0:1, :], lg[0:1, 1:2], AF.Exp,
                                 scale=1.0 / (2 ** M_POWER))
            ilam = stp.tile([64, 1], F32, name=f"ilam{li}", tag="ilam")
            nc.vector.reciprocal(rtr[0:1, :], lam_s[0:1, :])
            bcast64(rtr[0:1, :], ilam, 2)
            Gh = newmat("Gh")
            for pl in range(3):
                nc.vector.tensor_scalar_mul(Gh[:, pl, :], G[:, pl, :], ilam[:, 0:1])
            t2s = stp.tile([1, 1], F32, name=f"t2s{li}", tag="t2s")
            nc.scalar.activation(t2s[0:1, :], lam_t[0:1, NLAYERS + li:NLAYERS + li + 1],
                                 AF.Square, scale=TAU)
            ths = stp.tile([1, 1], F32, name=f"ths{li}", tag="ths")
            nc.vector.tensor_scalar_mul(ths[0:1, 0:1],
                                        lam_t[0:1, NLAYERS + li:NLAYERS + li + 1], TAU)
            onem = stp.tile([1, 1], F32, name=f"onem{li}", tag="onem")
            nc.vector.tensor_scalar(onem[0:1, :], t2s[0:1, :], -1.0, 1.0,
                                    op0=ALU.mult, op1=ALU.add)
            nc.vector.reciprocal(onem[0:1, :], onem[0:1, :])
            i1m = stp.tile([64, 1], F32, name=f"i1m{li}", tag="i1m")
            bcast64(onem[0:1, :], i1m, 3)
            nt2 = stp.tile([64, 1], F32, name=f"nt2{li}", tag="nt2")
            bcast64(t2s[0:1, :], nt2, 4)
            nc.vector.tensor_scalar_mul(nt2[:, 0:1], nt2[:, 0:1], -1.0)
            X = newmat("X")
            nc.vector.scalar_tensor_tensor(X[:, 0, :], C["I64"][:, :], nt2[:, 0:1],
                                           Gh[:, 0, :], op0=ALU.mult, op1=ALU.add)
            nc.vector.tensor_scalar_mul(X[:, 0, :], X[:, 0, :], i1m[:, 0:1])
            for pl in (1, 2):
                nc.vector.tensor_scalar_mul(X[:, pl, :], Gh[:, pl, :], i1m[:, 0:1])
            X2 = newmat("X2", alias="Bm")
            X4 = newmat("X4", alias="B2")
            Yp = newmat("Yp", alias="B2")
            for k_, (a_, b_, c_) in enumerate(SIGN_COEFFS):
                cmm(X2, X, X, f"s2_{li}_{k_}")
                cmm(X4, X2, X2, f"s4_{li}_{k_}")
                for pl in range(3):
                    nc.vector.tensor_scalar_mul(Yp[:, pl, :], X4[:, pl, :], c_)
                    nc.vector.scalar_tensor_tensor(Yp[:, pl, :], X2[:, pl, :], b_,
                                                   Yp[:, pl, :], op0=ALU.mult, op1=ALU.add)
                nc.vector.scalar_tensor_tensor(Yp[:, 0, :], C["I64"][:, :], a_,
                                               Yp[:, 0, :], op0=ALU.mult, op1=ALU.add)
                cmm(X, X, Yp, f"sx_{li}_{k_}")
            P = newmat("P", alias="Bm")
            for pl in range(3):
                nc.vector.tensor_scalar_mul(P[:, pl, :], X[:, pl, :], 0.5)
            nc.vector.scalar_tensor_tensor(P[:, 0, :], C["I64"][:, :], 0.5,
                                           P[:, 0, :], op0=ALU.mult, op1=ALU.add)
            T = newmat("T")
            for pl in range(3):
                nc.vector.tensor_scalar_mul(T[:, pl, :], Gh[:, pl, :], 2.0 / Q_HI)
            nc.vector.scalar_tensor_tensor(T[:, 0, :], C["I64"][:, :], -1.0,
                                           T[:, 0, :], op0=ALU.mult, op1=ALU.add)
            b1m = newmat("b1m", alias="G")
            b2m = newmat("b2m", alias="X")
            tm = newmat("tm", alias="B2")
            for pl in range(3):
                nc.vector.memset(b1m[:, pl, :], 0.0)
                nc.vector.memset(b2m[:, pl, :], 0.0)
            mats = [b1m, b2m, tm]
            for ci_idx, ci in enumerate(Q_COEF[::-1][:-1]):
                bb1, bb2, tt = mats
                cmm(tt, T, bb1, f"cl{li}_{ci_idx}")
                for pl in range(3):
                    nc.vector.scalar_tensor_tensor(tt[:, pl, :], tt[:, pl, :], 2.0,
                                                   bb2[:, pl, :], op0=ALU.mult,
                                                   op1=ALU.subtract)
                nc.vector.scalar_tensor_tensor(tt[:, 0, :], C["I64"][:, :], ci,
                                               tt[:, 0, :], op0=ALU.mult, op1=ALU.add)
                mats = [tt, bb1, bb2]
            bb1, bb2, _ = mats
            Q = newmat("Q", alias="Gh")
            cmm(Q, T, bb1, f"qf{li}")
            for pl in range(3):
                nc.vector.tensor_sub(Q[:, pl, :], Q[:, pl, :], bb2[:, pl, :])
            nc.vector.scalar_tensor_tensor(Q[:, 0, :], C["I64"][:, :], Q_COEF[0],
                                           Q[:, 0, :], op0=ALU.mult, op1=ALU.add)
            PQ = newmat("PQ", alias="X")
            cmm(PQ, P, Q, f"pq{li}")
            Wt = newmat("Wt", alias="B2")
            nth = stp.tile([64, 1], F32, name=f"nth{li}", tag="nth")
            bcast64(ths[0:1, 0:1], nth, 5)
            nc.vector.tensor_scalar_mul(nth[:, 0:1], nth[:, 0:1], -1.0)
            for pl in range(3):
                nc.vector.scalar_tensor_tensor(Wt[:, pl, :], PQ[:, pl, :], nth[:, 0:1],
                                               P[:, pl, :], op0=ALU.mult, op1=ALU.add)
            Wb = wp.tile([128, 3, 64], BF16, name=f"Wb{li}", tag="Wb")
            nc.vector.tensor_copy(Wb[0:64, :, :], Wt[:, :, :])
            nc.sync.dma_start(out=Wb[64:128, :, :], in_=Wb[0:64, :, :])

            # ---------- recon + V + A2A#2 ----------
            for b in range(2):
                bs = slice(64 * b, 64 * (b + 1))
                for ch in range(8):
                    cs = slice(512 * ch, 512 * (ch + 1))
                    pR = psp.tile([128, 512], F32, name=f"pR{li}_{b}_{ch}", tag="ps")
                    pI = psp.tile([128, 512], F32, name=f"pI{li}_{b}_{ch}", tag="ps")
                    wbs = slice(64 * b, 64 * (b + 1))
                    nc.tensor.matmul(pR[0:64, :], Wb[wbs, 0, :], y1p_r[bs, cs], start=True, stop=False)
                    nc.tensor.matmul(pR[0:64, :], Wb[wbs, 2, :], y1p_i[bs, cs], start=False, stop=True)
                    nc.tensor.matmul(pI[0:64, :], Wb[wbs, 0, :], y1p_i[bs, cs], start=True, stop=False)
                    nc.tensor.matmul(pI[0:64, :], Wb[wbs, 1, :], y1p_r[bs, cs], start=False, stop=True)
                    vst_r = rp.tile([64, 512], BF16, name=f"v_r{li}{b}{ch}", tag="vsr")
                    vst_i = rp.tile([64, 512], BF16, name=f"v_i{li}{b}{ch}", tag="vsi")
                    if not last:
                        nc.vector.scalar_tensor_tensor(vst_r[:, :], xpr[bs, cs], -0.5,
                                                       pR[0:64, :], op0=ALU.mult, op1=ALU.add)
                        nc.vector.tensor_scalar_mul(vst_r[:, :], vst_r[:, :], 2.0 * C2)
                        nc.vector.scalar_tensor_tensor(vst_i[:, :], xpi[bs, cs], -0.5,
                                                       pI[0:64, :], op0=ALU.mult, op1=ALU.add)
                        nc.vector.tensor_scalar_mul(vst_i[:, :], vst_i[:, :], 2.0 * C2)
                    else:
                        nc.vector.tensor_copy(vst_r[:, :], pR[0:64, :])
                        nc.vector.tensor_copy(vst_i[:, :], pI[0:64, :])
                    nc.vector.tensor_copy(xpr[bs, cs], pR[0:64, :])
                    nc.vector.tensor_copy(xpi[bs, cs], pI[0:64, :])
                    for dst in range(NCORE):
                        fs = slice(8 * dst, 8 * dst + 8)
                        nc.sync.dma_start(
                            out=b2_in[dst, 0, :, b, :, :]
                                .rearrange("f k y -> f (k y)")[:, cs],
                            in_=vst_r[fs, :])
                        nc.sync.dma_start(
                            out=b2_in[dst, 1, :, b, :, :]
                                .rearrange("f k y -> f (k y)")[:, cs],
                            in_=vst_i[fs, :])
            coll("AllToAll", ALU.bypass, RG, [b2_in.ap()], [b2_out.ap()])

            Qt = bigp.tile([128, 2, NFL, 512], F32, name=f"Qt{li}", tag="big", bufs=1)
            fwd_stageA(Qt, y1r, y1i, C["SI0"], C["SI1"], f"i{li}")
            for m in range(2):
                for fp in [1, 2, 3, 0]:
                    fsl = slice(2 * fp, 2 * fp + 2)
                    sl2 = (slice(None), m, fsl, slice(None))
                    pr = psp.tile([128, 512], F32, name=f"pm_r{li}{m}{fp}", tag="ps")
                    pi = psp.tile([128, 512], F32, name=f"pm_i{li}{m}{fp}", tag="ps")
                    stageB(pr[:, :], pi[:, :], Qt, m, fp, inv=True)
                    for ppp, y2t, x2t, hyt in ((pr, y2r, x2r, hy_r), (pi, y2i, x2i, hy_i)):
                        nc.tensor.matmul(ppp[:, :], C["I1b"][:, :], x2t[sl2],
                                         start=False, stop=False)
                        if fp == 3:
                            nc.tensor.matmul(ppp[:, 0:256], C["ITb"][:, :],
                                             y2t[:, m, 6, :], start=False, stop=False)
                            nc.tensor.matmul(ppp[:, 256:512], IT7[:, :],
                                             y2t[:, m, 7, :], start=False, stop=False)
                        else:
                            nc.tensor.matmul(ppp[:, :], C["ITb"][:, :], y2t[sl2],
                                             start=False, stop=False)
                        if fp == 0:
                            nc.tensor.matmul(ppp[:, 0:256], ITn0[:, :], hyt[:, m, :],
                                             start=False, stop=False)
                            nc.tensor.matmul(ppp[:, 256:512], C["ITnb"][:, :],
                                             y2t[:, m, 0, :], start=False, stop=True)
                        else:
                            nc.tensor.matmul(ppp[:, :], C["ITnb"][:, :],
                                             y2t[:, m, 2 * fp - 1:2 * fp + 1, :],
                                             start=False, stop=True)
                    if not last:
                        nc.vector.scalar_tensor_tensor(u2r[sl2], x2r[sl2], -0.5, pr[:, :],
                                                       op0=ALU.mult, op1=ALU.add)
                        nc.vector.tensor_scalar_mul(u2r[sl2], u2r[sl2], 2.0)
                        nc.vector.scalar_tensor_tensor(u2i[sl2], x2i[sl2], -0.5, pi[:, :],
                                                       op0=ALU.mult, op1=ALU.add)
                        nc.vector.tensor_scalar_mul(u2i[sl2], u2i[sl2], 2.0)
                    nc.vector.tensor_copy(x2r[sl2], pr[:, :])
                    nc.vector.tensor_copy(x2i[sl2], pi[:, :])
            if not last:
                push_u2_halo()

        # ---------------- final ----------------
        xfr = bigp.tile([128, 2, NFL, N], BF16, name="xfr", tag="bigs", bufs=2)
        xfi = bigp.tile([128, 2, NFL, N], BF16, name="xfi", tag="bigs", bufs=2)
        for p, xt in ((0, xfr), (1, xfi)):
            for m in range(2):
                for sq_ in range(4):
                    nc.sync.dma_start(
                        out=xt[32 * sq_:32 * (sq_ + 1), m, :, :],
                        in_=b2_out[4 * m + sq_, p, :, :, :, :]
                            .rearrange("f b k y -> (b k) f y"))
        Qtf = bigp.tile([128, 2, NFL, 512], F32, name="Qtf", tag="big", bufs=1)
        fwd_stageA(Qtf, xfr, xfi, C["SI0"], C["SI1"], "fin")
        for m in range(2):
            for fp in range(4):
                pr = psp.tile([128, 512], F32, name=f"pf_r{m}{fp}", tag="ps")
                pi = psp.tile([128, 512], F32, name=f"pf_i{m}{fp}", tag="ps")
                stageB(pr[:, :], pi[:, :], Qtf, m, fp, inv=True, stop=True)
                sl2 = (slice(None), m, slice(2 * fp, 2 * fp + 2), slice(None))
                op_r = bigp.tile([128, 2, N], F32, name=f"op_r{m}{fp}", tag="bigs", bufs=2)
                op_i = bigp.tile([128, 2, N], F32, name=f"op_i{m}{fp}", tag="bigs", bufs=2)
                nc.vector.scalar_tensor_tensor(op_r[:, :, :], pr[:, :], -1.0 / TAU,
                                               x2r[sl2], op0=ALU.mult, op1=ALU.add)
                nc.vector.scalar_tensor_tensor(op_i[:, :, :], pi[:, :], -1.0 / TAU,
                                               x2i[sl2], op0=ALU.mult, op1=ALU.add)
                nc.sync.dma_start(out=out_r[:, m, 2 * fp:2 * fp + 2, :], in_=op_r[:, :, :])
                nc.sync.dma_start(out=out_i[:, m, 2 * fp:2 * fp + 2, :], in_=op_i[:, :, :])

        stack.close()

    nc.compile()
    return nc


_CACHE = {}


def _get_nc(n_layers=NLAYERS):
    if n_layers not in _CACHE:
        _CACHE[n_layers] = build(n_layers)
    return _CACHE[n_layers]


def host_shard(d_real, d_imag, lambdaS, lambdaL):
    d_r = np.asarray(d_real, np.float32).reshape(NF, N, N)
    d_i = np.asarray(d_imag, np.float32).reshape(NF, N, N)
    dTr = d_r.transpose(0, 2, 1)
    dTi = d_i.transpose(0, 2, 1)
    in_maps = []
    for c in range(NCORE):
        fr = slice(8 * c, 8 * c + 8)
        dk_rc = dTr[fr].reshape(NFL, 2, 128, N).transpose(2, 1, 0, 3).copy()
        dk_ic = dTi[fr].reshape(NFL, 2, 128, N).transpose(2, 1, 0, 3).copy()
        blk_r = dTr[:, 32 * c:32 * c + 32, :]
        blk_i = dTi[:, 32 * c:32 * c + 32, :]
        dp_rc = blk_r.reshape(NF, 2, 16, N).transpose(1, 0, 2, 3).reshape(128, 4096).copy()
        dp_ic = blk_i.reshape(NF, 2, 16, N).transpose(1, 0, 2, 3).reshape(128, 4096).copy()
        m0 = 0.0 if c == 0 else 1.0
        m7 = 0.0 if c == NCORE - 1 else 1.0
        in_maps.append({
            "dk_r": dk_rc, "dk_i": dk_ic, "dp_r": dp_rc, "dp_i": dp_ic,
            "lamS": np.asarray(lambdaS, np.float32).reshape(1, NLAYERS).copy(),
            "lamL": np.asarray(lambdaL, np.float32).reshape(1, NLAYERS).copy(),
            "cmeta": np.array([[0, 0, m0, m7]], np.float32),
            "cmeta_u": np.array([[min(c + 1, NCORE - 1), max(c - 1, 0)]], np.uint32),
        })
    return in_maps


def host_gather(results):
    out = np.zeros((NF, N, N), np.complex64)
    for c, res in enumerate(results):
        img = (res["out_r"] + 1j * res["out_i"]).astype(np.complex64)
        out[8 * c:8 * c + 8] = img.transpose(2, 1, 0, 3).reshape(NFL, N, N)
    return out.reshape(1, 1, NF, N, N)


def kernel(d_real, d_imag, lambdaS, lambdaL):
    nc = _get_nc()
    in_maps = host_shard(d_real, d_imag, lambdaS, lambdaL)
    res = bass_utils.run_bass_kernel_spmd(nc, in_maps, core_ids=list(range(NCORE)))
    return host_gather(res.results)



# revision 4
# speedup vs baseline: 10.3287x; 10.3287x over previous
"""TRN2 Bass kernel: 8-layer Chambolle-Pock MRI reconstruction on 8 NeuronCores.

Sharding: frames (8/core) for FFTs + elementwise; k-space rows (8192 px/core)
for the low-rank prox (Gram trick + matmul-only spectral filter, no eigensolver).
Cross-core per layer: AllToAll (y1 frame->pixel), AllToAll (x1k V pixel->frame),
AllReduce (64x64 Gram), 2 small AllGathers (temporal halo).

v2: single packed input/output tensor (cuts per-call dispatch cost), pixel-side
init derived on device via an extra A2A (drops the dp input), d-hat held in SBUF
pre-scaled (no per-layer HBM reloads), bf16 DFT intermediates, rsqrt-based
soft-threshold, block-complex (128x128 real) W chain, merged-b recon PSUM,
and multi-dim-AP batched DMAs.
"""
import numpy as np

import concourse.bass as bass
import concourse.bacc as bacc
import concourse.mybir as mybir
import concourse.tile as tile
from concourse import bass_utils

F32 = mybir.dt.float32
BF16 = mybir.dt.bfloat16
AF = mybir.ActivationFunctionType
ALU = mybir.AluOpType

NCORE = 8
NF = 64
NFL = 8
N = 256
NLAYERS = 8
SIGMA = float(np.float32(1.0 / np.sqrt(8.0)))
TAU = SIGMA
C1 = float(np.float32(1.0 / (1.0 + SIGMA)))
C2 = C1 * SIGMA
M_POWER = 8
SIGN_COEFFS = [(3.4445, -4.7750, 2.0315)] * 7 + [(1.875, -1.25, 0.375)] * 3
RG = [list(range(NCORE))]

W_IN = 2 * 4096 + 32
W_OUT = 2 * 4096


def _fit_q(deg=14, lo=0.068, hi=1.05, npts=6000):
    xs_in = np.linspace(lo, hi, npts)
    xs_out = np.linspace(0, 0.05, 300)

    def cheb(x, d, b=hi):
        t = (2 * x - b) / b
        V = np.zeros((len(x), d + 1))
        V[:, 0] = 1
        if d >= 1:
            V[:, 1] = t
        for k in range(2, d + 1):
            V[:, k] = 2 * t * V[:, k - 1] - V[:, k - 2]
        return V

    Vi = cheb(xs_in, deg)
    Vo = cheb(xs_out, deg)
    A = np.vstack([np.sqrt(xs_in)[:, None] * Vi, 1e-5 * Vo])
    y = np.concatenate([np.ones(npts), np.zeros(len(xs_out))])
    coef, *_ = np.linalg.lstsq(A, y, rcond=None)
    return [float(c) for c in coef], hi


Q_COEF, Q_HI = _fit_q()


def _dft():
    k = np.arange(N)
    W = np.exp(-2j * np.pi * np.outer(k, k) / N)
    return W.real.astype(np.float32), W.imag.astype(np.float32)


def _chunk(a):
    return np.ascontiguousarray(np.stack([a[0:128], a[128:256]], axis=1))


def build(n_layers=NLAYERS, single_core=False):
    nc = bacc.Bacc("TRN2", target_bir_lowering=False, debug=False,
                   num_devices=1 if single_core else NCORE)

    xin = nc.dram_tensor("xin", [128, W_IN], F32, kind="ExternalInput")
    out = nc.dram_tensor("out", [128, W_OUT], F32, kind="ExternalOutput")

    Fr, Fi = _dft()
    sA = C1 * SIGMA / N
    sI = -TAU / N
    id128 = np.eye(128, dtype=np.float32)
    consts_np = {
        "SA0": (_chunk(np.concatenate([Fr * sA, Fi * sA], 1)), BF16),
        "SA1": (_chunk(np.concatenate([-Fi * sA, Fr * sA], 1)), BF16),
        "SI0": (_chunk(np.concatenate([Fr * sI, -Fi * sI], 1)), BF16),
        "SI1": (_chunk(np.concatenate([Fi * sI, Fr * sI], 1)), BF16),
        "Br": (_chunk(Fr), BF16),
        "Bi": (_chunk(Fi), BF16),
        "Bn": (_chunk(-Fi), BF16),
        "I128b": (id128, BF16),
        "Ic1": (id128 * C1, BF16),
        "I1b": (id128, BF16),
        "ITb": (id128 * TAU, BF16),
        "ITnb": (id128 * (-TAU), BF16),
        "I128f": (id128, F32),
        "I64": (np.eye(64, dtype=np.float32), F32),
    }
    handles = {}
    for k, (v, dt) in consts_np.items():
        handles[k] = nc.inline_tensor(v.astype(mybir.dt.np(dt)), name="c" + k)

    b1_in = nc.dram_tensor("b1_in", [NCORE, 2, NFL, 32, N], BF16)
    b1_out = nc.dram_tensor("b1_out", [NCORE, 2, NFL, 32, N], BF16)
    b2_in = nc.dram_tensor("b2_in", [NCORE, 2, NFL, 2, 16, N], BF16)
    b2_out = nc.dram_tensor("b2_out", [NCORE, 2, NFL, 2, 16, N], BF16)
    g_in = nc.dram_tensor("g_in", [64, 128], F32)
    g_out = nc.dram_tensor("g_out", [64, 128], F32, addr_space="Shared")
    nrm_in = nc.dram_tensor("nrm_in", [1, 1], F32)
    nrm_out = nc.dram_tensor("nrm_out", [1, 1], F32, addr_space="Shared")
    hu_in = nc.dram_tensor("hu_in", [1, 2, 128, 2, N], BF16)
    hu_out = nc.dram_tensor("hu_out", [NCORE, 2, 128, 2, N], BF16, addr_space="Shared")
    hy_in = nc.dram_tensor("hy_in", [1, 2, 128, 2, N], BF16)
    hy_out = nc.dram_tensor("hy_out", [NCORE, 2, 128, 2, N], BF16, addr_space="Shared")

    with tile.TileContext(nc) as tc:
        import contextlib
        stack = contextlib.ExitStack()

        def pool(name, bufs, space="SBUF"):
            return stack.enter_context(tc.tile_pool(name=name, bufs=bufs, space=space))

        cp = pool("cp", 1)
        sbp = pool("sbp", 1)
        stp = pool("stp", 1)
        rp = pool("rp", 2)
        bigp = pool("bigp", 1)
        tkp = pool("tkp", 2)
        wp = pool("wp", 1)
        psp = pool("psp", 4, space="PSUM")
        psm = pool("psm", 1, space="PSUM")
        psw = pool("psw", 3, space="PSUM")

        def coll(kind, op, replica_groups, ins, outs):
            if single_core:
                nc.sync.dma_start(out=outs[0], in_=ins[0]) if ins[0].size() == outs[0].size() \
                    else nc.sync.dma_start(out=outs[0][0:1], in_=ins[0][0:1])
            else:
                nc.gpsimd.collective_compute(kind, op, replica_groups=replica_groups,
                                             ins=ins, outs=outs)

        C = {}
        for k, (v, dt) in consts_np.items():
            t = cp.tile(list(v.shape), dt, name="k" + k, tag="k" + k)
            if v.ndim == 2:
                nc.sync.dma_start(out=t[:, :], in_=handles[k][:, :])
            else:
                nc.sync.dma_start(out=t[:, :, :], in_=handles[k][:, :, :])
            C[k] = t

        y1r = sbp.tile([128, 2, NFL, N], BF16, name="y1r", tag="y1r")
        y1i = sbp.tile([128, 2, NFL, N], BF16, name="y1i", tag="y1i")
        u2r = sbp.tile([128, 2, NFL, N], BF16, name="u2r", tag="u2r")
        u2i = sbp.tile([128, 2, NFL, N], BF16, name="u2i", tag="u2i")
        y2r = sbp.tile([128, 2, NFL, N], BF16, name="y2r", tag="y2r")
        y2i = sbp.tile([128, 2, NFL, N], BF16, name="y2i", tag="y2i")
        x2r = sbp.tile([128, 2, NFL, N], BF16, name="x2r", tag="x2r")
        x2i = sbp.tile([128, 2, NFL, N], BF16, name="x2i", tag="x2i")
        dhs_r = sbp.tile([128, 2, NFL, N], BF16, name="dhs_r", tag="dhs_r")
        dhs_i = sbp.tile([128, 2, NFL, N], BF16, name="dhs_i", tag="dhs_i")
        xpr = sbp.tile([128, 4096], BF16, name="xpr", tag="xpr")
        xpi = sbp.tile([128, 4096], BF16, name="xpi", tag="xpi")
        y1p_r = sbp.tile([128, 4096], BF16, name="y1p_r", tag="y1p_r")
        y1p_i = sbp.tile([128, 4096], BF16, name="y1p_i", tag="y1p_i")
        hu_r = sbp.tile([128, 2, N], BF16, name="hu_r", tag="hu_r")
        hu_i = sbp.tile([128, 2, N], BF16, name="hu_i", tag="hu_i")
        hy_r = sbp.tile([128, 2, N], BF16, name="hy_r", tag="hy_r")
        hy_i = sbp.tile([128, 2, N], BF16, name="hy_i", tag="hy_i")
        IT7 = sbp.tile([128, 128], BF16, name="IT7", tag="IT7")
        ITn0 = sbp.tile([128, 128], BF16, name="ITn0", tag="ITn0")
        sc = sbp.tile([128, 8], F32, name="sc", tag="sc")
        scal = sbp.tile([128, 32], F32, name="scal", tag="scal")

        def bc(col):
            return sc[:, col:col + 1]

        # ---------------- init ----------------
        nc.sync.dma_start(out=scal[:, :], in_=xin[:, 8192:8192 + 32])
        # rectify lambdas in place (cols 0:16)
        nc.vector.tensor_relu(scal[:, 0:16], scal[:, 0:16])

        dsr = bigp.tile([128, 2, NFL, N], F32, name="dsr", tag="bigs", bufs=2)
        dsi = bigp.tile([128, 2, NFL, N], F32, name="dsi", tag="bigs", bufs=2)
        nc.sync.dma_start(
            out=dsr[:, :, :, :],
            in_=xin[:, 0:4096].rearrange("p (m f y) -> p m f y", m=2, f=NFL))
        nc.sync.dma_start(
            out=dsi[:, :, :, :],
            in_=xin[:, 4096:8192].rearrange("p (m f y) -> p m f y", m=2, f=NFL))
        acc = stp.tile([128, 2], F32, name="acc", tag="acc")
        sq = bigp.tile([128, 2048], F32, name="sq", tag="big", bufs=1)
        nc.scalar.activation(sq[:, :], dsr[:, :, 0:4, :], AF.Square, accum_out=acc[:, 0:1])
        sq_b = bigp.tile([128, 2048], F32, name="sq_b", tag="big", bufs=1)
        nc.scalar.activation(sq_b[:, :], dsr[:, :, 4:8, :], AF.Square, accum_out=acc[:, 1:2])
        acs = stp.tile([128, 2], F32, name="acs", tag="acs")
        nc.vector.tensor_add(acs[:, 0:1], acc[:, 0:1], acc[:, 1:2])
        sq_c = bigp.tile([128, 2048], F32, name="sq_c", tag="big", bufs=1)
        nc.scalar.activation(sq_c[:, :], dsi[:, :, 0:4, :], AF.Square, accum_out=acc[:, 0:1])
        sq_d = bigp.tile([128, 2048], F32, name="sq_d", tag="big", bufs=1)
        nc.scalar.activation(sq_d[:, :], dsi[:, :, 4:8, :], AF.Square, accum_out=acc[:, 1:2])
        nc.vector.tensor_add(acs[:, 1:2], acc[:, 0:1], acc[:, 1:2])
        nc.vector.tensor_add(acs[:, 0:1], acs[:, 0:1], acs[:, 1:2])
        on1 = stp.tile([128, 1], F32, name="on1", tag="on1")
        nc.vector.memset(on1[:, :], 1.0)
        onr = stp.tile([1, 128], F32, name="onr", tag="onr")
        nc.vector.memset(onr[0:1, :], 1.0)
        on64 = stp.tile([64, 1], F32, name="on64", tag="on64")
        nc.vector.memset(on64[:, :], 1.0)

        ps0 = psw.tile([128, 8], F32, name="ps0", tag="pw")
        nc.tensor.matmul(ps0[0:1, 0:1], acs[:, 0:1], on1[:, :], start=True, stop=True)
        nrm_st = stp.tile([1, 1], F32, name="nrm_st", tag="nrm")
        nc.vector.tensor_copy(nrm_st[0:1, :], ps0[0:1, 0:1])
        nc.sync.dma_start(out=nrm_in[:, :], in_=nrm_st[0:1, :])
        coll("AllReduce", ALU.add, RG, [nrm_in.ap()], [nrm_out.ap()])
        nc.sync.dma_start(out=nrm_st[0:1, :], in_=nrm_out[:, :])
        rec = stp.tile([1, 1], F32, name="rec", tag="rec")
        nc.vector.reciprocal(rec[0:1, :], nrm_st[0:1, :])
        nc.scalar.activation(rec[0:1, :], rec[0:1, :], AF.Sqrt)
        ps1 = psw.tile([128, 8], F32, name="ps1", tag="pw")
        nc.tensor.matmul(ps1[:, 0:1], onr[0:1, :], rec[0:1, :], start=True, stop=True)
        nc.vector.tensor_copy(bc(0), ps1[:, 0:1])

        # normalize d in place (fp32), derive pre-scaled bf16 d-hat term for y1 update
        nc.vector.tensor_scalar_mul(dsr[:, :, :, :], dsr[:, :, :, :], bc(0))
        nc.vector.tensor_scalar_mul(dsi[:, :, :, :], dsi[:, :, :, :], bc(0))
        nc.vector.tensor_scalar_mul(dhs_r[:, :, :, :], dsr[:, :, :, :], -C1 * SIGMA)
        nc.vector.tensor_scalar_mul(dhs_i[:, :, :, :], dsi[:, :, :, :], -C1 * SIGMA)

        # b2_out prefill: V0 = c2 * d-hat (bf16) via piece staging
        for p, src in ((0, dsr), (1, dsi)):
            for s in range(NCORE):
                m, q = s // 4, s % 4
                vp = bigp.tile([32, 2, NFL, N], BF16, name=f"v0_{p}_{s}", tag="big", bufs=1)
                nc.vector.tensor_scalar_mul(vp[:, :, :, :],
                                            src[32 * q:32 * (q + 1), :, :, :], C2)
                nc.sync.dma_start(
                    out=b2_out[s, p, :, :, :, :].rearrange("f b k y -> (b k) f y"),
                    in_=vp[:, m, :, :])

        # stage d-hat (bf16) through the frame->pixel A2A to init x-tilde / y1p
        # (u2r/u2i double as the bf16 d-hat staging copies, as in the init ifft)
        nc.vector.tensor_copy(u2r[:, :, :, :], dsr[:, :, :, :])
        nc.vector.tensor_copy(u2i[:, :, :, :], dsi[:, :, :, :])
        for m in range(2):
            for p, src in ((0, u2r), (1, u2i)):
                for f in range(NFL):
                    nc.sync.dma_start(out=b1_in[4 * m:4 * m + 4, p, f, :, :],
                                      in_=src[:, m, f, :])
        coll("AllToAll", ALU.bypass, RG, [b1_in.ap()], [b1_out.ap()])
        for p, yt in ((0, y1p_r), (1, y1p_i)):
            for b in range(2):
                for s_ in range(NCORE):
                    nc.sync.dma_start(
                        out=yt[64 * b + 8 * s_:64 * b + 8 * s_ + 8, :],
                        in_=b1_out[s_, p, :, 16 * b:16 * (b + 1), :]
                            .rearrange("f k y -> f (k y)"))
        nc.vector.tensor_copy(xpr[:, :], y1p_r[:, :])
        nc.vector.tensor_copy(xpi[:, :], y1p_i[:, :])

        # per-core scalars: bc(1)=m0, bc(2)=m7, bc(3)=sigma*m7
        nc.vector.tensor_copy(bc(1), scal[:, 16:17])
        nc.vector.tensor_copy(bc(2), scal[:, 17:18])
        nc.vector.tensor_scalar_mul(bc(3), scal[:, 17:18], SIGMA)
        nc.vector.tensor_scalar_mul(IT7[:, :], C["ITb"][:, :], bc(2))
        nc.vector.tensor_scalar_mul(ITn0[:, :], C["ITnb"][:, :], bc(1))

        def fwd_stageA(dst, pr_t, pi_t, s0, s1, li):
            for f in range(NFL):
                for sl in range(2):
                    ps = psp.tile([128, 512], F32, name=f"pA{li}_{f}_{sl}", tag="ps")
                    nc.tensor.matmul(ps[:, :], pr_t[:, 0, f, sl * 128:(sl + 1) * 128],
                                     s0[:, 0, :], start=True, stop=False)
                    nc.tensor.matmul(ps[:, :], pi_t[:, 0, f, sl * 128:(sl + 1) * 128],
                                     s1[:, 0, :], start=False, stop=False)
                    nc.tensor.matmul(ps[:, :], pr_t[:, 1, f, sl * 128:(sl + 1) * 128],
                                     s0[:, 1, :], start=False, stop=False)
                    nc.tensor.matmul(ps[:, :], pi_t[:, 1, f, sl * 128:(sl + 1) * 128],
                                     s1[:, 1, :], start=False, stop=True)
                    nc.scalar.copy(out=dst[:, sl, f, :], in_=ps[:, :])

        def stageB(ps_r, ps_i, Yt, m, fp, inv, stop=False):
            for c in range(2):
                wr = C["Br"][:, c, m * 128:(m + 1) * 128]
                wi = C["Bi"][:, c, m * 128:(m + 1) * 128]
                wn = C["Bn"][:, c, m * 128:(m + 1) * 128]
                rr = Yt[:, c, 2 * fp:2 * fp + 2, 0:256]
                ri = Yt[:, c, 2 * fp:2 * fp + 2, 256:512]
                la = (c == 1) and stop
                if not inv:
                    nc.tensor.matmul(ps_r, wr, rr, start=(c == 0), stop=False)
                    nc.tensor.matmul(ps_r, wn, ri, start=False, stop=la)
                    nc.tensor.matmul(ps_i, wi, rr, start=(c == 0), stop=False)
                    nc.tensor.matmul(ps_i, wr, ri, start=False, stop=la)
                else:
                    nc.tensor.matmul(ps_r, wr, rr, start=(c == 0), stop=False)
                    nc.tensor.matmul(ps_r, wi, ri, start=False, stop=la)
                    nc.tensor.matmul(ps_i, wn, rr, start=(c == 0), stop=False)
                    nc.tensor.matmul(ps_i, wr, ri, start=False, stop=la)

        # startpoint: x2 = u2 = ifft2(d-hat) via bf16 copy of d-hat (in u2 tiles)
        Qt0 = bigp.tile([128, 2, NFL, 512], BF16, name="Qt0", tag="big", bufs=1)
        fwd_stageA(Qt0, u2r, u2i, C["SI0"], C["SI1"], "ini")
        for m in range(2):
            for fp in range(4):
                pr = psp.tile([128, 512], F32, name=f"pi0r{m}{fp}", tag="ps")
                pi = psp.tile([128, 512], F32, name=f"pi0i{m}{fp}", tag="ps")
                stageB(pr[:, :], pi[:, :], Qt0, m, fp, inv=True, stop=True)
                sl2 = (slice(None), m, slice(2 * fp, 2 * fp + 2), slice(None))
                nc.scalar.activation(x2r[sl2], pr[:, :], AF.Copy, scale=-1.0 / TAU)
                nc.scalar.activation(x2i[sl2], pi[:, :], AF.Copy, scale=-1.0 / TAU)
                nc.vector.tensor_scalar_mul(u2r[sl2], pr[:, :], -1.0 / TAU)
                nc.vector.tensor_scalar_mul(u2i[sl2], pi[:, :], -1.0 / TAU)
        for t in (y1r, y1i, y2r, y2i):
            nc.vector.memset(t[:, :, :, :], 0.0)

        eng = nc.sync
        pid = eng.alloc_register("pid")
        eng.reg_load(pid, nc.partition_id_tensor[0:1, 0:1])
        r_up = eng.alloc_register("r_up")
        eng.reg_add(r_up, pid, 1)
        eng.reg_mod(r_up, r_up, NCORE)
        idx_up = eng.snap(r_up, donate=True, min_val=0, max_val=NCORE - 1)
        r_dn = eng.alloc_register("r_dn")
        eng.reg_add(r_dn, pid, NCORE - 1)
        eng.reg_mod(r_dn, r_dn, NCORE)
        idx_dn = eng.snap(r_dn, donate=True, min_val=0, max_val=NCORE - 1)

        def push_u2_halo():
            nc.sync.dma_start(out=hu_in[0, 0, :, :, :], in_=u2r[:, :, 0, :])
            nc.sync.dma_start(out=hu_in[0, 1, :, :, :], in_=u2i[:, :, 0, :])
            coll("AllGather", ALU.bypass, RG, [hu_in.ap()], [hu_out.ap()])
            nc.sync.dma_start(
                out=hu_r[:, :, :],
                in_=hu_out[bass.ds(idx_up, 1), 0, :, :, :].rearrange("o p c y -> (o p) c y"))
            nc.sync.dma_start(
                out=hu_i[:, :, :],
                in_=hu_out[bass.ds(idx_up, 1), 1, :, :, :].rearrange("o p c y -> (o p) c y"))

        push_u2_halo()

        # ---- block-complex helpers: [128,128] f32 tiles hold [[R,-I],[I,R]] ----
        wm = {}

        def newmat(tag, alias=None):
            key = alias or tag
            if key not in wm:
                wm[key] = wp.tile([128, 128], F32, name="wm_" + key, tag="wm_" + key)
            return wm[key]

        def cmm(dst, A, B, nm):
            ps = psw.tile([128, 128], F32, name="cm" + nm, tag="pw")
            nc.tensor.matmul(ps[:, :], A[:, :], B[:, :], start=True, stop=True)
            nc.scalar.copy(out=dst[:, :], in_=ps[:, :])
            return ps

        # ======================= layers =======================
        for li in range(n_layers):
            last = (li == n_layers - 1)

            # ---------- phase K: fwd fft(u2) + y1 update + A2A#1 ----------
            Yt = bigp.tile([128, 2, NFL, 512], BF16, name=f"Yt{li}", tag="big", bufs=1)
            fwd_stageA(Yt, u2r, u2i, C["SA0"], C["SA1"], f"f{li}")
            for m in range(2):
                for fp in range(4):
                    fsl = slice(2 * fp, 2 * fp + 2)
                    sl2 = (slice(None), m, fsl, slice(None))
                    pr = psp.tile([128, 512], F32, name=f"pk_r{li}{m}{fp}", tag="ps")
                    pi = psp.tile([128, 512], F32, name=f"pk_i{li}{m}{fp}", tag="ps")
                    stageB(pr[:, :], pi[:, :], Yt, m, fp, inv=False)
                    vkp_r = rp.tile([128, 2, N], BF16, name=f"vkr{li}{m}{fp}", tag="vkr", bufs=2)
                    vkp_i = rp.tile([128, 2, N], BF16, name=f"vki{li}{m}{fp}", tag="vki", bufs=2)
                    for jf, fg in enumerate(range(2 * fp, 2 * fp + 2)):
                        nc.sync.dma_start(
                            out=vkp_r[:, jf, :],
                            in_=b2_out[4 * m:4 * m + 4, 0, fg, :, :, :]
                                .rearrange("s b k y -> s (b k) y"))
                        nc.sync.dma_start(
                            out=vkp_i[:, jf, :],
                            in_=b2_out[4 * m:4 * m + 4, 1, fg, :, :, :]
                                .rearrange("s b k y -> s (b k) y"))
                    nc.tensor.matmul(pr[:, :], C["Ic1"][:, :], y1r[sl2], start=False, stop=False)
                    nc.tensor.matmul(pr[:, :], C["I1b"][:, :], vkp_r[:, :, :], start=False, stop=False)
                    nc.tensor.matmul(pr[:, :], C["I1b"][:, :], dhs_r[sl2], start=False, stop=True)
                    nc.tensor.matmul(pi[:, :], C["Ic1"][:, :], y1i[sl2], start=False, stop=False)
                    nc.tensor.matmul(pi[:, :], C["I1b"][:, :], vkp_i[:, :, :], start=False, stop=False)
                    nc.tensor.matmul(pi[:, :], C["I1b"][:, :], dhs_i[sl2], start=False, stop=True)
                    nc.vector.tensor_copy(y1r[sl2], pr[:, :])
                    nc.vector.tensor_copy(y1i[sl2], pi[:, :])
                    for jf, fg in enumerate(range(2 * fp, 2 * fp + 2)):
                        nc.sync.dma_start(
                            out=b1_in[4 * m:4 * m + 4, 0, fg, :, :],
                            in_=y1r[:, m, fg, :])
                        nc.sync.dma_start(
                            out=b1_in[4 * m:4 * m + 4, 1, fg, :, :],
                            in_=y1i[:, m, fg, :])
            coll("AllToAll", ALU.bypass, RG, [b1_in.ap()], [b1_out.ap()])

            # ---------- image branch: y2 soft-threshold update ----------
            lnb = stp.tile([128, 1], F32, name=f"lnb{li}", tag="lnb")
            nc.scalar.activation(lnb[:, 0:1], scal[:, li:li + 1], AF.Ln)
            for m in range(2):
                for fp in range(4):
                    fsl = slice(2 * fp, 2 * fp + 2)
                    sl2 = (slice(None), m, fsl, slice(None))
                    af_r = rp.tile([128, 2, N], BF16, name=f"af_r{li}{m}{fp}", tag="afr", bufs=1)
                    af_i = rp.tile([128, 2, N], BF16, name=f"af_i{li}{m}{fp}", tag="afi", bufs=1)
                    mg = rp.tile([128, 2, N], F32, name=f"mg{li}{m}{fp}", tag="mg", bufs=1)
                    mg2 = psm.tile([128, 512], F32, name=f"mg2{li}{m}{fp}", tag="psm")
                    if fp < 3:
                        nc.vector.tensor_sub(af_r[:, :, :], u2r[:, m, 2 * fp + 1:2 * fp + 3, :], u2r[sl2])
                        nc.vector.tensor_sub(af_i[:, :, :], u2i[:, m, 2 * fp + 1:2 * fp + 3, :], u2i[sl2])
                        nc.vector.scalar_tensor_tensor(af_r[:, :, :], af_r[:, :, :], SIGMA,
                                                       y2r[sl2], op0=ALU.mult, op1=ALU.add)
                        nc.vector.scalar_tensor_tensor(af_i[:, :, :], af_i[:, :, :], SIGMA,
                                                       y2i[sl2], op0=ALU.mult, op1=ALU.add)
                    else:
                        nc.vector.tensor_sub(af_r[:, 0, :], u2r[:, m, 7, :], u2r[:, m, 6, :])
                        nc.vector.tensor_sub(af_i[:, 0, :], u2i[:, m, 7, :], u2i[:, m, 6, :])
                        nc.vector.tensor_sub(af_r[:, 1, :], hu_r[:, m, :], u2r[:, m, 7, :])
                        nc.vector.tensor_sub(af_i[:, 1, :], hu_i[:, m, :], u2i[:, m, 7, :])
                        nc.vector.scalar_tensor_tensor(af_r[:, 0, :], af_r[:, 0, :], SIGMA,
                                                       y2r[:, m, 6, :], op0=ALU.mult, op1=ALU.add)
                        nc.vector.scalar_tensor_tensor(af_i[:, 0, :], af_i[:, 0, :], SIGMA,
                                                       y2i[:, m, 6, :], op0=ALU.mult, op1=ALU.add)
                        nc.vector.scalar_tensor_tensor(af_r[:, 1, :], af_r[:, 1, :], bc(3),
                                                       y2r[:, m, 7, :], op0=ALU.mult, op1=ALU.add)
                        nc.vector.scalar_tensor_tensor(af_i[:, 1, :], af_i[:, 1, :], bc(3),
                                                       y2i[:, m, 7, :], op0=ALU.mult, op1=ALU.add)
                    # y2 = af * min(1, lamS / |af|), via exp(ln(lamS) - 0.5*ln(|af|^2))
                    nc.vector.tensor_mul(mg[:, :, :], af_r[:, :, :], af_r[:, :, :])
                    nc.scalar.activation(mg2[:, :], af_i[:, :, :], AF.Square)
                    nc.vector.tensor_add(mg[:, :, :].rearrange("p a b -> p (a b)"),
                                         mg[:, :, :].rearrange("p a b -> p (a b)"), mg2[:, :])
                    nc.scalar.activation(mg[:, :, :], mg[:, :, :], AF.Ln)
                    nc.scalar.activation(mg[:, :, :], mg[:, :, :], AF.Exp,
                                         bias=lnb[:, 0:1], scale=-0.5)
                    nc.vector.tensor_scalar_min(mg[:, :, :], mg[:, :, :], 1.0)
                    nc.vector.tensor_mul(y2r[sl2], af_r[:, :, :], mg[:, :, :])
                    nc.vector.tensor_mul(y2i[sl2], af_i[:, :, :], mg[:, :, :])
            # y2 halo AG
            nc.sync.dma_start(out=hy_in[0, 0, :, :, :], in_=y2r[:, :, 7, :])
            nc.sync.dma_start(out=hy_in[0, 1, :, :, :], in_=y2i[:, :, 7, :])
            coll("AllGather", ALU.bypass, RG, [hy_in.ap()], [hy_out.ap()])
            nc.sync.dma_start(
                out=hy_r[:, :, :],
                in_=hy_out[bass.ds(idx_dn, 1), 0, :, :, :].rearrange("o p c y -> (o p) c y"))
            nc.sync.dma_start(
                out=hy_i[:, :, :],
                in_=hy_out[bass.ds(idx_dn, 1), 1, :, :, :].rearrange("o p c y -> (o p) c y"))

            # ---------- pixel side: deposit, argg1k, Gram ----------
            for p, yt in ((0, y1p_r), (1, y1p_i)):
                for b in range(2):
                    for s_ in range(NCORE):
                        nc.sync.dma_start(
                            out=yt[64 * b + 8 * s_:64 * b + 8 * s_ + 8, :],
                            in_=b1_out[s_, p, :, 16 * b:16 * (b + 1), :]
                                .rearrange("f k y -> f (k y)"))
            nc.vector.scalar_tensor_tensor(y1p_r[:, :], y1p_r[:, :], -TAU, xpr[:, :],
                                           op0=ALU.mult, op1=ALU.add)
            nc.vector.scalar_tensor_tensor(y1p_i[:, :], y1p_i[:, :], -TAU, xpi[:, :],
                                           op0=ALU.mult, op1=ALU.add)
            psG = psw.tile([64, 128], F32, name=f"psG{li}", tag="pw")
            for k in range(32):
                ks = slice(128 * k, 128 * (k + 1))
                psT = psp.tile([128, 512], BF16, name=f"psT{li}_{k}", tag="ps")
                nc.tensor.transpose(psT[:, 0:128], y1p_r[:, ks], C["I128b"][:, :])
                nc.tensor.transpose(psT[:, 128:256], y1p_i[:, ks], C["I128b"][:, :])
                Tk = tkp.tile([128, 3, 128], BF16, name=f"Tk{li}_{k}", tag="Tk")
                nc.vector.tensor_copy(Tk[:, 0:2, :], psT[:, 0:256])
                nc.vector.tensor_scalar_mul(Tk[:, 2, :], psT[:, 0:128], -1.0)
                for b in range(2):
                    bs = slice(64 * b, 64 * (b + 1))
                    nc.tensor.matmul(psG[:, :], Tk[:, 0, bs], Tk[:, 0:2, bs],
                                     start=(k == 0 and b == 0), stop=False)
                    nc.tensor.matmul(psG[:, :], Tk[:, 1, bs], Tk[:, 1:3, bs],
                                     start=False, stop=(k == 31 and b == 1))
            gl = stp.tile([64, 128], F32, name=f"gl{li}", tag="gl")
            nc.vector.tensor_copy(gl[:, :], psG[:, :])
            nc.sync.dma_start(out=g_in[:, :], in_=gl[:, :])
            coll("AllReduce", ALU.add, RG, [g_in.ap()], [g_out.ap()])
            glr = stp.tile([64, 128], F32, name=f"glr{li}", tag="glr")
            nc.sync.dma_start(out=glr[:, :], in_=g_out[:, :])
            # block-complex G: [[R, -I], [I, R]]
            G = newmat("G")
            nc.vector.tensor_copy(G[0:64, 0:64], glr[:, 0:64])
            nc.vector.tensor_copy(G[64:128, 64:128], glr[:, 0:64])
            nc.vector.tensor_copy(G[64:128, 0:64], glr[:, 64:128])
            nc.vector.tensor_scalar_mul(G[0:64, 64:128], glr[:, 64:128], -1.0)

            # ---------- W chain (block-complex) ----------
            trs = stp.tile([64, 1], F32, name=f"trs{li}", tag="trs")
            scrap = stp.tile([64, 64], F32, name=f"scrap{li}", tag="scrap")
            lg = stp.tile([1, 4], F32, name=f"lg{li}", tag="lg")
            rtr = stp.tile([1, 1], F32, name=f"rtr{li}", tag="rtr")
            bres = stp.tile([128, 1], F32, name=f"bres{li}", tag="bres")
            psb = psw.tile([128, 8], F32, name=f"psb{li}", tag="pw")
            pst = psw.tile([1, 8], F32, name=f"pst{li}", tag="pw")

            def trace_of(Mt, dstcol, li=li, scrap=scrap, trs=trs, pst=pst):
                nc.vector.scalar_tensor_tensor(scrap[:, :], Mt[0:64, 0:64], 1.0,
                                               C["I64"][:, :], op0=ALU.mult,
                                               op1=ALU.mult, accum_out=trs[:, 0:1])
                nc.tensor.matmul(pst[0:1, dstcol:dstcol + 1], trs[:, :], on64[:, :],
                                 start=True, stop=True)

            def bcast128(src_ap, dst, col, psb=psb):
                nc.tensor.matmul(psb[:, col:col + 1], onr[0:1, :], src_ap,
                                 start=True, stop=True)
                nc.vector.tensor_copy(dst[:, 0:1], psb[:, col:col + 1])

            Bm = newmat("Bm")
            B2 = newmat("B2")
            trace_of(G, 0)
            nc.vector.tensor_copy(lg[0:1, 0:1], pst[0:1, 0:1])
            nc.scalar.activation(lg[0:1, 1:2], lg[0:1, 0:1], AF.Ln)
            nc.vector.reciprocal(rtr[0:1, :], lg[0:1, 0:1])
            bcast128(rtr[0:1, :], bres, 0)
            nc.vector.tensor_scalar_mul(Bm[:, :], G[:, :], bres[:, 0:1])
            for it in range(M_POWER):
                cmm(B2, Bm, Bm, f"q{li}_{it}")
                trace_of(B2, 1)
                nc.vector.tensor_copy(lg[0:1, 2:3], pst[0:1, 1:2])
                nc.scalar.activation(lg[0:1, 3:4], lg[0:1, 2:3], AF.Ln)
                nc.vector.tensor_scalar(lg[0:1, 1:2], lg[0:1, 1:2], 2.0, None, op0=ALU.mult)
                nc.vector.tensor_add(lg[0:1, 1:2], lg[0:1, 1:2], lg[0:1, 3:4])
                nc.vector.reciprocal(rtr[0:1, :], lg[0:1, 2:3])
                bcast128(rtr[0:1, :], bres, 1)
                nc.vector.tensor_scalar_mul(Bm[:, :], B2[:, :], bres[:, 0:1])
            lam_s = stp.tile([1, 1], F32, name=f"lam_s{li}", tag="lam_s")
            nc.scalar.activation(lam_s[0:1, :], lg[0:1, 1:2], AF.Exp,
                                 scale=1.0 / (2 ** M_POWER))
            ilam = stp.tile([128, 1], F32, name=f"ilam{li}", tag="ilam")
            nc.vector.reciprocal(rtr[0:1, :], lam_s[0:1, :])
            bcast128(rtr[0:1, :], ilam, 2)
            Gh = newmat("Gh")
            nc.vector.tensor_scalar_mul(Gh[:, :], G[:, :], ilam[:, 0:1])
            t2s = stp.tile([1, 1], F32, name=f"t2s{li}", tag="t2s")
            nc.scalar.activation(t2s[0:1, :], scal[0:1, 8 + li:8 + li + 1],
                                 AF.Square, scale=TAU)
            ths = stp.tile([1, 1], F32, name=f"ths{li}", tag="ths")
            nc.vector.tensor_scalar_mul(ths[0:1, 0:1],
                                        scal[0:1, 8 + li:8 + li + 1], TAU)
            onem = stp.tile([1, 1], F32, name=f"onem{li}", tag="onem")
            nc.vector.tensor_scalar(onem[0:1, :], t2s[0:1, :], -1.0, 1.0,
                                    op0=ALU.mult, op1=ALU.add)
            nc.vector.reciprocal(onem[0:1, :], onem[0:1, :])
            i1m = stp.tile([128, 1], F32, name=f"i1m{li}", tag="i1m")
            bcast128(onem[0:1, :], i1m, 3)
            nt2 = stp.tile([128, 1], F32, name=f"nt2{li}", tag="nt2")
            bcast128(t2s[0:1, :], nt2, 4)
            nc.vector.tensor_scalar_mul(nt2[:, 0:1], nt2[:, 0:1], -1.0)
            X = newmat("X")
            nc.vector.scalar_tensor_tensor(X[:, :], C["I128f"][:, :], nt2[:, 0:1],
                                           Gh[:, :], op0=ALU.mult, op1=ALU.add)
            nc.vector.tensor_scalar_mul(X[:, :], X[:, :], i1m[:, 0:1])
            X2 = newmat("X2", alias="Bm")
            X4 = newmat("X4", alias="B2")
            Yp = newmat("Yp", alias="B2")
            for k_, (a_, b_, c_) in enumerate(SIGN_COEFFS):
                cmm(X2, X, X, f"s2_{li}_{k_}")
                cmm(X4, X2, X2, f"s4_{li}_{k_}")
                nc.vector.tensor_scalar_mul(Yp[:, :], X4[:, :], c_)
                nc.vector.scalar_tensor_tensor(Yp[:, :], X2[:, :], b_,
                                               Yp[:, :], op0=ALU.mult, op1=ALU.add)
                nc.vector.scalar_tensor_tensor(Yp[:, :], C["I128f"][:, :], a_,
                                               Yp[:, :], op0=ALU.mult, op1=ALU.add)
                cmm(X, X, Yp, f"sx_{li}_{k_}")
            P = newmat("P", alias="Bm")
            nc.vector.tensor_scalar_mul(P[:, :], X[:, :], 0.5)
            nc.vector.scalar_tensor_tensor(P[:, :], C["I128f"][:, :], 0.5,
                                           P[:, :], op0=ALU.mult, op1=ALU.add)
            T = newmat("T")
            nc.vector.tensor_scalar_mul(T[:, :], Gh[:, :], 2.0 / Q_HI)
            nc.vector.scalar_tensor_tensor(T[:, :], C["I128f"][:, :], -1.0,
                                           T[:, :], op0=ALU.mult, op1=ALU.add)
            b1m = newmat("b1m", alias="G")
            b2m = newmat("b2m", alias="X")
            tm = newmat("tm", alias="B2")
            nc.vector.memset(b1m[:, :], 0.0)
            nc.vector.memset(b2m[:, :], 0.0)
            mats = [b1m, b2m, tm]
            for ci_idx, ci in enumerate(Q_COEF[::-1][:-1]):
                bb1, bb2, tt = mats
                cmm(tt, T, bb1, f"cl{li}_{ci_idx}")
                nc.vector.scalar_tensor_tensor(tt[:, :], tt[:, :], 2.0,
                                               bb2[:, :], op0=ALU.mult,
                                               op1=ALU.subtract)
                nc.vector.scalar_tensor_tensor(tt[:, :], C["I128f"][:, :], ci,
                                               tt[:, :], op0=ALU.mult, op1=ALU.add)
                mats = [tt, bb1, bb2]
            bb1, bb2, _ = mats
            Q = newmat("Q", alias="Gh")
            cmm(Q, T, bb1, f"qf{li}")
            nc.vector.tensor_sub(Q[:, :], Q[:, :], bb2[:, :])
            nc.vector.scalar_tensor_tensor(Q[:, :], C["I128f"][:, :], Q_COEF[0],
                                           Q[:, :], op0=ALU.mult, op1=ALU.add)
            PQ = newmat("PQ", alias="X")
            cmm(PQ, P, Q, f"pq{li}")
            Wt = newmat("Wt", alias="B2")
            nth = stp.tile([128, 1], F32, name=f"nth{li}", tag="nth")
            bcast128(ths[0:1, 0:1], nth, 5)
            nc.vector.tensor_scalar_mul(nth[:, 0:1], nth[:, 0:1], -1.0)
            nc.vector.scalar_tensor_tensor(Wt[:, :], PQ[:, :], nth[:, 0:1],
                                           P[:, :], op0=ALU.mult, op1=ALU.add)
            Wb = wp.tile([128, 128], BF16, name=f"Wb{li}", tag="Wb")
            Wn = wp.tile([128, 128], BF16, name=f"Wn{li}", tag="Wn")
            nc.vector.tensor_copy(Wb[:, :], Wt[:, :])
            nc.vector.tensor_scalar_mul(Wn[:, :], Wt[:, :], -1.0)

            # ---------- recon + V + A2A#2 (merged b halves) ----------
            # block W quadrants: Wb = [[Wr, -Wi], [Wi, Wr]], Wn = -Wb.
            # b=0 rows (parts 0:64): Wr=Wb[0:64,0:64], -Wi=Wb[0:64,64:128], Wi=Wn[0:64,64:128]
            # b=1 rows (parts 64:128): Wr=Wb[64:128,64:128], Wi=Wb[64:128,0:64], -Wi=Wn[64:128,0:64]
            for ch in range(8):
                cs = slice(512 * ch, 512 * (ch + 1))
                pR = psp.tile([128, 512], F32, name=f"pR{li}_{ch}", tag="ps")
                pI = psp.tile([128, 512], F32, name=f"pI{li}_{ch}", tag="ps")
                b0, b1s = slice(0, 64), slice(64, 128)
                nc.tensor.matmul(pR[b0, :], Wb[b0, 0:64], y1p_r[b0, cs], start=True, stop=False)
                nc.tensor.matmul(pR[b0, :], Wb[b0, 64:128], y1p_i[b0, cs], start=False, stop=True)
                nc.tensor.matmul(pR[b1s, :], Wb[b1s, 64:128], y1p_r[b1s, cs], start=True, stop=False)
                nc.tensor.matmul(pR[b1s, :], Wn[b1s, 0:64], y1p_i[b1s, cs], start=False, stop=True)
                nc.tensor.matmul(pI[b0, :], Wb[b0, 0:64], y1p_i[b0, cs], start=True, stop=False)
                nc.tensor.matmul(pI[b0, :], Wn[b0, 64:128], y1p_r[b0, cs], start=False, stop=True)
                nc.tensor.matmul(pI[b1s, :], Wb[b1s, 64:128], y1p_i[b1s, cs], start=True, stop=False)
                nc.tensor.matmul(pI[b1s, :], Wb[b1s, 0:64], y1p_r[b1s, cs], start=False, stop=True)
                vst_r = rp.tile([128, 512], BF16, name=f"v_r{li}{ch}", tag="vsr")
                vst_i = rp.tile([128, 512], BF16, name=f"v_i{li}{ch}", tag="vsi")
                if not last:
                    nc.vector.scalar_tensor_tensor(vst_r[:, :], xpr[:, cs], -0.5,
                                                   pR[:, :], op0=ALU.mult, op1=ALU.add)
                    nc.vector.tensor_scalar_mul(vst_r[:, :], vst_r[:, :], 2.0 * C2)
                    nc.vector.scalar_tensor_tensor(vst_i[:, :], xpi[:, cs], -0.5,
                                                   pI[:, :], op0=ALU.mult, op1=ALU.add)
                    nc.vector.tensor_scalar_mul(vst_i[:, :], vst_i[:, :], 2.0 * C2)
                else:
                    nc.vector.tensor_copy(vst_r[:, :], pR[:, :])
                    nc.vector.tensor_copy(vst_i[:, :], pI[:, :])
                nc.vector.tensor_copy(xpr[:, cs], pR[:, :])
                nc.vector.tensor_copy(xpi[:, cs], pI[:, :])
                for b in range(2):
                    bsl = slice(64 * b, 64 * (b + 1))
                    nc.sync.dma_start(
                        out=b2_in[:, 0, :, b, :, :]
                            .rearrange("d f k y -> d f (k y)")[:, :, cs],
                        in_=vst_r[bsl, :])
                    nc.sync.dma_start(
                        out=b2_in[:, 1, :, b, :, :]
                            .rearrange("d f k y -> d f (k y)")[:, :, cs],
                        in_=vst_i[bsl, :])
            coll("AllToAll", ALU.bypass, RG, [b2_in.ap()], [b2_out.ap()])

            # ---------- image branch: ifft(y1) + x2/u2 update ----------
            Qt = bigp.tile([128, 2, NFL, 512], BF16, name=f"Qt{li}", tag="big", bufs=1)
            fwd_stageA(Qt, y1r, y1i, C["SI0"], C["SI1"], f"i{li}")
            for m in range(2):
                for fp in [1, 2, 3, 0]:
                    fsl = slice(2 * fp, 2 * fp + 2)
                    sl2 = (slice(None), m, fsl, slice(None))
                    pr = psp.tile([128, 512], F32, name=f"pm_r{li}{m}{fp}", tag="ps")
                    pi = psp.tile([128, 512], F32, name=f"pm_i{li}{m}{fp}", tag="ps")
                    stageB(pr[:, :], pi[:, :], Qt, m, fp, inv=True)
                    for ppp, y2t, x2t, hyt in ((pr, y2r, x2r, hy_r), (pi, y2i, x2i, hy_i)):
                        nc.tensor.matmul(ppp[:, :], C["I1b"][:, :], x2t[sl2],
                                         start=False, stop=False)
                        if fp == 3:
                            nc.tensor.matmul(ppp[:, 0:256], C["ITb"][:, :],
                                             y2t[:, m, 6, :], start=False, stop=False)
                            nc.tensor.matmul(ppp[:, 256:512], IT7[:, :],
                                             y2t[:, m, 7, :], start=False, stop=False)
                        else:
                            nc.tensor.matmul(ppp[:, :], C["ITb"][:, :], y2t[sl2],
                                             start=False, stop=False)
                        if fp == 0:
                            nc.tensor.matmul(ppp[:, 0:256], ITn0[:, :], hyt[:, m, :],
                                             start=False, stop=False)
                            nc.tensor.matmul(ppp[:, 256:512], C["ITnb"][:, :],
                                             y2t[:, m, 0, :], start=False, stop=True)
                        else:
                            nc.tensor.matmul(ppp[:, :], C["ITnb"][:, :],
                                             y2t[:, m, 2 * fp - 1:2 * fp + 1, :],
                                             start=False, stop=True)
                    if not last:
                        nc.vector.scalar_tensor_tensor(u2r[sl2], x2r[sl2], -0.5, pr[:, :],
                                                       op0=ALU.mult, op1=ALU.add)
                        nc.vector.tensor_scalar_mul(u2r[sl2], u2r[sl2], 2.0)
                        nc.vector.scalar_tensor_tensor(u2i[sl2], x2i[sl2], -0.5, pi[:, :],
                                                       op0=ALU.mult, op1=ALU.add)
                        nc.vector.tensor_scalar_mul(u2i[sl2], u2i[sl2], 2.0)
                    nc.vector.tensor_copy(x2r[sl2], pr[:, :])
                    nc.vector.tensor_copy(x2i[sl2], pi[:, :])
            if not last:
                push_u2_halo()

        # ---------------- final ----------------
        xfr = bigp.tile([128, 2, NFL, N], BF16, name="xfr", tag="bigs", bufs=2)
        xfi = bigp.tile([128, 2, NFL, N], BF16, name="xfi", tag="bigs", bufs=2)
        for p, xt in ((0, xfr), (1, xfi)):
            for m in range(2):
                for f in range(NFL):
                    nc.sync.dma_start(
                        out=xt[:, m, f, :],
                        in_=b2_out[4 * m:4 * m + 4, p, f, :, :, :]
                            .rearrange("s b k y -> s (b k) y"))
        Qtf = bigp.tile([128, 2, NFL, 512], BF16, name="Qtf", tag="big", bufs=1)
        fwd_stageA(Qtf, xfr, xfi, C["SI0"], C["SI1"], "fin")
        for m in range(2):
            for fp in range(4):
                pr = psp.tile([128, 512], F32, name=f"pf_r{m}{fp}", tag="ps")
                pi = psp.tile([128, 512], F32, name=f"pf_i{m}{fp}", tag="ps")
                stageB(pr[:, :], pi[:, :], Qtf, m, fp, inv=True, stop=True)
                sl2 = (slice(None), m, slice(2 * fp, 2 * fp + 2), slice(None))
                op_r = bigp.tile([128, 2, N], F32, name=f"op_r{m}{fp}", tag="bigs", bufs=2)
                op_i = bigp.tile([128, 2, N], F32, name=f"op_i{m}{fp}", tag="bigs", bufs=2)
                nc.vector.scalar_tensor_tensor(op_r[:, :, :], pr[:, :], -1.0 / TAU,
                                               x2r[sl2], op0=ALU.mult, op1=ALU.add)
                nc.vector.scalar_tensor_tensor(op_i[:, :, :], pi[:, :], -1.0 / TAU,
                                               x2i[sl2], op0=ALU.mult, op1=ALU.add)
                nc.sync.dma_start(
                    out=out[:, 2048 * m + 512 * fp:2048 * m + 512 * fp + 512],
                    in_=op_r[:, :, :])
                nc.sync.dma_start(
                    out=out[:, 4096 + 2048 * m + 512 * fp:4096 + 2048 * m + 512 * fp + 512],
                    in_=op_i[:, :, :])

        stack.close()

    nc.compile()
    return nc


_CACHE = {}


def _get_nc(n_layers=NLAYERS):
    if n_layers not in _CACHE:
        _CACHE[n_layers] = build(n_layers)
    return _CACHE[n_layers]


def host_shard(d_real, d_imag, lambdaS, lambdaL):
    d_r = np.asarray(d_real, np.float32).reshape(NF, N, N)
    d_i = np.asarray(d_imag, np.float32).reshape(NF, N, N)
    dTr = d_r.transpose(0, 2, 1)
    dTi = d_i.transpose(0, 2, 1)
    lamS = np.asarray(lambdaS, np.float32).reshape(NLAYERS)
    lamL = np.asarray(lambdaL, np.float32).reshape(NLAYERS)
    in_maps = []
    for c in range(NCORE):
        fr = slice(8 * c, 8 * c + 8)
        dk_rc = dTr[fr].reshape(NFL, 2, 128, N).transpose(2, 1, 0, 3).reshape(128, 4096)
        dk_ic = dTi[fr].reshape(NFL, 2, 128, N).transpose(2, 1, 0, 3).reshape(128, 4096)
        m0 = 0.0 if c == 0 else 1.0
        m7 = 0.0 if c == NCORE - 1 else 1.0
        srow = np.zeros(32, np.float32)
        srow[0:8] = lamS
        srow[8:16] = lamL
        srow[16] = m0
        srow[17] = m7
        xin = np.empty((128, W_IN), np.float32)
        xin[:, 0:4096] = dk_rc
        xin[:, 4096:8192] = dk_ic
        xin[:, 8192:] = srow[None, :]
        in_maps.append({"xin": xin})
    return in_maps


def host_gather(results):
    full = np.zeros((NF, N, N), np.complex64)
    for c, res in enumerate(results):
        o = res["out"]
        img = (o[:, 0:4096] + 1j * o[:, 4096:8192]).astype(np.complex64)
        img = img.reshape(128, 2, NFL, N)
        full[8 * c:8 * c + 8] = img.transpose(2, 1, 0, 3).reshape(NFL, N, N)
    return full.reshape(1, 1, NF, N, N)


def kernel(d_real, d_imag, lambdaS, lambdaL):
    nc = _get_nc()
    in_maps = host_shard(d_real, d_imag, lambdaS, lambdaL)
    res = bass_utils.run_bass_kernel_spmd(nc, in_maps, core_ids=list(range(NCORE)))
    return host_gather(res.results)


# revision 25
# speedup vs baseline: 13.6447x; 1.3210x over previous
"""TRN2 Bass kernel: 8-layer Chambolle-Pock MRI reconstruction on 8 NeuronCores.

Sharding: frames (8/core) for FFTs + elementwise; k-space rows (8192 px/core)
for the low-rank prox (Gram trick + matmul-only spectral filter, no eigensolver).
Cross-core per layer: AllToAll (y1 frame->pixel), AllToAll (x1k V pixel->frame),
AllReduce (64x64 Gram), 2 small AllGathers (temporal halo).

v2: single packed input/output tensor (cuts per-call dispatch cost), pixel-side
init derived on device via an extra A2A (drops the dp input), d-hat held in SBUF
pre-scaled (no per-layer HBM reloads), bf16 DFT intermediates, rsqrt-based
soft-threshold, block-complex (128x128 real) W chain, merged-b recon PSUM,
and multi-dim-AP batched DMAs.
"""
import numpy as np

import concourse.bass as bass
import concourse.bacc as bacc
import concourse.mybir as mybir
import concourse.tile as tile
from concourse import bass_utils

F32 = mybir.dt.float32
BF16 = mybir.dt.bfloat16
AF = mybir.ActivationFunctionType
ALU = mybir.AluOpType

NCORE = 8
NF = 64
NFL = 8
N = 256
NLAYERS = 8
SIGMA = float(np.float32(1.0 / np.sqrt(8.0)))
TAU = SIGMA
C1 = float(np.float32(1.0 / (1.0 + SIGMA)))
C2 = C1 * SIGMA
M_POWER = 8
SIGN_COEFFS = [(3.4445, -4.7750, 2.0315)] * 7 + [(1.875, -1.25, 0.375)] * 3
RG = [list(range(NCORE))]

W_IN = 2 * 4096 + 32
W_OUT = 2 * 4096


def _fit_q(deg=14, lo=0.068, hi=1.05, npts=6000):
    xs_in = np.linspace(lo, hi, npts)
    xs_out = np.linspace(0, 0.05, 300)

    def cheb(x, d, b=hi):
        t = (2 * x - b) / b
        V = np.zeros((len(x), d + 1))
        V[:, 0] = 1
        if d >= 1:
            V[:, 1] = t
        for k in range(2, d + 1):
            V[:, k] = 2 * t * V[:, k - 1] - V[:, k - 2]
        return V

    Vi = cheb(xs_in, deg)
    Vo = cheb(xs_out, deg)
    A = np.vstack([np.sqrt(xs_in)[:, None] * Vi, 1e-5 * Vo])
    y = np.concatenate([np.ones(npts), np.zeros(len(xs_out))])
    coef, *_ = np.linalg.lstsq(A, y, rcond=None)
    return [float(c) for c in coef], hi


Q_COEF, Q_HI = _fit_q()


def _dft():
    k = np.arange(N)
    W = np.exp(-2j * np.pi * np.outer(k, k) / N)
    return W.real.astype(np.float32), W.imag.astype(np.float32)


def _chunk(a):
    return np.ascontiguousarray(np.stack([a[0:128], a[128:256]], axis=1))


def build(n_layers=NLAYERS, single_core=False):
    nc = bacc.Bacc("TRN2", target_bir_lowering=False, debug=False,
                   num_devices=1 if single_core else NCORE)

    xin = nc.dram_tensor("xin", [128, W_IN], BF16, kind="ExternalInput")
    cmeta_u = nc.dram_tensor("cmeta_u", [1, 2], mybir.dt.uint32, kind="ExternalInput")
    out = nc.dram_tensor("out", [128, W_OUT], BF16, kind="ExternalOutput")

    Fr, Fi = _dft()
    sA = C1 * SIGMA / N
    sI = -TAU / N
    id128 = np.eye(128, dtype=np.float32)
    consts_np = {
        "SA0": (_chunk(np.concatenate([Fr * sA, Fi * sA], 1)), BF16),
        "SA1": (_chunk(np.concatenate([-Fi * sA, Fr * sA], 1)), BF16),
        "SI0": (_chunk(np.concatenate([Fr * sI, -Fi * sI], 1)), BF16),
        "SI1": (_chunk(np.concatenate([Fi * sI, Fr * sI], 1)), BF16),
        "Br": (_chunk(Fr), BF16),
        "Bi": (_chunk(Fi), BF16),
        "Bn": (_chunk(-Fi), BF16),
        "I128b": (id128, BF16),
        "Ic1": (id128 * C1, BF16),
        "I1b": (id128, BF16),
        "ITb": (id128 * TAU, BF16),
        "ITnb": (id128 * (-TAU), BF16),
        "I128f": (id128, F32),
        "I64": (np.eye(64, dtype=np.float32), F32),
    }
    handles = {}
    for k, (v, dt) in consts_np.items():
        handles[k] = nc.inline_tensor(v.astype(mybir.dt.np(dt)), name="c" + k)

    b1a_in = nc.dram_tensor("b1a_in", [NCORE, 2, NFL // 2, 32, N], BF16)
    b1a_out = nc.dram_tensor("b1a_out", [NCORE, 2, NFL // 2, 32, N], BF16)
    b1b_in = nc.dram_tensor("b1b_in", [NCORE, 2, NFL // 2, 32, N], BF16)
    b1b_out = nc.dram_tensor("b1b_out", [NCORE, 2, NFL // 2, 32, N], BF16)
    b2_in = nc.dram_tensor("b2_in", [NCORE, 2, NFL, 2, 16, N], BF16)
    b2_out = nc.dram_tensor("b2_out", [NCORE, 2, NFL, 2, 16, N], BF16)
    g_in = nc.dram_tensor("g_in", [64, 128], F32)
    g_out = nc.dram_tensor("g_out", [64, 128], F32, addr_space="Shared")
    nrm_in = nc.dram_tensor("nrm_in", [1, 1], F32)
    nrm_out = nc.dram_tensor("nrm_out", [1, 1], F32, addr_space="Shared")
    hu_in = nc.dram_tensor("hu_in", [1, 2, 128, 2, N], BF16)
    hu_out = nc.dram_tensor("hu_out", [NCORE, 2, 128, 2, N], BF16, addr_space="Shared")
    hy_in = nc.dram_tensor("hy_in", [1, 2, 128, 2, N], BF16)
    hy_out = nc.dram_tensor("hy_out", [NCORE, 2, 128, 2, N], BF16, addr_space="Shared")

    with tile.TileContext(nc) as tc:
        import contextlib
        stack = contextlib.ExitStack()

        def pool(name, bufs, space="SBUF"):
            return stack.enter_context(tc.tile_pool(name=name, bufs=bufs, space=space))

        cp = pool("cp", 1)
        sbp = pool("sbp", 1)
        stp = pool("stp", 1)
        rp = pool("rp", 2)
        bigp = pool("bigp", 1)
        tkp = pool("tkp", 2)
        wp = pool("wp", 1)
        psp = pool("psp", 4, space="PSUM")
        psm = pool("psm", 1, space="PSUM")
        psw = pool("psw", 3, space="PSUM")

        def coll(kind, op, replica_groups, ins, outs):
            if single_core:
                nc.sync.dma_start(out=outs[0], in_=ins[0]) if ins[0].size() == outs[0].size() \
                    else nc.sync.dma_start(out=outs[0][0:1], in_=ins[0][0:1])
            else:
                nc.gpsimd.collective_compute(kind, op, replica_groups=replica_groups,
                                             ins=ins, outs=outs)

        C = {}
        for k, (v, dt) in consts_np.items():
            t = cp.tile(list(v.shape), dt, name="k" + k, tag="k" + k)
            if v.ndim == 2:
                nc.sync.dma_start(out=t[:, :], in_=handles[k][:, :])
            else:
                nc.sync.dma_start(out=t[:, :, :], in_=handles[k][:, :, :])
            C[k] = t

        y1r = sbp.tile([128, 2, NFL, N], BF16, name="y1r", tag="y1r")
        y1i = sbp.tile([128, 2, NFL, N], BF16, name="y1i", tag="y1i")
        u2r = sbp.tile([128, 2, NFL, N], BF16, name="u2r", tag="u2r")
        u2i = sbp.tile([128, 2, NFL, N], BF16, name="u2i", tag="u2i")
        y2r = sbp.tile([128, 2, NFL, N], BF16, name="y2r", tag="y2r")
        y2i = sbp.tile([128, 2, NFL, N], BF16, name="y2i", tag="y2i")
        x2r = sbp.tile([128, 2, NFL, N], BF16, name="x2r", tag="x2r")
        x2i = sbp.tile([128, 2, NFL, N], BF16, name="x2i", tag="x2i")
        dhs_r = sbp.tile([128, 2, NFL, N], BF16, name="dhs_r", tag="dhs_r")
        dhs_i = sbp.tile([128, 2, NFL, N], BF16, name="dhs_i", tag="dhs_i")
        xpr = sbp.tile([128, 4096], BF16, name="xpr", tag="xpr")
        xpi = sbp.tile([128, 4096], BF16, name="xpi", tag="xpi")
        y1p_r = sbp.tile([128, 4096], BF16, name="y1p_r", tag="y1p_r")
        y1p_i = sbp.tile([128, 4096], BF16, name="y1p_i", tag="y1p_i")
        hu_r = sbp.tile([128, 2, N], BF16, name="hu_r", tag="hu_r")
        hu_i = sbp.tile([128, 2, N], BF16, name="hu_i", tag="hu_i")
        hy_r = sbp.tile([128, 2, N], BF16, name="hy_r", tag="hy_r")
        hy_i = sbp.tile([128, 2, N], BF16, name="hy_i", tag="hy_i")
        IT7 = sbp.tile([128, 128], BF16, name="IT7", tag="IT7")
        ITn0 = sbp.tile([128, 128], BF16, name="ITn0", tag="ITn0")
        sc = sbp.tile([128, 8], F32, name="sc", tag="sc")
        scal = sbp.tile([128, 32], F32, name="scal", tag="scal")

        def bc(col):
            return sc[:, col:col + 1]

        # ---------------- init ----------------
        scb = stp.tile([128, 32], BF16, name="scb", tag="scb")
        nc.sync.dma_start(out=scb[:, :], in_=xin[:, 8192:8192 + 32])
        nc.vector.tensor_copy(scal[:, :], scb[:, :])
        # rectify lambdas in place (cols 0:16)
        nc.vector.tensor_relu(scal[:, 0:16], scal[:, 0:16])

        dsr = bigp.tile([128, 2, NFL, N], BF16, name="dsr", tag="bigs", bufs=2)
        dsi = bigp.tile([128, 2, NFL, N], BF16, name="dsi", tag="bigs", bufs=2)
        nc.sync.dma_start(
            out=dsr[:, :, :, :],
            in_=xin[:, 0:4096].rearrange("p (m f y) -> p m f y", m=2, f=NFL))
        nc.sync.dma_start(
            out=dsi[:, :, :, :],
            in_=xin[:, 4096:8192].rearrange("p (m f y) -> p m f y", m=2, f=NFL))
        acc = stp.tile([128, 2], F32, name="acc", tag="acc")
        sq = bigp.tile([128, 2048], F32, name="sq", tag="big", bufs=1)
        nc.scalar.activation(sq[:, :], dsr[:, :, 0:4, :], AF.Square, accum_out=acc[:, 0:1])
        sq_b = bigp.tile([128, 2048], F32, name="sq_b", tag="big", bufs=1)
        nc.scalar.activation(sq_b[:, :], dsr[:, :, 4:8, :], AF.Square, accum_out=acc[:, 1:2])
        acs = stp.tile([128, 2], F32, name="acs", tag="acs")
        nc.vector.tensor_add(acs[:, 0:1], acc[:, 0:1], acc[:, 1:2])
        sq_c = bigp.tile([128, 2048], F32, name="sq_c", tag="big", bufs=1)
        nc.scalar.activation(sq_c[:, :], dsi[:, :, 0:4, :], AF.Square, accum_out=acc[:, 0:1])
        sq_d = bigp.tile([128, 2048], F32, name="sq_d", tag="big", bufs=1)
        nc.scalar.activation(sq_d[:, :], dsi[:, :, 4:8, :], AF.Square, accum_out=acc[:, 1:2])
        nc.vector.tensor_add(acs[:, 1:2], acc[:, 0:1], acc[:, 1:2])
        nc.vector.tensor_add(acs[:, 0:1], acs[:, 0:1], acs[:, 1:2])
        on1 = stp.tile([128, 1], F32, name="on1", tag="on1")
        nc.vector.memset(on1[:, :], 1.0)
        onr = stp.tile([1, 128], F32, name="onr", tag="onr")
        nc.vector.memset(onr[0:1, :], 1.0)
        on64 = stp.tile([64, 1], F32, name="on64", tag="on64")
        nc.vector.memset(on64[:, :], 1.0)

        ps0 = psw.tile([128, 8], F32, name="ps0", tag="pw")
        nc.tensor.matmul(ps0[0:1, 0:1], acs[:, 0:1], on1[:, :], start=True, stop=True)
        nrm_st = stp.tile([1, 1], F32, name="nrm_st", tag="nrm")
        nc.vector.tensor_copy(nrm_st[0:1, :], ps0[0:1, 0:1])
        nc.sync.dma_start(out=nrm_in[:, :], in_=nrm_st[0:1, :])
        coll("AllReduce", ALU.add, RG, [nrm_in.ap()], [nrm_out.ap()])
        nc.sync.dma_start(out=nrm_st[0:1, :], in_=nrm_out[:, :])
        rec = stp.tile([1, 1], F32, name="rec", tag="rec")
        nc.vector.reciprocal(rec[0:1, :], nrm_st[0:1, :])
        nc.scalar.activation(rec[0:1, :], rec[0:1, :], AF.Sqrt)
        ps1 = psw.tile([128, 8], F32, name="ps1", tag="pw")
        nc.tensor.matmul(ps1[:, 0:1], onr[0:1, :], rec[0:1, :], start=True, stop=True)
        nc.vector.tensor_copy(bc(0), ps1[:, 0:1])

        # normalize d in place (fp32), derive pre-scaled bf16 d-hat term for y1 update
        nc.vector.tensor_scalar_mul(dsr[:, :, :, :], dsr[:, :, :, :], bc(0))
        nc.vector.tensor_scalar_mul(dsi[:, :, :, :], dsi[:, :, :, :], bc(0))
        nc.vector.tensor_scalar_mul(dhs_r[:, :, :, :], dsr[:, :, :, :], -C1 * SIGMA)
        nc.vector.tensor_scalar_mul(dhs_i[:, :, :, :], dsi[:, :, :, :], -C1 * SIGMA)

        # b2_out prefill: V0 = c2 * d-hat (bf16) via piece staging
        for p, src in ((0, dsr), (1, dsi)):
            for s in range(NCORE):
                m, q = s // 4, s % 4
                vp = bigp.tile([32, 2, NFL, N], BF16, name=f"v0_{p}_{s}", tag="big", bufs=1)
                nc.vector.tensor_scalar_mul(vp[:, :, :, :],
                                            src[32 * q:32 * (q + 1), :, :, :], C2)
                nc.sync.dma_start(
                    out=b2_out[s, p, :, :, :, :].rearrange("f b k y -> (b k) f y"),
                    in_=vp[:, m, :, :])

        # stage d-hat (bf16) through the frame->pixel A2A to init x-tilde / y1p
        # (u2r/u2i double as the bf16 d-hat staging copies, as in the init ifft)
        NH = NFL // 2

        def stage_b1(dst, src_r, src_i, fbase, fg):
            for m in range(2):
                for p, src in ((0, src_r), (1, src_i)):
                    nc.sync.dma_start(out=dst[4 * m:4 * m + 4, p, fg - fbase, :, :],
                                      in_=src[:, m, fg, :])

        def deposit_b1(src, fbase):
            for p, yt in ((0, y1p_r), (1, y1p_i)):
                for b in range(2):
                    for s_ in range(NCORE):
                        nc.sync.dma_start(
                            out=yt[64 * b + 8 * s_ + fbase:64 * b + 8 * s_ + fbase + NH, :],
                            in_=src[s_, p, :, 16 * b:16 * (b + 1), :]
                                .rearrange("f k y -> f (k y)"))

        nc.vector.tensor_copy(u2r[:, :, :, :], dsr[:, :, :, :])
        nc.vector.tensor_copy(u2i[:, :, :, :], dsi[:, :, :, :])
        for fg in range(NH):
            stage_b1(b1a_in, u2r, u2i, 0, fg)
        coll("AllToAll", ALU.bypass, RG, [b1a_in.ap()], [b1a_out.ap()])
        for fg in range(NH, NFL):
            stage_b1(b1b_in, u2r, u2i, NH, fg)
        coll("AllToAll", ALU.bypass, RG, [b1b_in.ap()], [b1b_out.ap()])
        deposit_b1(b1a_out, 0)
        deposit_b1(b1b_out, NH)
        nc.vector.tensor_copy(xpr[:, :], y1p_r[:, :])
        nc.vector.tensor_copy(xpi[:, :], y1p_i[:, :])

        # per-core scalars: bc(1)=m0, bc(2)=m7, bc(3)=sigma*m7
        nc.vector.tensor_copy(bc(1), scal[:, 16:17])
        nc.vector.tensor_copy(bc(2), scal[:, 17:18])
        nc.vector.tensor_scalar_mul(bc(3), scal[:, 17:18], SIGMA)
        nc.vector.tensor_scalar_mul(IT7[:, :], C["ITb"][:, :], bc(2))
        nc.vector.tensor_scalar_mul(ITn0[:, :], C["ITnb"][:, :], bc(1))

        def fwd_stageA(dst, pr_t, pi_t, s0, s1, li):
            for f in range(NFL):
                for sl in range(2):
                    ps = psp.tile([128, 512], F32, name=f"pA{li}_{f}_{sl}", tag="ps")
                    nc.tensor.matmul(ps[:, :], pr_t[:, 0, f, sl * 128:(sl + 1) * 128],
                                     s0[:, 0, :], start=True, stop=False)
                    nc.tensor.matmul(ps[:, :], pi_t[:, 0, f, sl * 128:(sl + 1) * 128],
                                     s1[:, 0, :], start=False, stop=False)
                    nc.tensor.matmul(ps[:, :], pr_t[:, 1, f, sl * 128:(sl + 1) * 128],
                                     s0[:, 1, :], start=False, stop=False)
                    nc.tensor.matmul(ps[:, :], pi_t[:, 1, f, sl * 128:(sl + 1) * 128],
                                     s1[:, 1, :], start=False, stop=True)
                    nc.scalar.copy(out=dst[:, sl, f, :], in_=ps[:, :])

        def stageB(ps_r, ps_i, Yt, m, fp, inv, stop=False):
            for c in range(2):
                wr = C["Br"][:, c, m * 128:(m + 1) * 128]
                wi = C["Bi"][:, c, m * 128:(m + 1) * 128]
                wn = C["Bn"][:, c, m * 128:(m + 1) * 128]
                rr = Yt[:, c, 2 * fp:2 * fp + 2, 0:256]
                ri = Yt[:, c, 2 * fp:2 * fp + 2, 256:512]
                la = (c == 1) and stop
                if not inv:
                    nc.tensor.matmul(ps_r, wr, rr, start=(c == 0), stop=False)
                    nc.tensor.matmul(ps_r, wn, ri, start=False, stop=la)
                    nc.tensor.matmul(ps_i, wi, rr, start=(c == 0), stop=False)
                    nc.tensor.matmul(ps_i, wr, ri, start=False, stop=la)
                else:
                    nc.tensor.matmul(ps_r, wr, rr, start=(c == 0), stop=False)
                    nc.tensor.matmul(ps_r, wi, ri, start=False, stop=la)
                    nc.tensor.matmul(ps_i, wn, rr, start=(c == 0), stop=False)
                    nc.tensor.matmul(ps_i, wr, ri, start=False, stop=la)

        # startpoint: x2 = u2 = ifft2(d-hat) via bf16 copy of d-hat (in u2 tiles)
        Qt0 = bigp.tile([128, 2, NFL, 512], BF16, name="Qt0", tag="big", bufs=1)
        fwd_stageA(Qt0, u2r, u2i, C["SI0"], C["SI1"], "ini")
        for m in range(2):
            for fp in range(4):
                pr = psp.tile([128, 512], F32, name=f"pi0r{m}{fp}", tag="ps")
                pi = psp.tile([128, 512], F32, name=f"pi0i{m}{fp}", tag="ps")
                stageB(pr[:, :], pi[:, :], Qt0, m, fp, inv=True, stop=True)
                sl2 = (slice(None), m, slice(2 * fp, 2 * fp + 2), slice(None))
                nc.scalar.activation(x2r[sl2], pr[:, :], AF.Copy, scale=-1.0 / TAU)
                nc.scalar.activation(x2i[sl2], pi[:, :], AF.Copy, scale=-1.0 / TAU)
                nc.vector.tensor_scalar_mul(u2r[sl2], pr[:, :], -1.0 / TAU)
                nc.vector.tensor_scalar_mul(u2i[sl2], pi[:, :], -1.0 / TAU)
        for t in (y1r, y1i, y2r, y2i):
            nc.vector.memset(t[:, :, :, :], 0.0)

        eng = nc.sync
        cmu = stp.tile([1, 2], mybir.dt.uint32, name="cmu", tag="cmu")
        nc.sync.dma_start(out=cmu[0:1, :], in_=cmeta_u[:, :])
        r_up = eng.alloc_register("r_up")
        eng.reg_load(r_up, cmu[0:1, 0:1])
        idx_up = eng.snap(r_up, donate=True, min_val=0, max_val=NCORE - 1)
        r_dn = eng.alloc_register("r_dn")
        eng.reg_load(r_dn, cmu[0:1, 1:2])
        idx_dn = eng.snap(r_dn, donate=True, min_val=0, max_val=NCORE - 1)

        def push_u2_halo():
            nc.sync.dma_start(out=hu_in[0, 0, :, :, :], in_=u2r[:, :, 0, :])
            nc.sync.dma_start(out=hu_in[0, 1, :, :, :], in_=u2i[:, :, 0, :])
            coll("AllGather", ALU.bypass, RG, [hu_in.ap()], [hu_out.ap()])
            nc.sync.dma_start(
                out=hu_r[:, :, :],
                in_=hu_out[bass.ds(idx_up, 1), 0, :, :, :].rearrange("o p c y -> (o p) c y"))
            nc.sync.dma_start(
                out=hu_i[:, :, :],
                in_=hu_out[bass.ds(idx_up, 1), 1, :, :, :].rearrange("o p c y -> (o p) c y"))

        push_u2_halo()

        # ---- block-complex helpers: [128,128] f32 tiles hold [[R,-I],[I,R]] ----
        wm = {}

        def newmat(tag, alias=None):
            key = alias or tag
            if key not in wm:
                wm[key] = wp.tile([128, 128], F32, name="wm_" + key, tag="wm_" + key)
            return wm[key]

        def cmm(dst, A, B, nm):
            ps = psw.tile([128, 128], F32, name="cm" + nm, tag="pw")
            nc.tensor.matmul(ps[:, :], A[:, :], B[:, :], start=True, stop=True)
            nc.scalar.copy(out=dst[:, :], in_=ps[:, :])
            return ps

        # ======================= layers =======================
        for li in range(n_layers):
            last = (li == n_layers - 1)

            # ---------- phase K: fwd fft(u2) + y1 update + A2A#1 ----------
            Yt = bigp.tile([128, 2, NFL, 512], BF16, name=f"Yt{li}", tag="big", bufs=1)
            fwd_stageA(Yt, u2r, u2i, C["SA0"], C["SA1"], f"f{li}")
            for fp in range(4):
                b1t = b1a_in if fp < 2 else b1b_in
                fb = 0 if fp < 2 else NH
                for m in range(2):
                    fsl = slice(2 * fp, 2 * fp + 2)
                    sl2 = (slice(None), m, fsl, slice(None))
                    pr = psp.tile([128, 512], F32, name=f"pk_r{li}{m}{fp}", tag="ps")
                    pi = psp.tile([128, 512], F32, name=f"pk_i{li}{m}{fp}", tag="ps")
                    stageB(pr[:, :], pi[:, :], Yt, m, fp, inv=False)
                    vkp_r = rp.tile([128, 2, N], BF16, name=f"vkr{li}{m}{fp}", tag="vkr", bufs=2)
                    vkp_i = rp.tile([128, 2, N], BF16, name=f"vki{li}{m}{fp}", tag="vki", bufs=2)
                    for jf, fg in enumerate(range(2 * fp, 2 * fp + 2)):
                        nc.sync.dma_start(
                            out=vkp_r[:, jf, :],
                            in_=b2_out[4 * m:4 * m + 4, 0, fg, :, :, :]
                                .rearrange("s b k y -> s (b k) y"))
                        nc.sync.dma_start(
                            out=vkp_i[:, jf, :],
                            in_=b2_out[4 * m:4 * m + 4, 1, fg, :, :, :]
                                .rearrange("s b k y -> s (b k) y"))
                    nc.tensor.matmul(pr[:, :], C["Ic1"][:, :], y1r[sl2], start=False, stop=False)
                    nc.tensor.matmul(pr[:, :], C["I1b"][:, :], vkp_r[:, :, :], start=False, stop=False)
                    nc.tensor.matmul(pr[:, :], C["I1b"][:, :], dhs_r[sl2], start=False, stop=True)
                    nc.tensor.matmul(pi[:, :], C["Ic1"][:, :], y1i[sl2], start=False, stop=False)
                    nc.tensor.matmul(pi[:, :], C["I1b"][:, :], vkp_i[:, :, :], start=False, stop=False)
                    nc.tensor.matmul(pi[:, :], C["I1b"][:, :], dhs_i[sl2], start=False, stop=True)
                    nc.vector.tensor_copy(y1r[sl2], pr[:, :])
                    nc.vector.tensor_copy(y1i[sl2], pi[:, :])
                    for jf, fg in enumerate(range(2 * fp, 2 * fp + 2)):
                        nc.sync.dma_start(
                            out=b1t[4 * m:4 * m + 4, 0, fg - fb, :, :],
                            in_=y1r[:, m, fg, :])
                        nc.sync.dma_start(
                            out=b1t[4 * m:4 * m + 4, 1, fg - fb, :, :],
                            in_=y1i[:, m, fg, :])
                if fp == 1:
                    coll("AllToAll", ALU.bypass, RG, [b1a_in.ap()], [b1a_out.ap()])
            coll("AllToAll", ALU.bypass, RG, [b1b_in.ap()], [b1b_out.ap()])

            # ---------- image branch: y2 soft-threshold update ----------
            lnb = stp.tile([128, 1], F32, name=f"lnb{li}", tag="lnb")
            nc.scalar.activation(lnb[:, 0:1], scal[:, li:li + 1], AF.Ln)
            for m in range(2):
                for fp in range(4):
                    fsl = slice(2 * fp, 2 * fp + 2)
                    sl2 = (slice(None), m, fsl, slice(None))
                    af_r = rp.tile([128, 2, N], BF16, name=f"af_r{li}{m}{fp}", tag="afr", bufs=1)
                    af_i = rp.tile([128, 2, N], BF16, name=f"af_i{li}{m}{fp}", tag="afi", bufs=1)
                    mg = rp.tile([128, 2, N], F32, name=f"mg{li}{m}{fp}", tag="mg", bufs=1)
                    mg2 = psm.tile([128, 512], F32, name=f"mg2{li}{m}{fp}", tag="psm")
                    if fp < 3:
                        nc.vector.tensor_sub(af_r[:, :, :], u2r[:, m, 2 * fp + 1:2 * fp + 3, :], u2r[sl2])
                        nc.vector.tensor_sub(af_i[:, :, :], u2i[:, m, 2 * fp + 1:2 * fp + 3, :], u2i[sl2])
                        nc.vector.scalar_tensor_tensor(af_r[:, :, :], af_r[:, :, :], SIGMA,
                                                       y2r[sl2], op0=ALU.mult, op1=ALU.add)
                        nc.vector.scalar_tensor_tensor(af_i[:, :, :], af_i[:, :, :], SIGMA,
                                                       y2i[sl2], op0=ALU.mult, op1=ALU.add)
                    else:
                        nc.vector.tensor_sub(af_r[:, 0, :], u2r[:, m, 7, :], u2r[:, m, 6, :])
                        nc.vector.tensor_sub(af_i[:, 0, :], u2i[:, m, 7, :], u2i[:, m, 6, :])
                        nc.vector.tensor_sub(af_r[:, 1, :], hu_r[:, m, :], u2r[:, m, 7, :])
                        nc.vector.tensor_sub(af_i[:, 1, :], hu_i[:, m, :], u2i[:, m, 7, :])
                        nc.vector.scalar_tensor_tensor(af_r[:, 0, :], af_r[:, 0, :], SIGMA,
                                                       y2r[:, m, 6, :], op0=ALU.mult, op1=ALU.add)
                        nc.vector.scalar_tensor_tensor(af_i[:, 0, :], af_i[:, 0, :], SIGMA,
                                                       y2i[:, m, 6, :], op0=ALU.mult, op1=ALU.add)
                        nc.vector.scalar_tensor_tensor(af_r[:, 1, :], af_r[:, 1, :], bc(3),
                                                       y2r[:, m, 7, :], op0=ALU.mult, op1=ALU.add)
                        nc.vector.scalar_tensor_tensor(af_i[:, 1, :], af_i[:, 1, :], bc(3),
                                                       y2i[:, m, 7, :], op0=ALU.mult, op1=ALU.add)
                    # y2 = af * min(1, lamS / |af|), via exp(ln(lamS) - 0.5*ln(|af|^2))
                    nc.vector.tensor_mul(mg[:, :, :], af_r[:, :, :], af_r[:, :, :])
                    nc.scalar.activation(mg2[:, :], af_i[:, :, :], AF.Square)
                    nc.vector.tensor_add(mg[:, :, :].rearrange("p a b -> p (a b)"),
                                         mg[:, :, :].rearrange("p a b -> p (a b)"), mg2[:, :])
                    nc.scalar.activation(mg[:, :, :], mg[:, :, :], AF.Ln)
                    nc.scalar.activation(mg[:, :, :], mg[:, :, :], AF.Exp,
                                         bias=lnb[:, 0:1], scale=-0.5)
                    nc.vector.tensor_scalar_min(mg[:, :, :], mg[:, :, :], 1.0)
                    nc.vector.tensor_mul(y2r[sl2], af_r[:, :, :], mg[:, :, :])
                    nc.vector.tensor_mul(y2i[sl2], af_i[:, :, :], mg[:, :, :])
            # y2 halo AG
            nc.sync.dma_start(out=hy_in[0, 0, :, :, :], in_=y2r[:, :, 7, :])
            nc.sync.dma_start(out=hy_in[0, 1, :, :, :], in_=y2i[:, :, 7, :])
            coll("AllGather", ALU.bypass, RG, [hy_in.ap()], [hy_out.ap()])
            nc.sync.dma_start(
                out=hy_r[:, :, :],
                in_=hy_out[bass.ds(idx_dn, 1), 0, :, :, :].rearrange("o p c y -> (o p) c y"))
            nc.sync.dma_start(
                out=hy_i[:, :, :],
                in_=hy_out[bass.ds(idx_dn, 1), 1, :, :, :].rearrange("o p c y -> (o p) c y"))

            # ---------- pixel side: deposit, argg1k, Gram ----------
            deposit_b1(b1a_out, 0)
            deposit_b1(b1b_out, NH)
            nc.vector.scalar_tensor_tensor(y1p_r[:, :], y1p_r[:, :], -TAU, xpr[:, :],
                                           op0=ALU.mult, op1=ALU.add)
            nc.vector.scalar_tensor_tensor(y1p_i[:, :], y1p_i[:, :], -TAU, xpi[:, :],
                                           op0=ALU.mult, op1=ALU.add)
            psG = psw.tile([64, 128], F32, name=f"psG{li}", tag="pw")
            for k in range(32):
                ks = slice(128 * k, 128 * (k + 1))
                psT = psp.tile([128, 512], BF16, name=f"psT{li}_{k}", tag="ps")
                nc.tensor.transpose(psT[:, 0:128], y1p_r[:, ks], C["I128b"][:, :])
                nc.tensor.transpose(psT[:, 128:256], y1p_i[:, ks], C["I128b"][:, :])
                Tk = tkp.tile([128, 3, 128], BF16, name=f"Tk{li}_{k}", tag="Tk")
                nc.vector.tensor_copy(Tk[:, 0:2, :], psT[:, 0:256])
                nc.vector.tensor_scalar_mul(Tk[:, 2, :], psT[:, 0:128], -1.0)
                for b in range(2):
                    bs = slice(64 * b, 64 * (b + 1))
                    nc.tensor.matmul(psG[:, :], Tk[:, 0, bs], Tk[:, 0:2, bs],
                                     start=(k == 0 and b == 0), stop=False)
                    nc.tensor.matmul(psG[:, :], Tk[:, 1, bs], Tk[:, 1:3, bs],
                                     start=False, stop=(k == 31 and b == 1))
            gl = stp.tile([64, 128], F32, name=f"gl{li}", tag="gl")
            nc.vector.tensor_copy(gl[:, :], psG[:, :])
            nc.sync.dma_start(out=g_in[:, :], in_=gl[:, :])
            coll("AllReduce", ALU.add, RG, [g_in.ap()], [g_out.ap()])
            glr = stp.tile([64, 128], F32, name=f"glr{li}", tag="glr")
            nc.sync.dma_start(out=glr[:, :], in_=g_out[:, :])
            # block-complex G: [[R, -I], [I, R]]
            G = newmat("G")
            nc.vector.tensor_copy(G[0:64, 0:64], glr[:, 0:64])
            nc.vector.tensor_copy(G[64:128, 64:128], glr[:, 0:64])
            nc.vector.tensor_copy(G[64:128, 0:64], glr[:, 64:128])
            nc.vector.tensor_scalar_mul(G[0:64, 64:128], glr[:, 64:128], -1.0)

            # ---------- W chain (block-complex) ----------
            trs = stp.tile([64, 1], F32, name=f"trs{li}", tag="trs")
            scrap = stp.tile([64, 64], F32, name=f"scrap{li}", tag="scrap")
            lg = stp.tile([1, 4], F32, name=f"lg{li}", tag="lg")
            rtr = stp.tile([1, 1], F32, name=f"rtr{li}", tag="rtr")
            bres = stp.tile([128, 1], F32, name=f"bres{li}", tag="bres")
            psb = psw.tile([128, 8], F32, name=f"psb{li}", tag="pw")
            pst = psw.tile([1, 8], F32, name=f"pst{li}", tag="pw")

            def trace_of(Mt, dstcol, li=li, scrap=scrap, trs=trs, pst=pst):
                nc.vector.scalar_tensor_tensor(scrap[:, :], Mt[0:64, 0:64], 1.0,
                                               C["I64"][:, :], op0=ALU.mult,
                                               op1=ALU.mult, accum_out=trs[:, 0:1])
                nc.tensor.matmul(pst[0:1, dstcol:dstcol + 1], trs[:, :], on64[:, :],
                                 start=True, stop=True)

            def bcast128(src_ap, dst, col, psb=psb):
                nc.tensor.matmul(psb[:, col:col + 1], onr[0:1, :], src_ap,
                                 start=True, stop=True)
                nc.vector.tensor_copy(dst[:, 0:1], psb[:, col:col + 1])

            # lambda-max estimate via M_POWER trace-normalized squarings; all
            # intermediates stay O(1) (HW transcendental/matmul range safety).
            Bm = newmat("Bm")
            B2 = newmat("B2")
            trace_of(G, 0)
            nc.vector.tensor_copy(lg[0:1, 0:1], pst[0:1, 0:1])
            nc.scalar.activation(lg[0:1, 1:2], lg[0:1, 0:1], AF.Ln)
            nc.vector.reciprocal(rtr[0:1, :], lg[0:1, 0:1])
            bcast128(rtr[0:1, :], bres, 0)
            nc.vector.tensor_scalar_mul(Bm[:, :], G[:, :], bres[:, 0:1])
            for it in range(M_POWER):
                cmm(B2, Bm, Bm, f"q{li}_{it}")
                trace_of(B2, 1)
                nc.vector.tensor_copy(lg[0:1, 2:3], pst[0:1, 1:2])
                nc.scalar.activation(lg[0:1, 3:4], lg[0:1, 2:3], AF.Ln)
                nc.vector.tensor_scalar(lg[0:1, 1:2], lg[0:1, 1:2], 2.0, None, op0=ALU.mult)
                nc.vector.tensor_add(lg[0:1, 1:2], lg[0:1, 1:2], lg[0:1, 3:4])
                nc.vector.reciprocal(rtr[0:1, :], lg[0:1, 2:3])
                bcast128(rtr[0:1, :], bres, 1)
                nc.vector.tensor_scalar_mul(Bm[:, :], B2[:, :], bres[:, 0:1])
            lam_s = stp.tile([1, 1], F32, name=f"lam_s{li}", tag="lam_s")
            nc.scalar.activation(lam_s[0:1, :], lg[0:1, 1:2], AF.Exp,
                                 scale=1.0 / (2 ** M_POWER))
            ilam = stp.tile([128, 1], F32, name=f"ilam{li}", tag="ilam")
            nc.vector.reciprocal(rtr[0:1, :], lam_s[0:1, :])
            bcast128(rtr[0:1, :], ilam, 2)
            Gh = newmat("Gh")
            nc.vector.tensor_scalar_mul(Gh[:, :], G[:, :], ilam[:, 0:1])
            t2s = stp.tile([1, 1], F32, name=f"t2s{li}", tag="t2s")
            nc.scalar.activation(t2s[0:1, :], scal[0:1, 8 + li:8 + li + 1],
                                 AF.Square, scale=TAU)
            ths = stp.tile([1, 1], F32, name=f"ths{li}", tag="ths")
            nc.vector.tensor_scalar_mul(ths[0:1, 0:1],
                                        scal[0:1, 8 + li:8 + li + 1], TAU)
            onem = stp.tile([1, 1], F32, name=f"onem{li}", tag="onem")
            nc.vector.tensor_scalar(onem[0:1, :], t2s[0:1, :], -1.0, 1.0,
                                    op0=ALU.mult, op1=ALU.add)
            nc.vector.reciprocal(onem[0:1, :], onem[0:1, :])
            i1m = stp.tile([128, 1], F32, name=f"i1m{li}", tag="i1m")
            bcast128(onem[0:1, :], i1m, 3)
            nt2 = stp.tile([128, 1], F32, name=f"nt2{li}", tag="nt2")
            bcast128(t2s[0:1, :], nt2, 4)
            nc.vector.tensor_scalar_mul(nt2[:, 0:1], nt2[:, 0:1], -1.0)
            X = newmat("X")
            nc.vector.scalar_tensor_tensor(X[:, :], C["I128f"][:, :], nt2[:, 0:1],
                                           Gh[:, :], op0=ALU.mult, op1=ALU.add)
            nc.vector.tensor_scalar_mul(X[:, :], X[:, :], i1m[:, 0:1])
            X2 = newmat("X2", alias="Bm")
            X4 = newmat("X4", alias="B2")
            Yp = newmat("Yp", alias="B2")
            for k_, (a_, b_, c_) in enumerate(SIGN_COEFFS):
                cmm(X2, X, X, f"s2_{li}_{k_}")
                cmm(X4, X2, X2, f"s4_{li}_{k_}")
                nc.vector.tensor_scalar_mul(Yp[:, :], X4[:, :], c_)
                nc.vector.scalar_tensor_tensor(Yp[:, :], X2[:, :], b_,
                                               Yp[:, :], op0=ALU.mult, op1=ALU.add)
                nc.vector.scalar_tensor_tensor(Yp[:, :], C["I128f"][:, :], a_,
                                               Yp[:, :], op0=ALU.mult, op1=ALU.add)
                cmm(X, X, Yp, f"sx_{li}_{k_}")
            P = newmat("P", alias="Bm")
            nc.vector.tensor_scalar_mul(P[:, :], X[:, :], 0.5)
            nc.vector.scalar_tensor_tensor(P[:, :], C["I128f"][:, :], 0.5,
                                           P[:, :], op0=ALU.mult, op1=ALU.add)
            T = newmat("T")
            nc.vector.tensor_scalar_mul(T[:, :], Gh[:, :], 2.0 / Q_HI)
            nc.vector.scalar_tensor_tensor(T[:, :], C["I128f"][:, :], -1.0,
                                           T[:, :], op0=ALU.mult, op1=ALU.add)
            b1m = newmat("b1m", alias="G")
            b2m = newmat("b2m", alias="X")
            tm = newmat("tm", alias="B2")
            nc.vector.memset(b1m[:, :], 0.0)
            nc.vector.memset(b2m[:, :], 0.0)
            mats = [b1m, b2m, tm]
            for ci_idx, ci in enumerate(Q_COEF[::-1][:-1]):
                bb1, bb2, tt = mats
                cmm(tt, T, bb1, f"cl{li}_{ci_idx}")
                nc.vector.scalar_tensor_tensor(tt[:, :], tt[:, :], 2.0,
                                               bb2[:, :], op0=ALU.mult,
                                               op1=ALU.subtract)
                nc.vector.scalar_tensor_tensor(tt[:, :], C["I128f"][:, :], ci,
                                               tt[:, :], op0=ALU.mult, op1=ALU.add)
                mats = [tt, bb1, bb2]
            bb1, bb2, _ = mats
            Q = newmat("Q", alias="Gh")
            cmm(Q, T, bb1, f"qf{li}")
            nc.vector.tensor_sub(Q[:, :], Q[:, :], bb2[:, :])
            nc.vector.scalar_tensor_tensor(Q[:, :], C["I128f"][:, :], Q_COEF[0],
                                           Q[:, :], op0=ALU.mult, op1=ALU.add)
            PQ = newmat("PQ", alias="X")
            cmm(PQ, P, Q, f"pq{li}")
            Wt = newmat("Wt", alias="B2")
            nth = stp.tile([128, 1], F32, name=f"nth{li}", tag="nth")
            bcast128(ths[0:1, 0:1], nth, 5)
            nc.vector.tensor_scalar_mul(nth[:, 0:1], nth[:, 0:1], -1.0)
            nc.vector.scalar_tensor_tensor(Wt[:, :], PQ[:, :], nth[:, 0:1],
                                           P[:, :], op0=ALU.mult, op1=ALU.add)
            Wb = wp.tile([128, 128], BF16, name=f"Wb{li}", tag="Wb")
            Wn = wp.tile([128, 128], BF16, name=f"Wn{li}", tag="Wn")
            nc.vector.tensor_copy(Wb[:, :], Wt[:, :])
            nc.vector.tensor_scalar_mul(Wn[:, :], Wt[:, :], -1.0)

            # ---------- recon + V + A2A#2 (merged b halves) ----------
            # block W quadrants: Wb = [[Wr, -Wi], [Wi, Wr]], Wn = -Wb.
            # b=0 rows (parts 0:64): Wr=Wb[0:64,0:64], -Wi=Wb[0:64,64:128], Wi=Wn[0:64,64:128]
            # b=1 rows (parts 64:128): Wr=Wb[64:128,64:128], Wi=Wb[64:128,0:64], -Wi=Wn[64:128,0:64]
            for ch in range(8):
                cs = slice(512 * ch, 512 * (ch + 1))
                pR = psp.tile([128, 512], F32, name=f"pR{li}_{ch}", tag="ps")
                pI = psp.tile([128, 512], F32, name=f"pI{li}_{ch}", tag="ps")
                b0, b1s = slice(0, 64), slice(64, 128)
                nc.tensor.matmul(pR[b0, :], Wb[b0, 0:64], y1p_r[b0, cs], start=True, stop=False)
                nc.tensor.matmul(pR[b0, :], Wb[b0, 64:128], y1p_i[b0, cs], start=False, stop=True)
                nc.tensor.matmul(pR[b1s, :], Wb[b1s, 64:128], y1p_r[b1s, cs], start=True, stop=False)
                nc.tensor.matmul(pR[b1s, :], Wn[b1s, 0:64], y1p_i[b1s, cs], start=False, stop=True)
                nc.tensor.matmul(pI[b0, :], Wb[b0, 0:64], y1p_i[b0, cs], start=True, stop=False)
                nc.tensor.matmul(pI[b0, :], Wn[b0, 64:128], y1p_r[b0, cs], start=False, stop=True)
                nc.tensor.matmul(pI[b1s, :], Wb[b1s, 64:128], y1p_i[b1s, cs], start=True, stop=False)
                nc.tensor.matmul(pI[b1s, :], Wb[b1s, 0:64], y1p_r[b1s, cs], start=False, stop=True)
                vst_r = rp.tile([128, 512], BF16, name=f"v_r{li}{ch}", tag="vsr")
                vst_i = rp.tile([128, 512], BF16, name=f"v_i{li}{ch}", tag="vsi")
                if not last:
                    nc.vector.scalar_tensor_tensor(vst_r[:, :], xpr[:, cs], -0.5,
                                                   pR[:, :], op0=ALU.mult, op1=ALU.add)
                    nc.vector.tensor_scalar_mul(vst_r[:, :], vst_r[:, :], 2.0 * C2)
                    nc.vector.scalar_tensor_tensor(vst_i[:, :], xpi[:, cs], -0.5,
                                                   pI[:, :], op0=ALU.mult, op1=ALU.add)
                    nc.vector.tensor_scalar_mul(vst_i[:, :], vst_i[:, :], 2.0 * C2)
                else:
                    nc.vector.tensor_copy(vst_r[:, :], pR[:, :])
                    nc.vector.tensor_copy(vst_i[:, :], pI[:, :])
                nc.vector.tensor_copy(xpr[:, cs], pR[:, :])
                nc.vector.tensor_copy(xpi[:, cs], pI[:, :])
                for b in range(2):
                    bsl = slice(64 * b, 64 * (b + 1))
                    nc.sync.dma_start(
                        out=b2_in[:, 0, :, b, :, :]
                            .rearrange("d f k y -> d f (k y)")[:, :, cs],
                        in_=vst_r[bsl, :])
                    nc.sync.dma_start(
                        out=b2_in[:, 1, :, b, :, :]
                            .rearrange("d f k y -> d f (k y)")[:, :, cs],
                        in_=vst_i[bsl, :])
            coll("AllToAll", ALU.bypass, RG, [b2_in.ap()], [b2_out.ap()])

            # ---------- image branch: ifft(y1) + x2/u2 update ----------
            Qt = bigp.tile([128, 2, NFL, 512], BF16, name=f"Qt{li}", tag="big2", bufs=1)
            fwd_stageA(Qt, y1r, y1i, C["SI0"], C["SI1"], f"i{li}")
            for m in range(2):
                for fp in [1, 2, 3, 0]:
                    fsl = slice(2 * fp, 2 * fp + 2)
                    sl2 = (slice(None), m, fsl, slice(None))
                    pr = psp.tile([128, 512], F32, name=f"pm_r{li}{m}{fp}", tag="ps")
                    pi = psp.tile([128, 512], F32, name=f"pm_i{li}{m}{fp}", tag="ps")
                    stageB(pr[:, :], pi[:, :], Qt, m, fp, inv=True)
                    for ppp, y2t, x2t, hyt in ((pr, y2r, x2r, hy_r), (pi, y2i, x2i, hy_i)):
                        nc.tensor.matmul(ppp[:, :], C["I1b"][:, :], x2t[sl2],
                                         start=False, stop=False)
                        if fp == 3:
                            nc.tensor.matmul(ppp[:, 0:256], C["ITb"][:, :],
                                             y2t[:, m, 6, :], start=False, stop=False)
                            nc.tensor.matmul(ppp[:, 256:512], IT7[:, :],
                                             y2t[:, m, 7, :], start=False, stop=False)
                        else:
                            nc.tensor.matmul(ppp[:, :], C["ITb"][:, :], y2t[sl2],
                                             start=False, stop=False)
                        if fp == 0:
                            nc.tensor.matmul(ppp[:, 0:256], ITn0[:, :], hyt[:, m, :],
                                             start=False, stop=False)
                            nc.tensor.matmul(ppp[:, 256:512], C["ITnb"][:, :],
                                             y2t[:, m, 0, :], start=False, stop=True)
                        else:
                            nc.tensor.matmul(ppp[:, :], C["ITnb"][:, :],
                                             y2t[:, m, 2 * fp - 1:2 * fp + 1, :],
                                             start=False, stop=True)
                    if not last:
                        nc.vector.scalar_tensor_tensor(u2r[sl2], x2r[sl2], -0.5, pr[:, :],
                                                       op0=ALU.mult, op1=ALU.add)
                        nc.vector.tensor_scalar_mul(u2r[sl2], u2r[sl2], 2.0)
                        nc.vector.scalar_tensor_tensor(u2i[sl2], x2i[sl2], -0.5, pi[:, :],
                                                       op0=ALU.mult, op1=ALU.add)
                        nc.vector.tensor_scalar_mul(u2i[sl2], u2i[sl2], 2.0)
                    nc.vector.tensor_copy(x2r[sl2], pr[:, :])
                    nc.vector.tensor_copy(x2i[sl2], pi[:, :])
            if not last:
                push_u2_halo()

        # ---------------- final ----------------
        xfr = bigp.tile([128, 2, NFL, N], BF16, name="xfr", tag="bigs", bufs=2)
        xfi = bigp.tile([128, 2, NFL, N], BF16, name="xfi", tag="bigs", bufs=2)
        for p, xt in ((0, xfr), (1, xfi)):
            for m in range(2):
                for f in range(NFL):
                    nc.sync.dma_start(
                        out=xt[:, m, f, :],
                        in_=b2_out[4 * m:4 * m + 4, p, f, :, :, :]
                            .rearrange("s b k y -> s (b k) y"))
        Qtf = bigp.tile([128, 2, NFL, 512], BF16, name="Qtf", tag="big", bufs=1)
        fwd_stageA(Qtf, xfr, xfi, C["SI0"], C["SI1"], "fin")
        for m in range(2):
            for fp in range(4):
                pr = psp.tile([128, 512], F32, name=f"pf_r{m}{fp}", tag="ps")
                pi = psp.tile([128, 512], F32, name=f"pf_i{m}{fp}", tag="ps")
                stageB(pr[:, :], pi[:, :], Qtf, m, fp, inv=True, stop=True)
                sl2 = (slice(None), m, slice(2 * fp, 2 * fp + 2), slice(None))
                op_r = bigp.tile([128, 2, N], BF16, name=f"op_r{m}{fp}", tag="bigs", bufs=2)
                op_i = bigp.tile([128, 2, N], BF16, name=f"op_i{m}{fp}", tag="bigs", bufs=2)
                nc.vector.scalar_tensor_tensor(op_r[:, :, :], pr[:, :], -1.0 / TAU,
                                               x2r[sl2], op0=ALU.mult, op1=ALU.add)
                nc.vector.scalar_tensor_tensor(op_i[:, :, :], pi[:, :], -1.0 / TAU,
                                               x2i[sl2], op0=ALU.mult, op1=ALU.add)
                nc.sync.dma_start(
                    out=out[:, 2048 * m + 512 * fp:2048 * m + 512 * fp + 512],
                    in_=op_r[:, :, :])
                nc.sync.dma_start(
                    out=out[:, 4096 + 2048 * m + 512 * fp:4096 + 2048 * m + 512 * fp + 512],
                    in_=op_i[:, :, :])

        stack.close()

    nc.compile()
    return nc


_CACHE = {}


def _get_nc(n_layers=NLAYERS):
    if n_layers not in _CACHE:
        _CACHE[n_layers] = build(n_layers)
    return _CACHE[n_layers]


def host_shard(d_real, d_imag, lambdaS, lambdaL):
    d_r = np.asarray(d_real, np.float32).reshape(NF, N, N)
    d_i = np.asarray(d_imag, np.float32).reshape(NF, N, N)
    dTr = d_r.transpose(0, 2, 1)
    dTi = d_i.transpose(0, 2, 1)
    lamS = np.asarray(lambdaS, np.float32).reshape(NLAYERS)
    lamL = np.asarray(lambdaL, np.float32).reshape(NLAYERS)
    in_maps = []
    for c in range(NCORE):
        fr = slice(8 * c, 8 * c + 8)
        dk_rc = dTr[fr].reshape(NFL, 2, 128, N).transpose(2, 1, 0, 3).reshape(128, 4096)
        dk_ic = dTi[fr].reshape(NFL, 2, 128, N).transpose(2, 1, 0, 3).reshape(128, 4096)
        m0 = 0.0 if c == 0 else 1.0
        m7 = 0.0 if c == NCORE - 1 else 1.0
        srow = np.zeros(32, np.float32)
        srow[0:8] = lamS
        srow[8:16] = lamL
        srow[16] = m0
        srow[17] = m7
        xin = np.empty((128, W_IN), np.float32)
        xin[:, 0:4096] = dk_rc
        xin[:, 4096:8192] = dk_ic
        xin[:, 8192:] = srow[None, :]
        in_maps.append({
            "xin": _to_bf16(xin),
            "cmeta_u": np.array([[min(c + 1, NCORE - 1), max(c - 1, 0)]], np.uint32),
        })
    return in_maps


def _to_bf16(a):
    import ml_dtypes
    return a.astype(ml_dtypes.bfloat16)


def host_gather(results):
    full = np.zeros((NF, N, N), np.complex64)
    for c, res in enumerate(results):
        o = np.asarray(res["out"], np.float32)
        img = (o[:, 0:4096] + 1j * o[:, 4096:8192]).astype(np.complex64)
        img = img.reshape(128, 2, NFL, N)
        full[8 * c:8 * c + 8] = img.transpose(2, 1, 0, 3).reshape(NFL, N, N)
    return full.reshape(1, 1, NF, N, N)


def kernel(d_real, d_imag, lambdaS, lambdaL):
    nc = _get_nc()
    in_maps = host_shard(d_real, d_imag, lambdaS, lambdaL)
    res = bass_utils.run_bass_kernel_spmd(nc, in_maps, core_ids=list(range(NCORE)))
    return host_gather(res.results)


# revision 30
# speedup vs baseline: 14.1932x; 1.0402x over previous
"""TRN2 Bass kernel: 8-layer Chambolle-Pock MRI reconstruction on 8 NeuronCores.

Sharding: frames (8/core) for FFTs + elementwise; k-space rows (8192 px/core)
for the low-rank prox (Gram trick + matmul-only spectral filter, no eigensolver).
Cross-core per layer: AllToAll (y1 frame->pixel), AllToAll (x1k V pixel->frame),
AllReduce (64x64 Gram), 2 small AllGathers (temporal halo).

v2: single packed input/output tensor (cuts per-call dispatch cost), pixel-side
init derived on device via an extra A2A (drops the dp input), d-hat held in SBUF
pre-scaled (no per-layer HBM reloads), bf16 DFT intermediates, rsqrt-based
soft-threshold, block-complex (128x128 real) W chain, merged-b recon PSUM,
and multi-dim-AP batched DMAs.
"""
import numpy as np

import concourse.bass as bass
import concourse.bacc as bacc
import concourse.mybir as mybir
import concourse.tile as tile
from concourse import bass_utils

F32 = mybir.dt.float32
BF16 = mybir.dt.bfloat16
AF = mybir.ActivationFunctionType
ALU = mybir.AluOpType

NCORE = 8
NF = 64
NFL = 8
N = 256
NLAYERS = 8
SIGMA = float(np.float32(1.0 / np.sqrt(8.0)))
TAU = SIGMA
C1 = float(np.float32(1.0 / (1.0 + SIGMA)))
C2 = C1 * SIGMA
M_POWER = 8
SIGN_COEFFS = [(3.4445, -4.7750, 2.0315)] * 7 + [(1.875, -1.25, 0.375)] * 3
RG = [list(range(NCORE))]

W_IN = 2 * 4096 + 32
W_OUT = 2 * 4096


def _fit_q(deg=14, lo=0.068, hi=1.05, npts=6000):
    xs_in = np.linspace(lo, hi, npts)
    xs_out = np.linspace(0, 0.05, 300)

    def cheb(x, d, b=hi):
        t = (2 * x - b) / b
        V = np.zeros((len(x), d + 1))
        V[:, 0] = 1
        if d >= 1:
            V[:, 1] = t
        for k in range(2, d + 1):
            V[:, k] = 2 * t * V[:, k - 1] - V[:, k - 2]
        return V

    Vi = cheb(xs_in, deg)
    Vo = cheb(xs_out, deg)
    A = np.vstack([np.sqrt(xs_in)[:, None] * Vi, 1e-5 * Vo])
    y = np.concatenate([np.ones(npts), np.zeros(len(xs_out))])
    coef, *_ = np.linalg.lstsq(A, y, rcond=None)
    return [float(c) for c in coef], hi


Q_COEF, Q_HI = _fit_q()


def _dft():
    k = np.arange(N)
    W = np.exp(-2j * np.pi * np.outer(k, k) / N)
    return W.real.astype(np.float32), W.imag.astype(np.float32)


def _chunk(a):
    return np.ascontiguousarray(np.stack([a[0:128], a[128:256]], axis=1))


def build(n_layers=NLAYERS, single_core=False):
    nc = bacc.Bacc("TRN2", target_bir_lowering=False, debug=False,
                   num_devices=1 if single_core else NCORE)

    xin = nc.dram_tensor("xin", [128, W_IN], BF16, kind="ExternalInput")
    out = nc.dram_tensor("out", [128, W_OUT], BF16, kind="ExternalOutput")

    Fr, Fi = _dft()
    sA = C1 * SIGMA / N
    sI = -TAU / N
    id128 = np.eye(128, dtype=np.float32)
    consts_np = {
        "SA0": (_chunk(np.concatenate([Fr * sA, Fi * sA], 1)), BF16),
        "SA1": (_chunk(np.concatenate([-Fi * sA, Fr * sA], 1)), BF16),
        "SI0": (_chunk(np.concatenate([Fr * sI, -Fi * sI], 1)), BF16),
        "SI1": (_chunk(np.concatenate([Fi * sI, Fr * sI], 1)), BF16),
        "Br": (_chunk(Fr), BF16),
        "Bi": (_chunk(Fi), BF16),
        "Bn": (_chunk(-Fi), BF16),
        "I128b": (id128, BF16),
        "Ic1": (id128 * C1, BF16),
        "I1b": (id128, BF16),
        "ITb": (id128 * TAU, BF16),
        "ITnb": (id128 * (-TAU), BF16),
        "I128f": (id128, F32),
        "I64": (np.eye(64, dtype=np.float32), F32),
    }
    handles = {}
    for k, (v, dt) in consts_np.items():
        handles[k] = nc.inline_tensor(v.astype(mybir.dt.np(dt)), name="c" + k)

    b1a_in = nc.dram_tensor("b1a_in", [NCORE, 2, NFL // 2, 32, N], BF16)
    b1a_out = nc.dram_tensor("b1a_out", [NCORE, 2, NFL // 2, 32, N], BF16)
    b1b_in = nc.dram_tensor("b1b_in", [NCORE, 2, NFL // 2, 32, N], BF16)
    b1b_out = nc.dram_tensor("b1b_out", [NCORE, 2, NFL // 2, 32, N], BF16)
    b2_in = nc.dram_tensor("b2_in", [NCORE, 2, NFL, 2, 16, N], BF16)
    b2_out = nc.dram_tensor("b2_out", [NCORE, 2, NFL, 2, 16, N], BF16)
    g_in = nc.dram_tensor("g_in", [64, 128], F32)
    g_out = nc.dram_tensor("g_out", [64, 128], F32, addr_space="Shared")
    nrm_in = nc.dram_tensor("nrm_in", [1, 1], F32)
    nrm_out = nc.dram_tensor("nrm_out", [1, 1], F32, addr_space="Shared")
    # halo AG outputs have one spare slot so neighbor reads are ds(partition_id)
    # with a static offset; the out-of-range slot is only read by the boundary
    # core whose halo contribution is masked to zero (m0/m7).
    hu_in = nc.dram_tensor("hu_in", [1, 2, 128, 2, N], BF16)
    hu_out = nc.dram_tensor("hu_out", [NCORE + 1, 2, 128, 2, N], BF16, addr_space="Shared")
    hy_in = nc.dram_tensor("hy_in", [1, 2, 128, 2, N], BF16)
    hy_out = nc.dram_tensor("hy_out", [NCORE + 1, 2, 128, 2, N], BF16, addr_space="Shared")

    with tile.TileContext(nc) as tc:
        import contextlib
        stack = contextlib.ExitStack()

        def pool(name, bufs, space="SBUF"):
            return stack.enter_context(tc.tile_pool(name=name, bufs=bufs, space=space))

        cp = pool("cp", 1)
        sbp = pool("sbp", 1)
        stp = pool("stp", 1)
        rp = pool("rp", 2)
        bigp = pool("bigp", 1)
        tkp = pool("tkp", 2)
        wp = pool("wp", 1)
        psp = pool("psp", 4, space="PSUM")
        psm = pool("psm", 1, space="PSUM")
        psw = pool("psw", 3, space="PSUM")

        def coll(kind, op, replica_groups, ins, outs):
            if single_core:
                nc.sync.dma_start(out=outs[0], in_=ins[0]) if ins[0].size() == outs[0].size() \
                    else nc.sync.dma_start(out=outs[0][0:1], in_=ins[0][0:1])
            else:
                nc.gpsimd.collective_compute(kind, op, replica_groups=replica_groups,
                                             ins=ins, outs=outs)

        C = {}
        for k, (v, dt) in consts_np.items():
            t = cp.tile(list(v.shape), dt, name="k" + k, tag="k" + k)
            if v.ndim == 2:
                nc.sync.dma_start(out=t[:, :], in_=handles[k][:, :])
            else:
                nc.sync.dma_start(out=t[:, :, :], in_=handles[k][:, :, :])
            C[k] = t

        y1r = sbp.tile([128, 2, NFL, N], BF16, name="y1r", tag="y1r")
        y1i = sbp.tile([128, 2, NFL, N], BF16, name="y1i", tag="y1i")
        u2r = sbp.tile([128, 2, NFL, N], BF16, name="u2r", tag="u2r")
        u2i = sbp.tile([128, 2, NFL, N], BF16, name="u2i", tag="u2i")
        y2r = sbp.tile([128, 2, NFL, N], BF16, name="y2r", tag="y2r")
        y2i = sbp.tile([128, 2, NFL, N], BF16, name="y2i", tag="y2i")
        x2r = sbp.tile([128, 2, NFL, N], BF16, name="x2r", tag="x2r")
        x2i = sbp.tile([128, 2, NFL, N], BF16, name="x2i", tag="x2i")
        dhs_r = sbp.tile([128, 2, NFL, N], BF16, name="dhs_r", tag="dhs_r")
        dhs_i = sbp.tile([128, 2, NFL, N], BF16, name="dhs_i", tag="dhs_i")
        xpr = sbp.tile([128, 4096], BF16, name="xpr", tag="xpr")
        xpi = sbp.tile([128, 4096], BF16, name="xpi", tag="xpi")
        y1p_r = sbp.tile([128, 4096], BF16, name="y1p_r", tag="y1p_r")
        y1p_i = sbp.tile([128, 4096], BF16, name="y1p_i", tag="y1p_i")
        hu_r = sbp.tile([128, 2, N], BF16, name="hu_r", tag="hu_r")
        hu_i = sbp.tile([128, 2, N], BF16, name="hu_i", tag="hu_i")
        hy_r = sbp.tile([128, 2, N], BF16, name="hy_r", tag="hy_r")
        hy_i = sbp.tile([128, 2, N], BF16, name="hy_i", tag="hy_i")
        IT7 = sbp.tile([128, 128], BF16, name="IT7", tag="IT7")
        ITn0 = sbp.tile([128, 128], BF16, name="ITn0", tag="ITn0")
        sc = sbp.tile([128, 8], F32, name="sc", tag="sc")
        scal = sbp.tile([128, 32], F32, name="scal", tag="scal")

        def bc(col):
            return sc[:, col:col + 1]

        # ---------------- init ----------------
        scb = stp.tile([128, 32], BF16, name="scb", tag="scb")
        nc.sync.dma_start(out=scb[:, :], in_=xin[:, 8192:8192 + 32])
        nc.vector.tensor_copy(scal[:, :], scb[:, :])
        # rectify lambdas in place (cols 0:16)
        nc.vector.tensor_relu(scal[:, 0:16], scal[:, 0:16])

        dsr = bigp.tile([128, 2, NFL, N], BF16, name="dsr", tag="bigs", bufs=2)
        dsi = bigp.tile([128, 2, NFL, N], BF16, name="dsi", tag="bigs", bufs=2)
        nc.sync.dma_start(
            out=dsr[:, :, :, :],
            in_=xin[:, 0:4096].rearrange("p (m f y) -> p m f y", m=2, f=NFL))
        nc.sync.dma_start(
            out=dsi[:, :, :, :],
            in_=xin[:, 4096:8192].rearrange("p (m f y) -> p m f y", m=2, f=NFL))
        acc = stp.tile([128, 2], F32, name="acc", tag="acc")
        sq = bigp.tile([128, 2048], F32, name="sq", tag="big", bufs=1)
        nc.scalar.activation(sq[:, :], dsr[:, :, 0:4, :], AF.Square, accum_out=acc[:, 0:1])
        sq_b = bigp.tile([128, 2048], F32, name="sq_b", tag="big", bufs=1)
        nc.scalar.activation(sq_b[:, :], dsr[:, :, 4:8, :], AF.Square, accum_out=acc[:, 1:2])
        acs = stp.tile([128, 2], F32, name="acs", tag="acs")
        nc.vector.tensor_add(acs[:, 0:1], acc[:, 0:1], acc[:, 1:2])
        sq_c = bigp.tile([128, 2048], F32, name="sq_c", tag="big", bufs=1)
        nc.scalar.activation(sq_c[:, :], dsi[:, :, 0:4, :], AF.Square, accum_out=acc[:, 0:1])
        sq_d = bigp.tile([128, 2048], F32, name="sq_d", tag="big", bufs=1)
        nc.scalar.activation(sq_d[:, :], dsi[:, :, 4:8, :], AF.Square, accum_out=acc[:, 1:2])
        nc.vector.tensor_add(acs[:, 1:2], acc[:, 0:1], acc[:, 1:2])
        nc.vector.tensor_add(acs[:, 0:1], acs[:, 0:1], acs[:, 1:2])
        on1 = stp.tile([128, 1], F32, name="on1", tag="on1")
        nc.vector.memset(on1[:, :], 1.0)
        onr = stp.tile([1, 128], F32, name="onr", tag="onr")
        nc.vector.memset(onr[0:1, :], 1.0)
        on64 = stp.tile([64, 1], F32, name="on64", tag="on64")
        nc.vector.memset(on64[:, :], 1.0)

        ps0 = psw.tile([128, 8], F32, name="ps0", tag="pw")
        nc.tensor.matmul(ps0[0:1, 0:1], acs[:, 0:1], on1[:, :], start=True, stop=True)
        nrm_st = stp.tile([1, 1], F32, name="nrm_st", tag="nrm")
        nc.vector.tensor_copy(nrm_st[0:1, :], ps0[0:1, 0:1])
        nc.sync.dma_start(out=nrm_in[:, :], in_=nrm_st[0:1, :])
        coll("AllReduce", ALU.add, RG, [nrm_in.ap()], [nrm_out.ap()])
        nc.sync.dma_start(out=nrm_st[0:1, :], in_=nrm_out[:, :])
        rec = stp.tile([1, 1], F32, name="rec", tag="rec")
        nc.vector.reciprocal(rec[0:1, :], nrm_st[0:1, :])
        nc.scalar.activation(rec[0:1, :], rec[0:1, :], AF.Sqrt)
        ps1 = psw.tile([128, 8], F32, name="ps1", tag="pw")
        nc.tensor.matmul(ps1[:, 0:1], onr[0:1, :], rec[0:1, :], start=True, stop=True)
        nc.vector.tensor_copy(bc(0), ps1[:, 0:1])

        # normalize d in place (fp32), derive pre-scaled bf16 d-hat term for y1 update
        nc.vector.tensor_scalar_mul(dsr[:, :, :, :], dsr[:, :, :, :], bc(0))
        nc.vector.tensor_scalar_mul(dsi[:, :, :, :], dsi[:, :, :, :], bc(0))
        nc.vector.tensor_scalar_mul(dhs_r[:, :, :, :], dsr[:, :, :, :], -C1 * SIGMA)
        nc.vector.tensor_scalar_mul(dhs_i[:, :, :, :], dsi[:, :, :, :], -C1 * SIGMA)

        # b2_out prefill: V0 = c2 * d-hat (bf16) via piece staging
        for p, src in ((0, dsr), (1, dsi)):
            for s in range(NCORE):
                m, q = s // 4, s % 4
                vp = bigp.tile([32, 2, NFL, N], BF16, name=f"v0_{p}_{s}", tag="big", bufs=1)
                nc.vector.tensor_scalar_mul(vp[:, :, :, :],
                                            src[32 * q:32 * (q + 1), :, :, :], C2)
                nc.sync.dma_start(
                    out=b2_out[s, p, :, :, :, :].rearrange("f b k y -> (b k) f y"),
                    in_=vp[:, m, :, :])

        # stage d-hat (bf16) through the frame->pixel A2A to init x-tilde / y1p
        # (u2r/u2i double as the bf16 d-hat staging copies, as in the init ifft)
        NH = NFL // 2

        def stage_b1(dst, src_r, src_i, fbase, fg):
            for m in range(2):
                for p, src in ((0, src_r), (1, src_i)):
                    nc.sync.dma_start(out=dst[4 * m:4 * m + 4, p, fg - fbase, :, :],
                                      in_=src[:, m, fg, :])

        def deposit_b1(src, fbase):
            for p, yt in ((0, y1p_r), (1, y1p_i)):
                for b in range(2):
                    for s_ in range(NCORE):
                        nc.sync.dma_start(
                            out=yt[64 * b + 8 * s_ + fbase:64 * b + 8 * s_ + fbase + NH, :],
                            in_=src[s_, p, :, 16 * b:16 * (b + 1), :]
                                .rearrange("f k y -> f (k y)"))

        nc.vector.tensor_copy(u2r[:, :, :, :], dsr[:, :, :, :])
        nc.vector.tensor_copy(u2i[:, :, :, :], dsi[:, :, :, :])
        for fg in range(NH):
            stage_b1(b1a_in, u2r, u2i, 0, fg)
        coll("AllToAll", ALU.bypass, RG, [b1a_in.ap()], [b1a_out.ap()])
        for fg in range(NH, NFL):
            stage_b1(b1b_in, u2r, u2i, NH, fg)
        coll("AllToAll", ALU.bypass, RG, [b1b_in.ap()], [b1b_out.ap()])
        deposit_b1(b1a_out, 0)
        deposit_b1(b1b_out, NH)
        nc.vector.tensor_copy(xpr[:, :], y1p_r[:, :])
        nc.vector.tensor_copy(xpi[:, :], y1p_i[:, :])

        # per-core scalars: bc(1)=m0, bc(2)=m7, bc(3)=sigma*m7
        nc.vector.tensor_copy(bc(1), scal[:, 16:17])
        nc.vector.tensor_copy(bc(2), scal[:, 17:18])
        nc.vector.tensor_scalar_mul(bc(3), scal[:, 17:18], SIGMA)
        nc.vector.tensor_scalar_mul(IT7[:, :], C["ITb"][:, :], bc(2))
        nc.vector.tensor_scalar_mul(ITn0[:, :], C["ITnb"][:, :], bc(1))

        def fwd_stageA(dst, pr_t, pi_t, s0, s1, li):
            for f in range(NFL):
                for sl in range(2):
                    ps = psp.tile([128, 512], F32, name=f"pA{li}_{f}_{sl}", tag="ps")
                    nc.tensor.matmul(ps[:, :], pr_t[:, 0, f, sl * 128:(sl + 1) * 128],
                                     s0[:, 0, :], start=True, stop=False)
                    nc.tensor.matmul(ps[:, :], pi_t[:, 0, f, sl * 128:(sl + 1) * 128],
                                     s1[:, 0, :], start=False, stop=False)
                    nc.tensor.matmul(ps[:, :], pr_t[:, 1, f, sl * 128:(sl + 1) * 128],
                                     s0[:, 1, :], start=False, stop=False)
                    nc.tensor.matmul(ps[:, :], pi_t[:, 1, f, sl * 128:(sl + 1) * 128],
                                     s1[:, 1, :], start=False, stop=True)
                    nc.scalar.copy(out=dst[:, sl, f, :], in_=ps[:, :])

        def stageB(ps_r, ps_i, Yt, m, fp, inv, stop=False):
            for c in range(2):
                wr = C["Br"][:, c, m * 128:(m + 1) * 128]
                wi = C["Bi"][:, c, m * 128:(m + 1) * 128]
                wn = C["Bn"][:, c, m * 128:(m + 1) * 128]
                rr = Yt[:, c, 2 * fp:2 * fp + 2, 0:256]
                ri = Yt[:, c, 2 * fp:2 * fp + 2, 256:512]
                la = (c == 1) and stop
                if not inv:
                    nc.tensor.matmul(ps_r, wr, rr, start=(c == 0), stop=False)
                    nc.tensor.matmul(ps_r, wn, ri, start=False, stop=la)
                    nc.tensor.matmul(ps_i, wi, rr, start=(c == 0), stop=False)
                    nc.tensor.matmul(ps_i, wr, ri, start=False, stop=la)
                else:
                    nc.tensor.matmul(ps_r, wr, rr, start=(c == 0), stop=False)
                    nc.tensor.matmul(ps_r, wi, ri, start=False, stop=la)
                    nc.tensor.matmul(ps_i, wn, rr, start=(c == 0), stop=False)
                    nc.tensor.matmul(ps_i, wr, ri, start=False, stop=la)

        # startpoint: x2 = u2 = ifft2(d-hat) via bf16 copy of d-hat (in u2 tiles)
        Qt0 = bigp.tile([128, 2, NFL, 512], BF16, name="Qt0", tag="big", bufs=1)
        fwd_stageA(Qt0, u2r, u2i, C["SI0"], C["SI1"], "ini")
        for m in range(2):
            for fp in range(4):
                pr = psp.tile([128, 512], F32, name=f"pi0r{m}{fp}", tag="ps")
                pi = psp.tile([128, 512], F32, name=f"pi0i{m}{fp}", tag="ps")
                stageB(pr[:, :], pi[:, :], Qt0, m, fp, inv=True, stop=True)
                sl2 = (slice(None), m, slice(2 * fp, 2 * fp + 2), slice(None))
                nc.scalar.activation(x2r[sl2], pr[:, :], AF.Copy, scale=-1.0 / TAU)
                nc.scalar.activation(x2i[sl2], pi[:, :], AF.Copy, scale=-1.0 / TAU)
                nc.vector.tensor_scalar_mul(u2r[sl2], pr[:, :], -1.0 / TAU)
                nc.vector.tensor_scalar_mul(u2i[sl2], pi[:, :], -1.0 / TAU)
        for t in (y1r, y1i, y2r, y2i):
            nc.vector.memset(t[:, :, :, :], 0.0)

        eng = nc.sync
        pid = eng.partition_id()

        # fill the spare halo slots with finite data once (they are read only by
        # boundary cores whose halo contribution is masked to zero, but NaN*0=NaN)
        nc.sync.dma_start(out=hu_out[NCORE, 0, :, :, :], in_=u2r[:, :, 0, :])
        nc.sync.dma_start(out=hu_out[NCORE, 1, :, :, :], in_=u2i[:, :, 0, :])
        nc.sync.dma_start(out=hy_out[0, 0, :, :, :], in_=y2r[:, :, 7, :])
        nc.sync.dma_start(out=hy_out[0, 1, :, :, :], in_=y2i[:, :, 7, :])

        def push_u2_halo():
            nc.sync.dma_start(out=hu_in[0, 0, :, :, :], in_=u2r[:, :, 0, :])
            nc.sync.dma_start(out=hu_in[0, 1, :, :, :], in_=u2i[:, :, 0, :])
            # AG fills slots 0..7; core c reads slot c+1 via static +1 offset
            coll("AllGather", ALU.bypass, RG, [hu_in.ap()],
                 [hu_out[0:NCORE, :, :, :, :]])
            nc.sync.dma_start(
                out=hu_r[:, :, :],
                in_=hu_out[1:NCORE + 1, :, :, :, :][bass.ds(pid, 1), 0, :, :, :]
                    .rearrange("o p c y -> (o p) c y"))
            nc.sync.dma_start(
                out=hu_i[:, :, :],
                in_=hu_out[1:NCORE + 1, :, :, :, :][bass.ds(pid, 1), 1, :, :, :]
                    .rearrange("o p c y -> (o p) c y"))

        push_u2_halo()

        # ---- block-complex helpers: [128,128] f32 tiles hold [[R,-I],[I,R]] ----
        wm = {}

        def newmat(tag, alias=None):
            key = alias or tag
            if key not in wm:
                wm[key] = wp.tile([128, 128], F32, name="wm_" + key, tag="wm_" + key)
            return wm[key]

        def cmm(dst, A, B, nm):
            ps = psw.tile([128, 128], F32, name="cm" + nm, tag="pw")
            nc.tensor.matmul(ps[:, :], A[:, :], B[:, :], start=True, stop=True)
            nc.scalar.copy(out=dst[:, :], in_=ps[:, :])
            return ps

        # ======================= layers =======================
        for li in range(n_layers):
            last = (li == n_layers - 1)

            # ---------- phase K: fwd fft(u2) + y1 update + A2A#1 ----------
            Yt = bigp.tile([128, 2, NFL, 512], BF16, name=f"Yt{li}", tag="big", bufs=1)
            fwd_stageA(Yt, u2r, u2i, C["SA0"], C["SA1"], f"f{li}")
            for fp in range(4):
                b1t = b1a_in if fp < 2 else b1b_in
                fb = 0 if fp < 2 else NH
                for m in range(2):
                    fsl = slice(2 * fp, 2 * fp + 2)
                    sl2 = (slice(None), m, fsl, slice(None))
                    pr = psp.tile([128, 512], F32, name=f"pk_r{li}{m}{fp}", tag="ps")
                    pi = psp.tile([128, 512], F32, name=f"pk_i{li}{m}{fp}", tag="ps")
                    stageB(pr[:, :], pi[:, :], Yt, m, fp, inv=False)
                    vkp_r = rp.tile([128, 2, N], BF16, name=f"vkr{li}{m}{fp}", tag="vkr", bufs=2)
                    vkp_i = rp.tile([128, 2, N], BF16, name=f"vki{li}{m}{fp}", tag="vki", bufs=2)
                    for jf, fg in enumerate(range(2 * fp, 2 * fp + 2)):
                        nc.sync.dma_start(
                            out=vkp_r[:, jf, :],
                            in_=b2_out[4 * m:4 * m + 4, 0, fg, :, :, :]
                                .rearrange("s b k y -> s (b k) y"))
                        nc.sync.dma_start(
                            out=vkp_i[:, jf, :],
                            in_=b2_out[4 * m:4 * m + 4, 1, fg, :, :, :]
                                .rearrange("s b k y -> s (b k) y"))
                    nc.tensor.matmul(pr[:, :], C["Ic1"][:, :], y1r[sl2], start=False, stop=False)
                    nc.tensor.matmul(pr[:, :], C["I1b"][:, :], vkp_r[:, :, :], start=False, stop=False)
                    nc.tensor.matmul(pr[:, :], C["I1b"][:, :], dhs_r[sl2], start=False, stop=True)
                    nc.tensor.matmul(pi[:, :], C["Ic1"][:, :], y1i[sl2], start=False, stop=False)
                    nc.tensor.matmul(pi[:, :], C["I1b"][:, :], vkp_i[:, :, :], start=False, stop=False)
                    nc.tensor.matmul(pi[:, :], C["I1b"][:, :], dhs_i[sl2], start=False, stop=True)
                    nc.vector.tensor_copy(y1r[sl2], pr[:, :])
                    nc.vector.tensor_copy(y1i[sl2], pi[:, :])
                    for jf, fg in enumerate(range(2 * fp, 2 * fp + 2)):
                        nc.sync.dma_start(
                            out=b1t[4 * m:4 * m + 4, 0, fg - fb, :, :],
                            in_=y1r[:, m, fg, :])
                        nc.sync.dma_start(
                            out=b1t[4 * m:4 * m + 4, 1, fg - fb, :, :],
                            in_=y1i[:, m, fg, :])
                if fp == 1:
                    coll("AllToAll", ALU.bypass, RG, [b1a_in.ap()], [b1a_out.ap()])
            coll("AllToAll", ALU.bypass, RG, [b1b_in.ap()], [b1b_out.ap()])

            # ---------- image branch: y2 soft-threshold update ----------
            lnb = stp.tile([128, 1], F32, name=f"lnb{li}", tag="lnb")
            nc.scalar.activation(lnb[:, 0:1], scal[:, li:li + 1], AF.Ln)
            for m in range(2):
                for fp in range(4):
                    fsl = slice(2 * fp, 2 * fp + 2)
                    sl2 = (slice(None), m, fsl, slice(None))
                    af_r = rp.tile([128, 2, N], BF16, name=f"af_r{li}{m}{fp}", tag="afr", bufs=1)
                    af_i = rp.tile([128, 2, N], BF16, name=f"af_i{li}{m}{fp}", tag="afi", bufs=1)
                    mg = rp.tile([128, 2, N], F32, name=f"mg{li}{m}{fp}", tag="mg", bufs=1)
                    mg2 = psm.tile([128, 512], F32, name=f"mg2{li}{m}{fp}", tag="psm")
                    if fp < 3:
                        nc.vector.tensor_sub(af_r[:, :, :], u2r[:, m, 2 * fp + 1:2 * fp + 3, :], u2r[sl2])
                        nc.vector.tensor_sub(af_i[:, :, :], u2i[:, m, 2 * fp + 1:2 * fp + 3, :], u2i[sl2])
                        nc.vector.scalar_tensor_tensor(af_r[:, :, :], af_r[:, :, :], SIGMA,
                                                       y2r[sl2], op0=ALU.mult, op1=ALU.add)
                        nc.vector.scalar_tensor_tensor(af_i[:, :, :], af_i[:, :, :], SIGMA,
                                                       y2i[sl2], op0=ALU.mult, op1=ALU.add)
                    else:
                        nc.vector.tensor_sub(af_r[:, 0, :], u2r[:, m, 7, :], u2r[:, m, 6, :])
                        nc.vector.tensor_sub(af_i[:, 0, :], u2i[:, m, 7, :], u2i[:, m, 6, :])
                        nc.vector.tensor_sub(af_r[:, 1, :], hu_r[:, m, :], u2r[:, m, 7, :])
                        nc.vector.tensor_sub(af_i[:, 1, :], hu_i[:, m, :], u2i[:, m, 7, :])
                        nc.vector.scalar_tensor_tensor(af_r[:, 0, :], af_r[:, 0, :], SIGMA,
                                                       y2r[:, m, 6, :], op0=ALU.mult, op1=ALU.add)
                        nc.vector.scalar_tensor_tensor(af_i[:, 0, :], af_i[:, 0, :], SIGMA,
                                                       y2i[:, m, 6, :], op0=ALU.mult, op1=ALU.add)
                        nc.vector.scalar_tensor_tensor(af_r[:, 1, :], af_r[:, 1, :], bc(3),
                                                       y2r[:, m, 7, :], op0=ALU.mult, op1=ALU.add)
                        nc.vector.scalar_tensor_tensor(af_i[:, 1, :], af_i[:, 1, :], bc(3),
                                                       y2i[:, m, 7, :], op0=ALU.mult, op1=ALU.add)
                    # y2 = af * min(1, lamS / |af|), via exp(ln(lamS) - 0.5*ln(|af|^2))
                    nc.vector.tensor_mul(mg[:, :, :], af_r[:, :, :], af_r[:, :, :])
                    nc.scalar.activation(mg2[:, :], af_i[:, :, :], AF.Square)
                    nc.vector.tensor_add(mg[:, :, :].rearrange("p a b -> p (a b)"),
                                         mg[:, :, :].rearrange("p a b -> p (a b)"), mg2[:, :])
                    nc.scalar.activation(mg[:, :, :], mg[:, :, :], AF.Ln)
                    nc.scalar.activation(mg[:, :, :], mg[:, :, :], AF.Exp,
                                         bias=lnb[:, 0:1], scale=-0.5)
                    nc.vector.tensor_scalar_min(mg[:, :, :], mg[:, :, :], 1.0)
                    nc.vector.tensor_mul(y2r[sl2], af_r[:, :, :], mg[:, :, :])
                    nc.vector.tensor_mul(y2i[sl2], af_i[:, :, :], mg[:, :, :])
            # y2 halo AG into slots 1..8; core c reads slot c = core c-1's halo
            nc.sync.dma_start(out=hy_in[0, 0, :, :, :], in_=y2r[:, :, 7, :])
            nc.sync.dma_start(out=hy_in[0, 1, :, :, :], in_=y2i[:, :, 7, :])
            coll("AllGather", ALU.bypass, RG, [hy_in.ap()],
                 [hy_out[1:NCORE + 1, :, :, :, :]])
            nc.sync.dma_start(
                out=hy_r[:, :, :],
                in_=hy_out[bass.ds(pid, 1), 0, :, :, :].rearrange("o p c y -> (o p) c y"))
            nc.sync.dma_start(
                out=hy_i[:, :, :],
                in_=hy_out[bass.ds(pid, 1), 1, :, :, :].rearrange("o p c y -> (o p) c y"))

            # ---------- pixel side: deposit, argg1k, Gram ----------
            deposit_b1(b1a_out, 0)
            deposit_b1(b1b_out, NH)
            nc.vector.scalar_tensor_tensor(y1p_r[:, :], y1p_r[:, :], -TAU, xpr[:, :],
                                           op0=ALU.mult, op1=ALU.add)
            nc.vector.scalar_tensor_tensor(y1p_i[:, :], y1p_i[:, :], -TAU, xpi[:, :],
                                           op0=ALU.mult, op1=ALU.add)
            psG = psw.tile([64, 128], F32, name=f"psG{li}", tag="pw")
            for k in range(32):
                ks = slice(128 * k, 128 * (k + 1))
                psT = psp.tile([128, 512], BF16, name=f"psT{li}_{k}", tag="ps")
                nc.tensor.transpose(psT[:, 0:128], y1p_r[:, ks], C["I128b"][:, :])
                nc.tensor.transpose(psT[:, 128:256], y1p_i[:, ks], C["I128b"][:, :])
                Tk = tkp.tile([128, 3, 128], BF16, name=f"Tk{li}_{k}", tag="Tk")
                nc.vector.tensor_copy(Tk[:, 0:2, :], psT[:, 0:256])
                nc.vector.tensor_scalar_mul(Tk[:, 2, :], psT[:, 0:128], -1.0)
                for b in range(2):
                    bs = slice(64 * b, 64 * (b + 1))
                    nc.tensor.matmul(psG[:, :], Tk[:, 0, bs], Tk[:, 0:2, bs],
                                     start=(k == 0 and b == 0), stop=False)
                    nc.tensor.matmul(psG[:, :], Tk[:, 1, bs], Tk[:, 1:3, bs],
                                     start=False, stop=(k == 31 and b == 1))
            gl = stp.tile([64, 128], F32, name=f"gl{li}", tag="gl")
            nc.vector.tensor_copy(gl[:, :], psG[:, :])
            nc.sync.dma_start(out=g_in[:, :], in_=gl[:, :])
            coll("AllReduce", ALU.add, RG, [g_in.ap()], [g_out.ap()])
            glr = stp.tile([64, 128], F32, name=f"glr{li}", tag="glr")
            nc.sync.dma_start(out=glr[:, :], in_=g_out[:, :])
            # block-complex G: [[R, -I], [I, R]]
            G = newmat("G")
            nc.vector.tensor_copy(G[0:64, 0:64], glr[:, 0:64])
            nc.vector.tensor_copy(G[64:128, 64:128], glr[:, 0:64])
            nc.vector.tensor_copy(G[64:128, 0:64], glr[:, 64:128])
            nc.vector.tensor_scalar_mul(G[0:64, 64:128], glr[:, 64:128], -1.0)

            # ---------- W chain (block-complex) ----------
            trs = stp.tile([64, 1], F32, name=f"trs{li}", tag="trs")
            scrap = stp.tile([64, 64], F32, name=f"scrap{li}", tag="scrap")
            lg = stp.tile([1, 4], F32, name=f"lg{li}", tag="lg")
            rtr = stp.tile([1, 1], F32, name=f"rtr{li}", tag="rtr")
            bres = stp.tile([128, 1], F32, name=f"bres{li}", tag="bres")
            psb = psw.tile([128, 8], F32, name=f"psb{li}", tag="pw")
            pst = psw.tile([1, 8], F32, name=f"pst{li}", tag="pw")

            def trace_of(Mt, dstcol, li=li, scrap=scrap, trs=trs, pst=pst):
                nc.vector.scalar_tensor_tensor(scrap[:, :], Mt[0:64, 0:64], 1.0,
                                               C["I64"][:, :], op0=ALU.mult,
                                               op1=ALU.mult, accum_out=trs[:, 0:1])
                nc.tensor.matmul(pst[0:1, dstcol:dstcol + 1], trs[:, :], on64[:, :],
                                 start=True, stop=True)

            def bcast128(src_ap, dst, col, psb=psb):
                nc.tensor.matmul(psb[:, col:col + 1], onr[0:1, :], src_ap,
                                 start=True, stop=True)
                nc.vector.tensor_copy(dst[:, 0:1], psb[:, col:col + 1])

            # lambda-max estimate via M_POWER trace-normalized squarings; all
            # intermediates stay O(1) (HW transcendental/matmul range safety).
            Bm = newmat("Bm")
            B2 = newmat("B2")
            trace_of(G, 0)
            nc.vector.tensor_copy(lg[0:1, 0:1], pst[0:1, 0:1])
            nc.scalar.activation(lg[0:1, 1:2], lg[0:1, 0:1], AF.Ln)
            nc.vector.reciprocal(rtr[0:1, :], lg[0:1, 0:1])
            bcast128(rtr[0:1, :], bres, 0)
            nc.vector.tensor_scalar_mul(Bm[:, :], G[:, :], bres[:, 0:1])
            for it in range(M_POWER):
                cmm(B2, Bm, Bm, f"q{li}_{it}")
                trace_of(B2, 1)
                nc.vector.tensor_copy(lg[0:1, 2:3], pst[0:1, 1:2])
                nc.scalar.activation(lg[0:1, 3:4], lg[0:1, 2:3], AF.Ln)
                nc.vector.tensor_scalar(lg[0:1, 1:2], lg[0:1, 1:2], 2.0, None, op0=ALU.mult)
                nc.vector.tensor_add(lg[0:1, 1:2], lg[0:1, 1:2], lg[0:1, 3:4])
                nc.vector.reciprocal(rtr[0:1, :], lg[0:1, 2:3])
                bcast128(rtr[0:1, :], bres, 1)
                nc.vector.tensor_scalar_mul(Bm[:, :], B2[:, :], bres[:, 0:1])
            lam_s = stp.tile([1, 1], F32, name=f"lam_s{li}", tag="lam_s")
            nc.scalar.activation(lam_s[0:1, :], lg[0:1, 1:2], AF.Exp,
                                 scale=1.0 / (2 ** M_POWER))
            ilam = stp.tile([128, 1], F32, name=f"ilam{li}", tag="ilam")
            nc.vector.reciprocal(rtr[0:1, :], lam_s[0:1, :])
            bcast128(rtr[0:1, :], ilam, 2)
            Gh = newmat("Gh")
            nc.vector.tensor_scalar_mul(Gh[:, :], G[:, :], ilam[:, 0:1])
            t2s = stp.tile([1, 1], F32, name=f"t2s{li}", tag="t2s")
            nc.scalar.activation(t2s[0:1, :], scal[0:1, 8 + li:8 + li + 1],
                                 AF.Square, scale=TAU)
            ths = stp.tile([1, 1], F32, name=f"ths{li}", tag="ths")
            nc.vector.tensor_scalar_mul(ths[0:1, 0:1],
                                        scal[0:1, 8 + li:8 + li + 1], TAU)
            onem = stp.tile([1, 1], F32, name=f"onem{li}", tag="onem")
            nc.vector.tensor_scalar(onem[0:1, :], t2s[0:1, :], -1.0, 1.0,
                                    op0=ALU.mult, op1=ALU.add)
            nc.vector.reciprocal(onem[0:1, :], onem[0:1, :])
            i1m = stp.tile([128, 1], F32, name=f"i1m{li}", tag="i1m")
            bcast128(onem[0:1, :], i1m, 3)
            nt2 = stp.tile([128, 1], F32, name=f"nt2{li}", tag="nt2")
            bcast128(t2s[0:1, :], nt2, 4)
            nc.vector.tensor_scalar_mul(nt2[:, 0:1], nt2[:, 0:1], -1.0)
            X = newmat("X")
            nc.vector.scalar_tensor_tensor(X[:, :], C["I128f"][:, :], nt2[:, 0:1],
                                           Gh[:, :], op0=ALU.mult, op1=ALU.add)
            nc.vector.tensor_scalar_mul(X[:, :], X[:, :], i1m[:, 0:1])
            X2 = newmat("X2", alias="Bm")
            X4 = newmat("X4", alias="B2")
            Yp = newmat("Yp", alias="B2")
            for k_, (a_, b_, c_) in enumerate(SIGN_COEFFS):
                cmm(X2, X, X, f"s2_{li}_{k_}")
                cmm(X4, X2, X2, f"s4_{li}_{k_}")
                nc.vector.tensor_scalar_mul(Yp[:, :], X4[:, :], c_)
                nc.vector.scalar_tensor_tensor(Yp[:, :], X2[:, :], b_,
                                               Yp[:, :], op0=ALU.mult, op1=ALU.add)
                nc.vector.scalar_tensor_tensor(Yp[:, :], C["I128f"][:, :], a_,
                                               Yp[:, :], op0=ALU.mult, op1=ALU.add)
                cmm(X, X, Yp, f"sx_{li}_{k_}")
            P = newmat("P", alias="Bm")
            nc.vector.tensor_scalar_mul(P[:, :], X[:, :], 0.5)
            nc.vector.scalar_tensor_tensor(P[:, :], C["I128f"][:, :], 0.5,
                                           P[:, :], op0=ALU.mult, op1=ALU.add)
            T = newmat("T")
            nc.vector.tensor_scalar_mul(T[:, :], Gh[:, :], 2.0 / Q_HI)
            nc.vector.scalar_tensor_tensor(T[:, :], C["I128f"][:, :], -1.0,
                                           T[:, :], op0=ALU.mult, op1=ALU.add)
            b1m = newmat("b1m", alias="G")
            b2m = newmat("b2m", alias="X")
            tm = newmat("tm", alias="B2")
            nc.vector.memset(b1m[:, :], 0.0)
            nc.vector.memset(b2m[:, :], 0.0)
            mats = [b1m, b2m, tm]
            for ci_idx, ci in enumerate(Q_COEF[::-1][:-1]):
                bb1, bb2, tt = mats
                cmm(tt, T, bb1, f"cl{li}_{ci_idx}")
                nc.vector.scalar_tensor_tensor(tt[:, :], tt[:, :], 2.0,
                                               bb2[:, :], op0=ALU.mult,
                                               op1=ALU.subtract)
                nc.vector.scalar_tensor_tensor(tt[:, :], C["I128f"][:, :], ci,
                                               tt[:, :], op0=ALU.mult, op1=ALU.add)
                mats = [tt, bb1, bb2]
            bb1, bb2, _ = mats
            Q = newmat("Q", alias="Gh")
            cmm(Q, T, bb1, f"qf{li}")
            nc.vector.tensor_sub(Q[:, :], Q[:, :], bb2[:, :])
            nc.vector.scalar_tensor_tensor(Q[:, :], C["I128f"][:, :], Q_COEF[0],
                                           Q[:, :], op0=ALU.mult, op1=ALU.add)
            PQ = newmat("PQ", alias="X")
            cmm(PQ, P, Q, f"pq{li}")
            Wt = newmat("Wt", alias="B2")
            nth = stp.tile([128, 1], F32, name=f"nth{li}", tag="nth")
            bcast128(ths[0:1, 0:1], nth, 5)
            nc.vector.tensor_scalar_mul(nth[:, 0:1], nth[:, 0:1], -1.0)
            nc.vector.scalar_tensor_tensor(Wt[:, :], PQ[:, :], nth[:, 0:1],
                                           P[:, :], op0=ALU.mult, op1=ALU.add)
            Wb = wp.tile([128, 128], BF16, name=f"Wb{li}", tag="Wb")
            Wn = wp.tile([128, 128], BF16, name=f"Wn{li}", tag="Wn")
            nc.vector.tensor_copy(Wb[:, :], Wt[:, :])
            nc.vector.tensor_scalar_mul(Wn[:, :], Wt[:, :], -1.0)

            # ---------- recon + V + A2A#2 (merged b halves) ----------
            # block W quadrants: Wb = [[Wr, -Wi], [Wi, Wr]], Wn = -Wb.
            # b=0 rows (parts 0:64): Wr=Wb[0:64,0:64], -Wi=Wb[0:64,64:128], Wi=Wn[0:64,64:128]
            # b=1 rows (parts 64:128): Wr=Wb[64:128,64:128], Wi=Wb[64:128,0:64], -Wi=Wn[64:128,0:64]
            for ch in range(8):
                cs = slice(512 * ch, 512 * (ch + 1))
                pR = psp.tile([128, 512], F32, name=f"pR{li}_{ch}", tag="ps")
                pI = psp.tile([128, 512], F32, name=f"pI{li}_{ch}", tag="ps")
                b0, b1s = slice(0, 64), slice(64, 128)
                nc.tensor.matmul(pR[b0, :], Wb[b0, 0:64], y1p_r[b0, cs], start=True, stop=False)
                nc.tensor.matmul(pR[b0, :], Wb[b0, 64:128], y1p_i[b0, cs], start=False, stop=True)
                nc.tensor.matmul(pR[b1s, :], Wb[b1s, 64:128], y1p_r[b1s, cs], start=True, stop=False)
                nc.tensor.matmul(pR[b1s, :], Wn[b1s, 0:64], y1p_i[b1s, cs], start=False, stop=True)
                nc.tensor.matmul(pI[b0, :], Wb[b0, 0:64], y1p_i[b0, cs], start=True, stop=False)
                nc.tensor.matmul(pI[b0, :], Wn[b0, 64:128], y1p_r[b0, cs], start=False, stop=True)
                nc.tensor.matmul(pI[b1s, :], Wb[b1s, 64:128], y1p_i[b1s, cs], start=True, stop=False)
                nc.tensor.matmul(pI[b1s, :], Wb[b1s, 0:64], y1p_r[b1s, cs], start=False, stop=True)
                vst_r = rp.tile([128, 512], BF16, name=f"v_r{li}{ch}", tag="vsr")
                vst_i = rp.tile([128, 512], BF16, name=f"v_i{li}{ch}", tag="vsi")
                if not last:
                    nc.vector.scalar_tensor_tensor(vst_r[:, :], xpr[:, cs], -0.5,
                                                   pR[:, :], op0=ALU.mult, op1=ALU.add)
                    nc.vector.tensor_scalar_mul(vst_r[:, :], vst_r[:, :], 2.0 * C2)
                    nc.vector.scalar_tensor_tensor(vst_i[:, :], xpi[:, cs], -0.5,
                                                   pI[:, :], op0=ALU.mult, op1=ALU.add)
                    nc.vector.tensor_scalar_mul(vst_i[:, :], vst_i[:, :], 2.0 * C2)
                else:
                    nc.vector.tensor_copy(vst_r[:, :], pR[:, :])
                    nc.vector.tensor_copy(vst_i[:, :], pI[:, :])
                nc.vector.tensor_copy(xpr[:, cs], pR[:, :])
                nc.vector.tensor_copy(xpi[:, cs], pI[:, :])
                for b in range(2):
                    bsl = slice(64 * b, 64 * (b + 1))
                    nc.sync.dma_start(
                        out=b2_in[:, 0, :, b, :, :]
                            .rearrange("d f k y -> d f (k y)")[:, :, cs],
                        in_=vst_r[bsl, :])
                    nc.sync.dma_start(
                        out=b2_in[:, 1, :, b, :, :]
                            .rearrange("d f k y -> d f (k y)")[:, :, cs],
                        in_=vst_i[bsl, :])
            coll("AllToAll", ALU.bypass, RG, [b2_in.ap()], [b2_out.ap()])

            # ---------- image branch: ifft(y1) + x2/u2 update ----------
            Qt = bigp.tile([128, 2, NFL, 512], BF16, name=f"Qt{li}", tag="big2", bufs=1)
            fwd_stageA(Qt, y1r, y1i, C["SI0"], C["SI1"], f"i{li}")
            for m in range(2):
                for fp in [1, 2, 3, 0]:
                    fsl = slice(2 * fp, 2 * fp + 2)
                    sl2 = (slice(None), m, fsl, slice(None))
                    pr = psp.tile([128, 512], F32, name=f"pm_r{li}{m}{fp}", tag="ps")
                    pi = psp.tile([128, 512], F32, name=f"pm_i{li}{m}{fp}", tag="ps")
                    stageB(pr[:, :], pi[:, :], Qt, m, fp, inv=True)
                    for ppp, y2t, x2t, hyt in ((pr, y2r, x2r, hy_r), (pi, y2i, x2i, hy_i)):
                        nc.tensor.matmul(ppp[:, :], C["I1b"][:, :], x2t[sl2],
                                         start=False, stop=False)
                        if fp == 3:
                            nc.tensor.matmul(ppp[:, 0:256], C["ITb"][:, :],
                                             y2t[:, m, 6, :], start=False, stop=False)
                            nc.tensor.matmul(ppp[:, 256:512], IT7[:, :],
                                             y2t[:, m, 7, :], start=False, stop=False)
                        else:
                            nc.tensor.matmul(ppp[:, :], C["ITb"][:, :], y2t[sl2],
                                             start=False, stop=False)
                        if fp == 0:
                            nc.tensor.matmul(ppp[:, 0:256], ITn0[:, :], hyt[:, m, :],
                                             start=False, stop=False)
                            nc.tensor.matmul(ppp[:, 256:512], C["ITnb"][:, :],
                                             y2t[:, m, 0, :], start=False, stop=True)
                        else:
                            nc.tensor.matmul(ppp[:, :], C["ITnb"][:, :],
                                             y2t[:, m, 2 * fp - 1:2 * fp + 1, :],
                                             start=False, stop=True)
                    if not last:
                        nc.vector.scalar_tensor_tensor(u2r[sl2], x2r[sl2], -0.5, pr[:, :],
                                                       op0=ALU.mult, op1=ALU.add)
                        nc.vector.tensor_scalar_mul(u2r[sl2], u2r[sl2], 2.0)
                        nc.vector.scalar_tensor_tensor(u2i[sl2], x2i[sl2], -0.5, pi[:, :],
                                                       op0=ALU.mult, op1=ALU.add)
                        nc.vector.tensor_scalar_mul(u2i[sl2], u2i[sl2], 2.0)
                    nc.vector.tensor_copy(x2r[sl2], pr[:, :])
                    nc.vector.tensor_copy(x2i[sl2], pi[:, :])
            if not last:
                push_u2_halo()

        # ---------------- final ----------------
        xfr = bigp.tile([128, 2, NFL, N], BF16, name="xfr", tag="bigs", bufs=2)
        xfi = bigp.tile([128, 2, NFL, N], BF16, name="xfi", tag="bigs", bufs=2)
        for p, xt in ((0, xfr), (1, xfi)):
            for m in range(2):
                for f in range(NFL):
                    nc.sync.dma_start(
                        out=xt[:, m, f, :],
                        in_=b2_out[4 * m:4 * m + 4, p, f, :, :, :]
                            .rearrange("s b k y -> s (b k) y"))
        Qtf = bigp.tile([128, 2, NFL, 512], BF16, name="Qtf", tag="big", bufs=1)
        fwd_stageA(Qtf, xfr, xfi, C["SI0"], C["SI1"], "fin")
        for m in range(2):
            for fp in range(4):
                pr = psp.tile([128, 512], F32, name=f"pf_r{m}{fp}", tag="ps")
                pi = psp.tile([128, 512], F32, name=f"pf_i{m}{fp}", tag="ps")
                stageB(pr[:, :], pi[:, :], Qtf, m, fp, inv=True, stop=True)
                sl2 = (slice(None), m, slice(2 * fp, 2 * fp + 2), slice(None))
                op_r = bigp.tile([128, 2, N], BF16, name=f"op_r{m}{fp}", tag="bigs", bufs=2)
                op_i = bigp.tile([128, 2, N], BF16, name=f"op_i{m}{fp}", tag="bigs", bufs=2)
                nc.vector.scalar_tensor_tensor(op_r[:, :, :], pr[:, :], -1.0 / TAU,
                                               x2r[sl2], op0=ALU.mult, op1=ALU.add)
                nc.vector.scalar_tensor_tensor(op_i[:, :, :], pi[:, :], -1.0 / TAU,
                                               x2i[sl2], op0=ALU.mult, op1=ALU.add)
                nc.sync.dma_start(
                    out=out[:, 2048 * m + 512 * fp:2048 * m + 512 * fp + 512],
                    in_=op_r[:, :, :])
                nc.sync.dma_start(
                    out=out[:, 4096 + 2048 * m + 512 * fp:4096 + 2048 * m + 512 * fp + 512],
                    in_=op_i[:, :, :])

        stack.close()

    nc.compile()
    return nc


_CACHE = {}


def _get_nc(n_layers=NLAYERS):
    if n_layers not in _CACHE:
        _CACHE[n_layers] = build(n_layers)
    return _CACHE[n_layers]


def host_shard(d_real, d_imag, lambdaS, lambdaL):
    d_r = np.asarray(d_real, np.float32).reshape(NF, N, N)
    d_i = np.asarray(d_imag, np.float32).reshape(NF, N, N)
    dTr = d_r.transpose(0, 2, 1)
    dTi = d_i.transpose(0, 2, 1)
    lamS = np.asarray(lambdaS, np.float32).reshape(NLAYERS)
    lamL = np.asarray(lambdaL, np.float32).reshape(NLAYERS)
    in_maps = []
    for c in range(NCORE):
        fr = slice(8 * c, 8 * c + 8)
        dk_rc = dTr[fr].reshape(NFL, 2, 128, N).transpose(2, 1, 0, 3).reshape(128, 4096)
        dk_ic = dTi[fr].reshape(NFL, 2, 128, N).transpose(2, 1, 0, 3).reshape(128, 4096)
        m0 = 0.0 if c == 0 else 1.0
        m7 = 0.0 if c == NCORE - 1 else 1.0
        srow = np.zeros(32, np.float32)
        srow[0:8] = lamS
        srow[8:16] = lamL
        srow[16] = m0
        srow[17] = m7
        xin = np.empty((128, W_IN), np.float32)
        xin[:, 0:4096] = dk_rc
        xin[:, 4096:8192] = dk_ic
        xin[:, 8192:] = srow[None, :]
        in_maps.append({"xin": _to_bf16(xin)})
    return in_maps


def _to_bf16(a):
    import ml_dtypes
    return a.astype(ml_dtypes.bfloat16)


def host_gather(results):
    full = np.zeros((NF, N, N), np.complex64)
    for c, res in enumerate(results):
        o = np.asarray(res["out"], np.float32)
        img = (o[:, 0:4096] + 1j * o[:, 4096:8192]).astype(np.complex64)
        img = img.reshape(128, 2, NFL, N)
        full[8 * c:8 * c + 8] = img.transpose(2, 1, 0, 3).reshape(NFL, N, N)
    return full.reshape(1, 1, NF, N, N)


def kernel(d_real, d_imag, lambdaS, lambdaL):
    nc = _get_nc()
    in_maps = host_shard(d_real, d_imag, lambdaS, lambdaL)
    res = bass_utils.run_bass_kernel_spmd(nc, in_maps, core_ids=list(range(NCORE)))
    return host_gather(res.results)
